# revision 1
# baseline (speedup 1.0000x reference)
"""Trainium2 Bass kernel for nn_GRIC_31550829756424 (GCN-attention block).

Data-parallel over batch: 8 batches -> 8 NeuronCores, one full batch per core.
Weights/B_bias replicated (B_bias pre-transposed + bf16-cast on host: weight
repack). All heavy matmuls run as float32r (1 cyc/row at N=512) except the
PV / output-projection matmuls which run bf16.

Self-contained: hardcodes all shapes; imports only the in-container concourse
stack.
"""

import sys

sys.path.insert(0, "/opt/trn_rl_repo")

import numpy as np
import ml_dtypes
from contextlib import ExitStack

import concourse.bass as bass
import concourse.tile as tile
from concourse import bacc
from concourse import mybir
from concourse.bass_utils import run_bass_kernel_spmd
from concourse.masks import make_identity

F32 = mybir.dt.float32
F32R = mybir.dt.float32r
BF16 = mybir.dt.bfloat16
AF = mybir.ActivationFunctionType
OP = mybir.AluOpType
AX = mybir.AxisListType

B = 8
N = 1024
D = 128
HEADS = 8
DV = 128
HD = HEADS * DV  # 1024
P = 128
NT = N // P  # 8 tiles of 128 rows
DK = 1.0 / float(np.sqrt(np.float32(D)))
EPS = 1e-5

_prog_cache = {}


def _ln_free(nc, small, out_ap, in_ap, eps_ap, gb, beb, extra_eps_ap=None):
    """LayerNorm over the free dim (width D) of [P, D] in_ap -> out_ap.

    If extra_eps_ap is given it is used as the bias of the Sqrt (must already
    be eps or eps*rs^2); otherwise eps_ap ([P,1] memset eps) is used.
    """
    s6 = small.tile([P, 6], F32, tag="s6")
    mv = small.tile([P, 2], F32, tag="mv")
    nc.vector.bn_stats(out=s6, in_=in_ap)
    nc.vector.bn_aggr(out=mv, in_=s6)
    std = small.tile([P, 1], F32, tag="std")
    nc.scalar.activation(
        out=std, in_=mv[:, 1:2], func=AF.Sqrt,
        bias=(extra_eps_ap if extra_eps_ap is not None else eps_ap),
    )
    rstd = small.tile([P, 1], F32, tag="rstd")
    nc.vector.reciprocal(out=rstd, in_=std)
    nc.vector.tensor_scalar(
        out=out_ap, in0=in_ap, scalar1=mv[:, 0:1], scalar2=rstd,
        op0=OP.subtract, op1=OP.mult,
    )
    nc.vector.tensor_mul(out=out_ap, in0=out_ap, in1=gb)
    nc.vector.tensor_add(out=out_ap, in0=out_ap, in1=beb)


def _bcast_load(nc, dst, src):
    """DMA-load 1D DRAM vector src [W] replicated across all P partitions of
    dst [P, W]."""
    rep = bass.AP(tensor=src.tensor, offset=src.offset, ap=[[0, P]] + list(src.ap))
    nc.gpsimd.dma_start(out=dst, in_=rep)


def _build_program():
    nc = bacc.Bacc(None)

    h_in = nc.declare_dram_parameter("h", [N, D], F32, isOutput=False)
    a_in = nc.declare_dram_parameter("a", [N, N], F32, isOutput=False)
    bt_in = nc.declare_dram_parameter("bt", [HEADS, N, N], BF16, isOutput=False)
    wq_in = nc.declare_dram_parameter("wq", [D, HD], F32, isOutput=False)
    wk_in = nc.declare_dram_parameter("wk", [D, HD], F32, isOutput=False)
    wv_in = nc.declare_dram_parameter("wv", [D, HD], F32, isOutput=False)
    bqr_in = nc.declare_dram_parameter("bqr", [P, NT], F32, isOutput=False)
    bkr_in = nc.declare_dram_parameter("bkr", [P, NT], F32, isOutput=False)
    bv_in = nc.declare_dram_parameter("bv", [HD], F32, isOutput=False)
    wo_in = nc.declare_dram_parameter("wo", [HD, D], BF16, isOutput=False)
    w1_in = nc.declare_dram_parameter("w1", [D, D], F32, isOutput=False)
    w2_in = nc.declare_dram_parameter("w2", [D, D], F32, isOutput=False)
    b1_in = nc.declare_dram_parameter("b1", [D, 1], F32, isOutput=False)
    b2_in = nc.declare_dram_parameter("b2", [D, 1], F32, isOutput=False)
    g_in = {}
    be_in = {}
    for i in range(4):
        g_in[i] = nc.declare_dram_parameter(f"g{i}", [D], F32, isOutput=False)
        be_in[i] = nc.declare_dram_parameter(f"be{i}", [D], F32, isOutput=False)
    out_dram = nc.declare_dram_parameter("out", [N, D], F32, isOutput=True)

    with tile.TileContext(nc) as tc, ExitStack() as ctx:
        consts = ctx.enter_context(tc.tile_pool(name="consts", bufs=1))
        persist = ctx.enter_context(tc.tile_pool(name="persist", bufs=1))
        small = ctx.enter_context(tc.tile_pool(name="small", bufs=12))
        stg = ctx.enter_context(tc.tile_pool(name="stg", bufs=3))
        psA = ctx.enter_context(
            tc.tile_pool(name="psA", bufs=2, space=bass.MemorySpace.PSUM))
        psT = ctx.enter_context(
            tc.tile_pool(name="psT", bufs=2, space=bass.MemorySpace.PSUM))
        psB = ctx.enter_context(
            tc.tile_pool(name="psB", bufs=4, space=bass.MemorySpace.PSUM))

        # ---- constants -------------------------------------------------
        ident = consts.tile([P, P], F32)
        make_identity(nc, ident)
        omi = consts.tile([P, P], F32)  # 1 - I
        nc.gpsimd.memset(omi, 1.0)
        nc.gpsimd.affine_select(
            out=omi, in_=omi, compare_op=OP.not_equal, fill=0.0,
            base=0, pattern=[[-1, P]], channel_multiplier=1)
        eps_t = consts.tile([P, 1], F32)
        nc.vector.memset(eps_t, EPS)

        gb = {}
        beb = {}
        for i in range(4):
            gb[i] = consts.tile([P, D], F32, name=f"g{i}b", tag=f"g{i}b")
            _bcast_load(nc, gb[i], g_in[i][:])
            beb[i] = consts.tile([P, D], F32, name=f"be{i}b", tag=f"be{i}b")
            _bcast_load(nc, beb[i], be_in[i][:])
        bvb = consts.tile([P, HD], F32)
        _bcast_load(nc, bvb, bv_in[:])

        w_sb = {}
        for nm, t in (("q", wq_in), ("k", wk_in), ("v", wv_in)):
            wstage = stg.tile([P, HD], F32, name=f"w{nm}s", tag="wstage")
            nc.gpsimd.dma_start(out=wstage, in_=t[:, :])
            w_sb[nm] = consts.tile([P, HD], F32R, name=f"w{nm}", tag=f"w{nm}")
            nc.vector.tensor_copy(out=w_sb[nm], in_=wstage)
        bqr = consts.tile([P, NT], F32)
        nc.gpsimd.dma_start(out=bqr, in_=bqr_in[:, :])
        bkr = consts.tile([P, NT], F32)
        nc.gpsimd.dma_start(out=bkr, in_=bkr_in[:, :])
        # W_O as [p, h, d] where partition p = f-within-tile, h = f-tile
        wo_sb = consts.tile([P, HEADS, D], BF16)
        nc.gpsimd.dma_start(
            out=wo_sb, in_=wo_in.rearrange("(hh p) d -> p hh d", p=P))
        w1_stage = stg.tile([P, D], F32, name="w1s", tag="wstage2")
        nc.gpsimd.dma_start(out=w1_stage, in_=w1_in[:, :])
        w1_sb = consts.tile([P, D], F32R, tag="w1")
        nc.vector.tensor_copy(out=w1_sb, in_=w1_stage)
        w2_stage = stg.tile([P, D], F32, name="w2s", tag="wstage2")
        nc.gpsimd.dma_start(out=w2_stage, in_=w2_in[:, :])
        w2_sb = consts.tile([P, D], F32R, tag="w2")
        nc.vector.tensor_copy(out=w2_sb, in_=w2_stage)
        b1_sb = consts.tile([P, 1], F32, tag="b1")
        nc.gpsimd.dma_start(out=b1_sb, in_=b1_in[:, :])
        b2_sb = consts.tile([P, 1], F32, tag="b2")
        nc.gpsimd.dma_start(out=b2_sb, in_=b2_in[:, :])

        # ---- phase A: H load + LN + transpose --------------------------
        h_sb = persist.tile([P, NT, D], F32, tag="h")
        nc.gpsimd.dma_start(out=h_sb, in_=h_in.rearrange("(t p) d -> p t d", p=P))
        hnT = persist.tile([P, N], F32R, tag="hnT")
        for i in range(NT):
            hn_t = stg.tile([P, D], F32, name="hn_t", tag="hn_t")
            _ln_free(nc, small, hn_t, h_sb[:, i, :], eps_t, gb[0], beb[0])
            pt = psT.tile([P, P], F32, tag="pt")
            nc.tensor.transpose(pt, hn_t, ident)
            nc.vector.tensor_copy(out=hnT[:, i * P:(i + 1) * P], in_=pt)

        # ---- persistent attention operands -----------------------------
        qT = persist.tile([P, HEADS, N], BF16, tag="qT")
        kT = persist.tile([P, HEADS, N], BF16, tag="kT")
        vna = persist.tile([P, NT, HEADS, DV + 1], BF16, tag="v")
        nc.vector.memset(vna[:, :, :, DV:DV + 1], 1.0)
        dis = persist.tile([P, NT], F32, tag="dis")

        # ---- phase B: adjacency prep -----------------------------------
        with tc.tile_pool(name="gcn", bufs=1) as gcn_pool:
            ahatTs = gcn_pool.tile([P, NT, N], F32R, tag="ahatTs")
            with tc.tile_pool(name="apool", bufs=2) as apool:
                for i in range(NT):
                    at = apool.tile([P, N], F32, tag="at")
                    nc.gpsimd.dma_start(out=at, in_=a_in[i * P:(i + 1) * P, :])
                    db = at[:, i * P:(i + 1) * P]
                    nc.vector.tensor_mul(out=db, in0=db, in1=omi)
                    nc.vector.tensor_add(out=db, in0=db, in1=ident)
                    rs = small.tile([P, 1], F32, tag="rs")
                    nc.vector.reduce_sum(out=rs, in_=at, axis=AX.X)
                    nc.vector.tensor_scalar_max(out=rs, in0=rs, scalar1=1.0)
                    sq = small.tile([P, 1], F32, tag="sq")
                    nc.scalar.activation(out=sq, in_=rs, func=AF.Sqrt)
                    di = dis[:, i:i + 1]
                    nc.vector.reciprocal(out=di, in_=sq)
                    nc.vector.tensor_scalar_mul(out=at, in0=at, scalar1=di)
                    for j in range(NT):
                        pt = psT.tile([P, P], F32, tag="pt")
                        nc.tensor.transpose(pt, at[:, j * P:(j + 1) * P], ident)
                        nc.vector.tensor_copy(
                            out=ahatTs[:, j, i * P:(i + 1) * P], in_=pt)

            # ---- phase C: GCN for Q, K, V ------------------------------
            with tc.tile_pool(name="xsp", bufs=1) as xsp:
                for w in ("q", "k", "v"):
                    xs = xsp.tile([P, NT, HD], F32R, tag="xs")
                    for i in range(NT):
                        for c in range(2):
                            ps = psA.tile([P, 512], F32, tag="ps")
                            nc.tensor.matmul(
                                ps,
                                hnT[:, i * P:(i + 1) * P],
                                w_sb[w][:, c * 512:(c + 1) * 512],
                                start=True, stop=True)
                            nc.vector.tensor_scalar_mul(
                                out=xs[:, i, c * 512:(c + 1) * 512], in0=ps,
                                scalar1=dis[:, i:i + 1])
                    if w in ("q", "k"):
                        dstT = qT if w == "q" else kT
                        bias = bqr if w == "q" else bkr
                        for hh in range(HEADS):
                            for c in range(2):
                                ps = psA.tile([P, 512], F32, tag="ps")
                                for j in range(NT):
                                    nc.tensor.matmul(
                                        ps,
                                        xs[:, j, hh * P:(hh + 1) * P],
                                        ahatTs[:, j, c * 512:(c + 1) * 512],
                                        start=(j == 0), stop=(j == NT - 1))
                                if w == "q":
                                    nc.vector.tensor_scalar(
                                        out=dstT[:, hh, c * 512:(c + 1) * 512],
                                        in0=ps, scalar1=bias[:, hh:hh + 1],
                                        scalar2=DK, op0=OP.add, op1=OP.mult)
                                else:
                                    nc.vector.tensor_scalar_add(
                                        out=dstT[:, hh, c * 512:(c + 1) * 512],
                                        in0=ps, scalar1=bias[:, hh:hh + 1])
                    else:
                        for i in range(NT):
                            for c in range(2):
                                ps = psA.tile([P, 512], F32, tag="ps")
                                for j in range(NT):
                                    nc.tensor.matmul(
                                        ps,
                                        ahatTs[:, j, i * P:(i + 1) * P],
                                        xs[:, j, c * 512:(c + 1) * 512],
                                        start=(j == 0), stop=(j == NT - 1))
                                nc.vector.tensor_add(
                                    out=vna[:, i, c * 4:(c + 1) * 4, 0:DV],
                                    in0=ps.rearrange("p (a b) -> p a b", a=4),
                                    in1=bvb[:, c * 512:(c + 1) * 512].rearrange(
                                        "p (a b) -> p a b", a=4))

        # ---- phase D: attention per head -------------------------------
        mhcT = persist.tile([P, HEADS, N], BF16, tag="mhcT")
        with tc.tile_pool(name="etp", bufs=2) as etp, \
             tc.tile_pool(name="btp", bufs=3) as btp:
            for hh in range(HEADS):
                et = etp.tile([P, NT, N], BF16, tag="et")
                for j in range(NT):
                    btt = btp.tile([P, N], BF16, tag="bt")
                    nc.gpsimd.dma_start(
                        out=btt, in_=bt_in[hh, j * P:(j + 1) * P, :])
                    for c in range(2):
                        ps = psA.tile([P, 512], F32, tag="ps")
                        nc.tensor.matmul(
                            ps,
                            kT[:, hh, j * P:(j + 1) * P],
                            qT[:, hh, c * 512:(c + 1) * 512],
                            start=True, stop=True)
                        st = stg.tile([P, 512], BF16, tag="st")
                        nc.vector.tensor_add(
                            out=st, in0=ps, in1=btt[:, c * 512:(c + 1) * 512])
                        nc.scalar.activation(
                            out=et[:, j, c * 512:(c + 1) * 512], in_=st,
                            func=AF.Exp)
                for i in range(NT):
                    pm = psB.tile([P, DV + 1], F32, tag="pm")
                    for j in range(NT):
                        nc.tensor.matmul(
                            pm,
                            et[:, j, i * P:(i + 1) * P],
                            vna[:, j, hh, :],
                            start=(j == 0), stop=(j == NT - 1))
                    # LN over d with the exact eps*rs^2 correction; the
                    # rowsum division cancels out of LayerNorm entirely.
                    s6 = small.tile([P, 6], F32, tag="s6")
                    mv = small.tile([P, 2], F32, tag="mv")
                    nc.vector.bn_stats(out=s6, in_=pm[:, 0:DV])
                    nc.vector.bn_aggr(out=mv, in_=s6)
                    rs_sb = small.tile([P, 1], F32, tag="rssb")
                    nc.vector.tensor_copy(out=rs_sb, in_=pm[:, DV:DV + 1])
                    t = small.tile([P, 1], F32, tag="t")
                    nc.vector.tensor_mul(out=t, in0=rs_sb, in1=rs_sb)
                    nc.scalar.mul(out=t, in_=t, mul=EPS)
                    std = small.tile([P, 1], F32, tag="std")
                    nc.scalar.activation(
                        out=std, in_=mv[:, 1:2], func=AF.Sqrt, bias=t)
                    rstd = small.tile([P, 1], F32, tag="rstd")
                    nc.vector.reciprocal(out=rstd, in_=std)
                    mh = stg.tile([P, DV], F32, tag="mh")
                    nc.vector.tensor_scalar(
                        out=mh, in0=pm[:, 0:DV], scalar1=mv[:, 0:1],
                        scalar2=rstd, op0=OP.subtract, op1=OP.mult)
                    nc.vector.tensor_mul(out=mh, in0=mh, in1=gb[1])
                    nc.vector.tensor_add(out=mh, in0=mh, in1=beb[1])
                    pt = psT.tile([P, P], F32, tag="pt")
                    nc.tensor.transpose(pt, mh, ident)
                    nc.vector.tensor_copy(
                        out=mhcT[:, hh, i * P:(i + 1) * P], in_=pt)

        # ---- phase E: output projection + MLP --------------------------
        o_ln = persist.tile([P, NT, D], F32, tag="oln")
        for i in range(NT):
            ps = psB.tile([P, DV + 1], F32, tag="pm")
            for hh in range(HEADS):
                nc.tensor.matmul(
                    ps[:, 0:D],
                    mhcT[:, hh, i * P:(i + 1) * P],
                    wo_sb[:, hh, :],
                    start=(hh == 0), stop=(hh == HEADS - 1))
            orow = stg.tile([P, D], F32, tag="orow")
            nc.vector.tensor_add(out=orow, in0=ps[:, 0:D], in1=h_sb[:, i, :])
            _ln_free(nc, small, o_ln[:, i, :], orow, eps_t, gb[2], beb[2])
        oT = persist.tile([P, N], F32R, tag="oT")
        for i in range(NT):
            pt = psT.tile([P, P], F32, tag="pt")
            nc.tensor.transpose(pt, o_ln[:, i, :], ident)
            nc.vector.tensor_copy(out=oT[:, i * P:(i + 1) * P], in_=pt)

        r1T = persist.tile([P, N], F32R, tag="r1T")
        for c in range(2):
            ps = psA.tile([P, 512], F32, tag="ps")
            nc.tensor.matmul(
                ps, w1_sb,
                oT[:, c * 512:(c + 1) * 512],
                start=True, stop=True)
            nc.scalar.activation(
                out=r1T[:, c * 512:(c + 1) * 512], in_=ps, func=AF.Relu,
                bias=b1_sb)
        r2T = persist.tile([P, N], F32, tag="r2T")
        for c in range(2):
            ps = psA.tile([P, 512], F32, tag="ps")
            nc.tensor.matmul(
                ps, w2_sb,
                r1T[:, c * 512:(c + 1) * 512],
                start=True, stop=True)
            nc.scalar.activation(
                out=r2T[:, c * 512:(c + 1) * 512], in_=ps, func=AF.Relu,
                bias=b2_sb)

        out_sb = persist.tile([P, NT, D], F32, tag="osb")
        for i in range(NT):
            pt = psT.tile([P, P], F32, tag="pt")
            nc.tensor.transpose(pt, r2T[:, i * P:(i + 1) * P], ident)
            r2 = stg.tile([P, D], F32, tag="r2")
            nc.vector.tensor_copy(out=r2, in_=pt)
            ro = stg.tile([P, D], F32, tag="ro")
            _ln_free(nc, small, ro, r2, eps_t, gb[3], beb[3])
            nc.vector.tensor_add(out=out_sb[:, i, :], in0=o_ln[:, i, :], in1=ro)
        nc.gpsimd.dma_start(
            out=out_dram.rearrange("(t p) d -> p t d", p=P), in_=out_sb)

    nc.compile()
    return nc


def _get_program():
    if "nc" not in _prog_cache:
        _prog_cache["nc"] = _build_program()
    return _prog_cache["nc"]


def kernel(**inputs):
    nc = _get_program()
    f32 = np.float32
    bf16 = ml_dtypes.bfloat16

    H = np.asarray(inputs["H"], dtype=f32)
    A = np.asarray(inputs["A"], dtype=f32)
    BT = np.ascontiguousarray(
        np.asarray(inputs["B_bias"], dtype=f32).transpose(0, 2, 1)).astype(bf16)
    base = {
        "bt": BT,
        "wq": np.asarray(inputs["W_Q"], dtype=f32),
        "wk": np.asarray(inputs["W_K"], dtype=f32),
        "wv": np.asarray(inputs["W_V"], dtype=f32),
        "bqr": np.ascontiguousarray(
            np.asarray(inputs["b_Q"], dtype=f32).reshape(NT, P).T),
        "bkr": np.ascontiguousarray(
            np.asarray(inputs["b_K"], dtype=f32).reshape(NT, P).T),
        "bv": np.asarray(inputs["b_V"], dtype=f32),
        "wo": np.asarray(inputs["W_O"], dtype=f32).astype(bf16),
        "w1": np.asarray(inputs["W1"], dtype=f32),
        "w2": np.asarray(inputs["W2"], dtype=f32),
        "b1": np.asarray(inputs["b1"], dtype=f32).reshape(D, 1),
        "b2": np.asarray(inputs["b2"], dtype=f32).reshape(D, 1),
    }
    for i, (g, be) in enumerate(
            (("g0", "be0"), ("g1", "be1"), ("g2", "be2"), ("g3", "be3"))):
        base[f"g{i}"] = np.asarray(inputs[g], dtype=f32)
        base[f"be{i}"] = np.asarray(inputs[be], dtype=f32)

    in_maps = []
    for c in range(B):
        m = dict(base)
        m["h"] = np.ascontiguousarray(H[c])
        m["a"] = np.ascontiguousarray(A[c])
        in_maps.append(m)

    res = run_bass_kernel_spmd(nc, in_maps, list(range(B)))
    out = np.stack([res.results[c]["out"] for c in range(B)], axis=0)
    return out.astype(np.float32)


if __name__ == "__main__":
    _get_program()
    print("program built ok")



# revision 3
# speedup vs baseline: 1.4757x; 1.4757x over previous
"""Trainium2 Bass kernel for nn_GRIC_31550829756424 (GCN-attention block).

Data-parallel over batch: 8 batches -> 8 NeuronCores, one full batch per core.

Key structure (v2):
- GCN reassociated: adjHnT = Hn^T @ adj_norm^T computed once (shared by
  Q/K/V), then Q/K/V are single-step K=128 matmuls.  A is host-transposed
  and bf16; both degree scalings fold into A^T via one scalar_tensor_tensor.
- All rsqrt computed as exp(-0.5*ln(x)) so the ONLY activation table used is
  natural_log_exp_and_others (Exp/Ln/Relu/Copy/Identity) -> 1 table load.
- Attention bias B added into the QK PSUM via an fp8 DoubleRow matmul
  (identity/32 stationary, bias*16 moving) instead of DVE adds.
- LN1 gain/bias folded into W_O on the host; MH transposed via DMA XBAR.

Self-contained: hardcodes all shapes; imports only the in-container concourse
stack.
"""

import sys

sys.path.insert(0, "/opt/trn_rl_repo")

import numpy as np
import ml_dtypes
from contextlib import ExitStack

import concourse.bass as bass
import concourse.tile as tile
from concourse import bacc
from concourse import mybir
from concourse.bass_utils import run_bass_kernel_spmd
from concourse.masks import make_identity

F32 = mybir.dt.float32
BF16 = mybir.dt.bfloat16
F8 = mybir.dt.float8e4
AF = mybir.ActivationFunctionType
OP = mybir.AluOpType
PM = mybir.MatmulPerfMode

B = 8
N = 1024
D = 128
HEADS = 8
DV = 128
HD = HEADS * DV  # 1024
P = 128
NT = N // P  # 8 tiles of 128 rows
DK = 1.0 / float(np.sqrt(np.float32(D)))
EPS = 1e-5

_prog_cache = {}


def _bcast_load(nc, dst, src):
    """DMA-load 1D DRAM vector src [W] replicated across all P partitions of
    dst [P, W] (same dtype)."""
    rep = bass.AP(tensor=src.tensor, offset=src.offset, ap=[[0, P]] + list(src.ap))
    nc.sync.dma_start(out=dst, in_=rep)


def _dup2(ap):
    """View a [P, W] AP as [P, 2, W] with a stride-0 middle dim (DoubleRow
    moving operand reading the same data in both slots)."""
    return bass.AP(
        tensor=ap.tensor, offset=ap.offset,
        ap=[list(ap.ap[0]), [0, 2]] + [list(a) for a in ap.ap[1:]],
    )


def _build_program():
    nc = bacc.Bacc(None)

    h_in = nc.declare_dram_parameter("h", [N, D], BF16, isOutput=False)
    at_in = nc.declare_dram_parameter("at", [N, N], BF16, isOutput=False)
    bt_in = nc.declare_dram_parameter("bt", [HEADS, N, N], F8, isOutput=False)
    wq_in = nc.declare_dram_parameter("wq", [D, HD], BF16, isOutput=False)
    wk_in = nc.declare_dram_parameter("wk", [D, HD], BF16, isOutput=False)
    wv_in = nc.declare_dram_parameter("wv", [D, HD], BF16, isOutput=False)
    bqr_in = nc.declare_dram_parameter("bqr", [P, NT], F32, isOutput=False)
    bkr_in = nc.declare_dram_parameter("bkr", [P, NT], F32, isOutput=False)
    bv_in = nc.declare_dram_parameter("bv", [HD], F32, isOutput=False)
    wo_in = nc.declare_dram_parameter("wo", [HD, D], BF16, isOutput=False)
    bo_in = nc.declare_dram_parameter("bo", [1, D], BF16, isOutput=False)
    w1_in = nc.declare_dram_parameter("w1", [D, D], BF16, isOutput=False)
    w2_in = nc.declare_dram_parameter("w2", [D, D], BF16, isOutput=False)
    b1_in = nc.declare_dram_parameter("b1", [D, 1], F32, isOutput=False)
    b2_in = nc.declare_dram_parameter("b2", [D, 1], F32, isOutput=False)
    g_in = {}
    be_in = {}
    for i in (0, 2, 3):
        g_in[i] = nc.declare_dram_parameter(f"g{i}", [D], BF16, isOutput=False)
        be_in[i] = nc.declare_dram_parameter(f"be{i}", [D], BF16, isOutput=False)
    out_dram = nc.declare_dram_parameter("out", [N, D], F32, isOutput=True)

    with tile.TileContext(nc) as tc, ExitStack() as ctx:
        consts = ctx.enter_context(tc.tile_pool(name="consts", bufs=1))
        persist = ctx.enter_context(tc.tile_pool(name="persist", bufs=1))
        small = ctx.enter_context(tc.tile_pool(name="small", bufs=12))
        stg = ctx.enter_context(tc.tile_pool(name="stg", bufs=3))
        # 2-bank 512-wide PSUM pool: open through B/C, reused in E via scope.
        ps512 = ctx.enter_context(
            tc.tile_pool(name="ps512", bufs=2, space=bass.MemorySpace.PSUM))

        # ---- constants -------------------------------------------------
        identb = consts.tile([P, P], BF16)
        make_identity(nc, identb)
        omib = consts.tile([P, P], BF16)  # 1 - I
        nc.gpsimd.memset(omib, 1.0)
        nc.gpsimd.affine_select(
            out=omib, in_=omib, compare_op=OP.not_equal, fill=0.0,
            base=0, pattern=[[-1, P]], channel_multiplier=1)
        # fp8 DoubleRow stationary: two slots of I/32 (moving is bias*16).
        id2 = consts.tile([P, 2, P], F8)
        nc.gpsimd.memset(id2, 0.0)
        for s in range(2):
            nc.gpsimd.affine_select(
                out=id2[:, s, :], in_=id2[:, s, :], compare_op=OP.not_equal,
                fill=1.0 / 32.0, base=0, pattern=[[-1, P]], channel_multiplier=1)
        eps_t = consts.tile([P, 1], F32)
        nc.vector.memset(eps_t, EPS)
        onesb = consts.tile([1, P], BF16)
        nc.vector.memset(onesb, 1.0)
        onescol = consts.tile([P, 1], BF16)
        nc.vector.memset(onescol, 1.0)

        gb = {}
        beb = {}
        for i in (0, 2, 3):
            gb[i] = consts.tile([P, D], BF16, name=f"g{i}b", tag=f"g{i}b")
            _bcast_load(nc, gb[i], g_in[i][:])
            beb[i] = consts.tile([P, D], BF16, name=f"be{i}b", tag=f"be{i}b")
            _bcast_load(nc, beb[i], be_in[i][:])
        bvb = consts.tile([P, HD], F32)
        _bcast_load(nc, bvb, bv_in[:])
        bqr = consts.tile([P, NT], F32)
        nc.sync.dma_start(out=bqr, in_=bqr_in[:, :])
        bkr = consts.tile([P, NT], F32)
        nc.sync.dma_start(out=bkr, in_=bkr_in[:, :])
        wq_sb = consts.tile([P, HD], BF16)
        nc.sync.dma_start(out=wq_sb, in_=wq_in[:, :])
        wk_sb = consts.tile([P, HD], BF16)
        nc.sync.dma_start(out=wk_sb, in_=wk_in[:, :])
        wv_sb = consts.tile([P, HD], BF16)
        nc.sync.dma_start(out=wv_sb, in_=wv_in[:, :])
        # W_O as [p=dv-within-head, h, d]
        wo_sb = consts.tile([P, HEADS, D], BF16)
        nc.sync.dma_start(
            out=wo_sb, in_=wo_in.rearrange("(hh p) d -> p hh d", p=P))
        bo_sb = consts.tile([1, D], BF16)
        nc.sync.dma_start(out=bo_sb, in_=bo_in[:, :])
        w1_sb = consts.tile([P, D], BF16)
        nc.sync.dma_start(out=w1_sb, in_=w1_in[:, :])
        w2_sb = consts.tile([P, D], BF16)
        nc.sync.dma_start(out=w2_sb, in_=w2_in[:, :])
        b1_sb = consts.tile([P, 1], F32)
        nc.sync.dma_start(out=b1_sb, in_=b1_in[:, :])
        b2_sb = consts.tile([P, 1], F32)
        nc.sync.dma_start(out=b2_sb, in_=b2_in[:, :])

        # ---- persistent tensors ---------------------------------------
        h_sb = persist.tile([P, NT, D], BF16, tag="h")
        nc.sync.dma_start(out=h_sb, in_=h_in.rearrange("(t p) d -> p t d", p=P))
        hn_t = persist.tile([P, NT, D], BF16, tag="hnt")
        ats = persist.tile([P, NT, N], BF16, tag="ats")  # A^T -> adj_norm^T
        disb = persist.tile([P, N], BF16, tag="disb")
        dis_tok = persist.tile([P, NT], F32, tag="distok")
        adjHnT = persist.tile([P, N], BF16, tag="adjhnt")
        qT = persist.tile([P, HEADS, N], BF16, tag="qT")
        kT = persist.tile([P, HEADS, N], BF16, tag="kT")
        vna = persist.tile([P, NT, HEADS, DV + 1], BF16, tag="v")
        nc.vector.memset(vna[:, :, :, DV:DV + 1], 1.0)
        mhcT = persist.tile([P, HEADS, N], BF16, tag="mhcT")

        # ---- phase A: H LayerNorm (token-major) ------------------------
        for i in range(NT):
            s6 = small.tile([P, 6], F32, tag="s6")
            mv = small.tile([P, 2], F32, tag="mv")
            nc.vector.bn_stats(out=s6, in_=h_sb[:, i, :])
            nc.vector.bn_aggr(out=mv, in_=s6)
            lnv = small.tile([P, 1], F32, tag="lnv")
            nc.scalar.activation(out=lnv, in_=mv[:, 1:2], func=AF.Ln, bias=eps_t)
            rstd = small.tile([P, 1], F32, tag="rstd")
            nc.scalar.activation(out=rstd, in_=lnv, func=AF.Exp, scale=-0.5)
            hw = stg.tile([P, D], BF16, name="hw", tag="hw")
            nc.vector.tensor_scalar(
                out=hw, in0=h_sb[:, i, :], scalar1=mv[:, 0:1], scalar2=rstd,
                op0=OP.subtract, op1=OP.mult)
            nc.vector.tensor_mul(out=hw, in0=hw, in1=gb[0])
            nc.vector.tensor_add(out=hn_t[:, i, :], in0=hw, in1=beb[0])

        # ---- phase B: adjacency prep ------------------------------------
        with tc.tile_pool(name="psR", bufs=1, space=bass.MemorySpace.PSUM) as psR, \
             tc.tile_pool(name="psTb", bufs=2, space=bass.MemorySpace.PSUM) as psTb:
            rs_ps = [psR.tile([1, 512], F32, name=f"rsps{c}", tag=f"rsps{c}")
                     for c in range(2)]
            for j in range(NT):
                nc.sync.dma_start(
                    out=ats[:, j, :],
                    in_=at_in[:, :].rearrange("(t p) n -> p t n", p=P)[:, j, :])
                db = ats[:, j, j * P:(j + 1) * P]
                nc.vector.tensor_mul(out=db, in0=db, in1=omib)
                nc.vector.tensor_add(out=db, in0=db, in1=identb)
                for c in range(2):
                    nc.tensor.matmul(
                        rs_ps[c], onescol, ats[:, j, c * 512:(c + 1) * 512],
                        start=(j == 0), stop=(j == NT - 1))
            rs_sb = small.tile([1, N], F32, tag="rssb", bufs=1)
            for c in range(2):
                nc.vector.tensor_scalar_max(
                    out=rs_sb[:, c * 512:(c + 1) * 512], in0=rs_ps[c], scalar1=1.0)
            lnr = small.tile([1, N], F32, tag="lnr", bufs=1)
            nc.scalar.activation(out=lnr, in_=rs_sb, func=AF.Ln)
            disrow = small.tile([1, N], BF16, tag="disrow", bufs=1)
            nc.scalar.activation(out=disrow, in_=lnr, func=AF.Exp, scale=-0.5)
            # broadcast di over partitions: disb[q, n] = di_n
            for c in range(2):
                psd = ps512.tile([P, 512], F32, tag="ps512")
                nc.tensor.matmul(
                    psd, onesb, disrow[:, c * 512:(c + 1) * 512],
                    start=True, stop=True)
                nc.vector.tensor_copy(
                    out=disb[:, c * 512:(c + 1) * 512], in_=psd)
            # dis_tok[p, j] = di_{j*P+p} via PE transpose of disb chunks
            for j in range(NT):
                ptb = psTb.tile([P, P], BF16, tag="ptb")
                nc.tensor.transpose(ptb, disb[:, j * P:(j + 1) * P], identb)
                nc.vector.tensor_copy(out=dis_tok[:, j:j + 1], in_=ptb[:, 0:1])
            # fold both degree scalings into A^T (in place):
            # ats[m, n] = At[m, n] * di_m * di_n
            for j in range(NT):
                nc.vector.scalar_tensor_tensor(
                    out=ats[:, j, :], in0=ats[:, j, :],
                    scalar=dis_tok[:, j:j + 1], in1=disb,
                    op0=OP.mult, op1=OP.mult)

        # ---- phase C: shared GCN trunk + Q/K/V --------------------------
        # adjHnT[d, n] = sum_m hn[m, d] * adjn^T[m, n]
        for c in range(2):
            psc = ps512.tile([P, 512], F32, tag="ps512")
            for j in range(NT):
                nc.tensor.matmul(
                    psc, hn_t[:, j, :], ats[:, j, c * 512:(c + 1) * 512],
                    start=(j == 0), stop=(j == NT - 1))
            nc.vector.tensor_copy(out=adjHnT[:, c * 512:(c + 1) * 512], in_=psc)
        # V first so heads 0-3 unblock early
        for c in range(2):
            for i in range(NT):
                psc = ps512.tile([P, 512], F32, tag="ps512")
                nc.tensor.matmul(
                    psc, adjHnT[:, i * P:(i + 1) * P],
                    wv_sb[:, c * 512:(c + 1) * 512], start=True, stop=True)
                nc.vector.tensor_add(
                    out=vna[:, i, c * 4:(c + 1) * 4, 0:DV],
                    in0=psc.rearrange("p (a b) -> p a b", a=4),
                    in1=bvb[:, c * 512:(c + 1) * 512].rearrange(
                        "p (a b) -> p a b", a=4))
        for hh in range(HEADS):
            for c in range(2):
                psc = ps512.tile([P, 512], F32, tag="ps512")
                nc.tensor.matmul(
                    psc, wq_sb[:, hh * P:(hh + 1) * P],
                    adjHnT[:, c * 512:(c + 1) * 512], start=True, stop=True)
                nc.vector.tensor_scalar_add(
                    out=qT[:, hh, c * 512:(c + 1) * 512], in0=psc,
                    scalar1=bqr[:, hh:hh + 1])
                psc = ps512.tile([P, 512], F32, tag="ps512")
                nc.tensor.matmul(
                    psc, wk_sb[:, hh * P:(hh + 1) * P],
                    adjHnT[:, c * 512:(c + 1) * 512], start=True, stop=True)
                nc.vector.tensor_scalar_add(
                    out=kT[:, hh, c * 512:(c + 1) * 512], in0=psc,
                    scalar1=bkr[:, hh:hh + 1])

        # ---- phase D: attention per head --------------------------------
        with tc.tile_pool(name="psE", bufs=2, space=bass.MemorySpace.PSUM) as psE, \
             tc.tile_pool(name="psPM", bufs=2, space=bass.MemorySpace.PSUM) as psPM, \
             tc.tile_pool(name="etp", bufs=2) as etp, \
             tc.tile_pool(name="btp", bufs=2) as btp, \
             tc.tile_pool(name="mhp", bufs=2) as mhp:
            for hh in range(HEADS):
                btile = btp.tile([P, NT, N], F8, tag="bt")
                nc.sync.dma_start(
                    out=btile,
                    in_=bt_in[hh].rearrange("(t p) n -> p t n", p=P))
                et = etp.tile([P, NT, N], BF16, tag="et")
                for j in range(NT):
                    pse = psE.tile([P, N], F32, tag="pse")
                    for c in range(2):
                        nc.tensor.matmul(
                            pse[:, c * 512:(c + 1) * 512],
                            kT[:, hh, j * P:(j + 1) * P],
                            qT[:, hh, c * 512:(c + 1) * 512],
                            start=True, stop=False)
                        nc.tensor.matmul(
                            pse[:, c * 512:(c + 1) * 512],
                            id2, _dup2(btile[:, j, c * 512:(c + 1) * 512]),
                            start=False, stop=True, perf_mode=PM.DoubleRow)
                    nc.scalar.activation(out=et[:, j, :], in_=pse, func=AF.Exp)
                mh_head = mhp.tile([P, NT, DV], BF16, tag="mh")
                # PV in pairs of query tiles; LN rstd batched per pair.
                for i0 in range(0, NT, 2):
                    pms = []
                    mvs = small.tile([P, 2, 2], F32, tag="mvs", bufs=4)
                    vee = small.tile([P, 2], F32, tag="vee", bufs=4)
                    for k in range(2):
                        i = i0 + k
                        pm = psPM.tile([P, DV + 1], F32, tag="pm")
                        pms.append(pm)
                        for j in range(NT):
                            nc.tensor.matmul(
                                pm, et[:, j, i * P:(i + 1) * P],
                                vna[:, j, hh, :],
                                start=(j == 0), stop=(j == NT - 1))
                        s6 = small.tile([P, 6], F32, tag="s6")
                        nc.vector.bn_stats(out=s6, in_=pm[:, 0:DV])
                        nc.vector.bn_aggr(out=mvs[:, k, :], in_=s6)
                        # vee = var + eps * rowsum^2
                        t = small.tile([P, 1], F32, tag="t")
                        nc.vector.tensor_scalar(
                            out=t, in0=pm[:, DV:DV + 1],
                            scalar1=pm[:, DV:DV + 1], scalar2=EPS,
                            op0=OP.mult, op1=OP.mult)
                        nc.vector.tensor_add(
                            out=vee[:, k:k + 1], in0=t, in1=mvs[:, k, 1:2])
                    lnv2 = small.tile([P, 2], F32, tag="lnv2", bufs=4)
                    nc.scalar.activation(out=lnv2, in_=vee, func=AF.Ln)
                    rstd2 = small.tile([P, 2], F32, tag="rstd2", bufs=4)
                    nc.scalar.activation(out=rstd2, in_=lnv2, func=AF.Exp,
                                         scale=-0.5)
                    for k in range(2):
                        i = i0 + k
                        nc.vector.tensor_scalar(
                            out=mh_head[:, i, :], in0=pms[k][:, 0:DV],
                            scalar1=mvs[:, k, 0:1], scalar2=rstd2[:, k:k + 1],
                            op0=OP.subtract, op1=OP.mult)
                # transpose all 8 [128,128] tiles via DMA XBAR
                nc.sync.dma_start(
                    out=mhcT[:, hh, :].rearrange("p (t f) -> p t f", t=NT),
                    in_=mh_head, transpose=True)

        # ---- phase E: output projection + MLP ---------------------------
        o_ln = persist.tile([P, NT, D], BF16, tag="oln")
        orows = persist.tile([P, NT, D], F32, tag="orows")
        mvE = small.tile([P, NT, 2], F32, tag="mvE", bufs=1)
        veE = small.tile([P, NT], F32, tag="veE", bufs=1)
        with tc.tile_pool(name="psO", bufs=2, space=bass.MemorySpace.PSUM) as psO, \
             tc.tile_pool(name="psTe", bufs=2, space=bass.MemorySpace.PSUM) as psTe:
            for i in range(NT):
                pso = psO.tile([P, D], F32, tag="pso")
                for hh in range(HEADS):
                    nc.tensor.matmul(
                        pso, mhcT[:, hh, i * P:(i + 1) * P], wo_sb[:, hh, :],
                        start=(hh == 0), stop=False)
                nc.tensor.matmul(pso, onesb, bo_sb, start=False, stop=True)
                nc.vector.tensor_add(
                    out=orows[:, i, :], in0=pso, in1=h_sb[:, i, :])
                s6 = small.tile([P, 6], F32, tag="s6")
                nc.vector.bn_stats(out=s6, in_=orows[:, i, :])
                nc.vector.bn_aggr(out=mvE[:, i, :], in_=s6)
                nc.vector.tensor_scalar_add(
                    out=veE[:, i:i + 1], in0=mvE[:, i, 1:2], scalar1=EPS)
            lnE = small.tile([P, NT], F32, tag="lnE", bufs=1)
            nc.scalar.activation(out=lnE, in_=veE, func=AF.Ln)
            rstdE = small.tile([P, NT], F32, tag="rstdE", bufs=1)
            nc.scalar.activation(out=rstdE, in_=lnE, func=AF.Exp, scale=-0.5)
            oT = persist.tile([P, N], BF16, tag="oT")
            for i in range(NT):
                nc.vector.tensor_scalar(
                    out=o_ln[:, i, :], in0=orows[:, i, :],
                    scalar1=mvE[:, i, 0:1], scalar2=rstdE[:, i:i + 1],
                    op0=OP.subtract, op1=OP.mult)
                nc.vector.tensor_mul(out=o_ln[:, i, :], in0=o_ln[:, i, :],
                                     in1=gb[2])
                nc.vector.tensor_add(out=o_ln[:, i, :], in0=o_ln[:, i, :],
                                     in1=beb[2])
                ptb = psTe.tile([P, P], BF16, tag="pte")
                nc.tensor.transpose(ptb, o_ln[:, i, :], identb)
                nc.vector.tensor_copy(out=oT[:, i * P:(i + 1) * P], in_=ptb)

            r1T = persist.tile([P, N], BF16, tag="r1T")
            for c in range(2):
                psc = ps512.tile([P, 512], F32, tag="ps512")
                nc.tensor.matmul(
                    psc, w1_sb, oT[:, c * 512:(c + 1) * 512],
                    start=True, stop=True)
                nc.scalar.activation(
                    out=r1T[:, c * 512:(c + 1) * 512], in_=psc, func=AF.Relu,
                    bias=b1_sb)
            r2T = persist.tile([P, N], BF16, tag="r2T")
            for c in range(2):
                psc = ps512.tile([P, 512], F32, tag="ps512")
                nc.tensor.matmul(
                    psc, w2_sb, r1T[:, c * 512:(c + 1) * 512],
                    start=True, stop=True)
                nc.scalar.activation(
                    out=r2T[:, c * 512:(c + 1) * 512], in_=psc, func=AF.Relu,
                    bias=b2_sb)

            r2tok = persist.tile([P, NT, D], BF16, tag="r2tok")
            mvR = small.tile([P, NT, 2], F32, tag="mvR", bufs=1)
            veR = small.tile([P, NT], F32, tag="veR", bufs=1)
            for i in range(NT):
                ptb = psTe.tile([P, P], BF16, tag="pte")
                nc.tensor.transpose(ptb, r2T[:, i * P:(i + 1) * P], identb)
                nc.vector.tensor_copy(out=r2tok[:, i, :], in_=ptb)
                s6 = small.tile([P, 6], F32, tag="s6")
                nc.vector.bn_stats(out=s6, in_=r2tok[:, i, :])
                nc.vector.bn_aggr(out=mvR[:, i, :], in_=s6)
                nc.vector.tensor_scalar_add(
                    out=veR[:, i:i + 1], in0=mvR[:, i, 1:2], scalar1=EPS)
            lnR = small.tile([P, NT], F32, tag="lnR", bufs=1)
            nc.scalar.activation(out=lnR, in_=veR, func=AF.Ln)
            rstdR = small.tile([P, NT], F32, tag="rstdR", bufs=1)
            nc.scalar.activation(out=rstdR, in_=lnR, func=AF.Exp, scale=-0.5)
            out_sb = persist.tile([P, NT, D], F32, tag="osb")
            for i in range(NT):
                ro = stg.tile([P, D], BF16, name="ro", tag="ro")
                nc.vector.tensor_scalar(
                    out=ro, in0=r2tok[:, i, :], scalar1=mvR[:, i, 0:1],
                    scalar2=rstdR[:, i:i + 1], op0=OP.subtract, op1=OP.mult)
                nc.vector.tensor_mul(out=ro, in0=ro, in1=gb[3])
                nc.vector.tensor_add(out=ro, in0=ro, in1=beb[3])
                nc.vector.tensor_add(out=out_sb[:, i, :], in0=o_ln[:, i, :],
                                     in1=ro)
            nc.sync.dma_start(
                out=out_dram.rearrange("(t p) d -> p t d", p=P), in_=out_sb)

    nc.compile()
    return nc


def _get_program():
    if "nc" not in _prog_cache:
        _prog_cache["nc"] = _build_program()
    return _prog_cache["nc"]


def kernel(**inputs):
    nc = _get_program()
    f32 = np.float32
    bf16 = ml_dtypes.bfloat16
    f8 = ml_dtypes.float8_e4m3fn

    H = np.asarray(inputs["H"], dtype=f32)
    A = np.asarray(inputs["A"], dtype=f32)
    g1 = np.asarray(inputs["g1"], dtype=f32)
    be1 = np.asarray(inputs["be1"], dtype=f32)
    WO = np.asarray(inputs["W_O"], dtype=f32)
    # fold LN1 gain/bias into the output projection
    WO_fold = WO * np.tile(g1, HEADS)[:, None]
    bO = np.tile(be1, HEADS) @ WO

    BT = np.asarray(inputs["B_bias"], dtype=f32).transpose(0, 2, 1)
    base = {
        "bt": np.ascontiguousarray(BT * 16.0).astype(f8),
        "wq": (np.asarray(inputs["W_Q"], dtype=f32) * DK).astype(bf16),
        "wk": np.asarray(inputs["W_K"], dtype=f32).astype(bf16),
        "wv": np.asarray(inputs["W_V"], dtype=f32).astype(bf16),
        "bqr": np.ascontiguousarray(
            (np.asarray(inputs["b_Q"], dtype=f32) * DK).reshape(NT, P).T),
        "bkr": np.ascontiguousarray(
            np.asarray(inputs["b_K"], dtype=f32).reshape(NT, P).T),
        "bv": np.asarray(inputs["b_V"], dtype=f32),
        "wo": WO_fold.astype(bf16),
        "bo": bO.reshape(1, D).astype(bf16),
        "w1": np.asarray(inputs["W1"], dtype=f32).astype(bf16),
        "w2": np.asarray(inputs["W2"], dtype=f32).astype(bf16),
        "b1": np.asarray(inputs["b1"], dtype=f32).reshape(D, 1),
        "b2": np.asarray(inputs["b2"], dtype=f32).reshape(D, 1),
    }
    for i in (0, 2, 3):
        base[f"g{i}"] = np.asarray(inputs[f"g{i}"], dtype=f32).astype(bf16)
        base[f"be{i}"] = np.asarray(inputs[f"be{i}"], dtype=f32).astype(bf16)

    in_maps = []
    for c in range(B):
        m = dict(base)
        m["h"] = H[c].astype(bf16)
        m["at"] = np.ascontiguousarray(A[c].T).astype(bf16)
        in_maps.append(m)

    res = run_bass_kernel_spmd(nc, in_maps, list(range(B)))
    out = np.stack([res.results[c]["out"] for c in range(B)], axis=0)
    return out.astype(np.float32)


if __name__ == "__main__":
    nc = _get_program()
    print("program built ok")
    from concourse.timeline_sim import TimelineSim
    ns = TimelineSim(nc, trace=False).simulate()
    print(f"TimelineSim: {ns:.0f} ns")


# revision 6
# speedup vs baseline: 1.9672x; 1.3330x over previous
"""Trainium2 Bass kernel for nn_GRIC_31550829756424 (GCN-attention block).

Data-parallel over batch: 8 batches -> 8 NeuronCores, one full batch per core.

Key structure (v2):
- GCN reassociated: adjHnT = Hn^T @ adj_norm^T computed once (shared by
  Q/K/V), then Q/K/V are single-step K=128 matmuls.  A is host-transposed
  and bf16; both degree scalings fold into A^T via one scalar_tensor_tensor.
- All rsqrt computed as exp(-0.5*ln(x)) so the ONLY activation table used is
  natural_log_exp_and_others (Exp/Ln/Relu/Copy/Identity) -> 1 table load.
- Attention bias B added into the QK PSUM via an fp8 DoubleRow matmul
  (identity/32 stationary, bias*16 moving) instead of DVE adds.
- LN1 gain/bias folded into W_O on the host; MH transposed via DMA XBAR.

Self-contained: hardcodes all shapes; imports only the in-container concourse
stack.
"""

import sys

sys.path.insert(0, "/opt/trn_rl_repo")

import numpy as np
import ml_dtypes
from contextlib import ExitStack

import concourse.bass as bass
import concourse.tile as tile
from concourse import bacc
from concourse import mybir
from concourse.bass_utils import run_bass_kernel_spmd
from concourse.masks import make_identity

F32 = mybir.dt.float32
BF16 = mybir.dt.bfloat16
F8 = mybir.dt.float8e4
AF = mybir.ActivationFunctionType
OP = mybir.AluOpType
PM = mybir.MatmulPerfMode

B = 8
N = 1024
D = 128
HEADS = 8
DV = 128
HD = HEADS * DV  # 1024
P = 128
NT = N // P  # 8 tiles of 128 rows
DK = 1.0 / float(np.sqrt(np.float32(D)))
EPS = 1e-5

_prog_cache = {}


def _bcast_load(nc, dst, src):
    """DMA-load 1D DRAM vector src [W] replicated across all P partitions of
    dst [P, W] (same dtype)."""
    rep = bass.AP(tensor=src.tensor, offset=src.offset, ap=[[0, P]] + list(src.ap))
    nc.sync.dma_start(out=dst, in_=rep)


def _dup2(ap):
    """View a [P, W] AP as [P, 2, W] with a stride-0 middle dim (DoubleRow
    moving operand reading the same data in both slots)."""
    return bass.AP(
        tensor=ap.tensor, offset=ap.offset,
        ap=[list(ap.ap[0]), [0, 2]] + [list(a) for a in ap.ap[1:]],
    )


def _build_program():
    nc = bacc.Bacc(None)

    h_in = nc.declare_dram_parameter("h", [N, D], BF16, isOutput=False)
    at_in = nc.declare_dram_parameter("at", [N, N], BF16, isOutput=False)
    bt_in = nc.declare_dram_parameter("bt", [HEADS, N, N], F8, isOutput=False)
    wq_in = nc.declare_dram_parameter("wq", [D, HD], BF16, isOutput=False)
    wk_in = nc.declare_dram_parameter("wk", [D, HD], BF16, isOutput=False)
    wv_in = nc.declare_dram_parameter("wv", [D, HD], BF16, isOutput=False)
    bqr_in = nc.declare_dram_parameter("bqr", [P, NT], F32, isOutput=False)
    bkr_in = nc.declare_dram_parameter("bkr", [P, NT], F32, isOutput=False)
    bv_in = nc.declare_dram_parameter("bv", [HD], F32, isOutput=False)
    wo_in = nc.declare_dram_parameter("wo", [HD, D], BF16, isOutput=False)
    bo_in = nc.declare_dram_parameter("bo", [1, D], BF16, isOutput=False)
    w1_in = nc.declare_dram_parameter("w1", [D, D], BF16, isOutput=False)
    w2_in = nc.declare_dram_parameter("w2", [D, D], BF16, isOutput=False)
    b1_in = nc.declare_dram_parameter("b1", [D, 1], F32, isOutput=False)
    b2_in = nc.declare_dram_parameter("b2", [D, 1], F32, isOutput=False)
    g_in = {}
    be_in = {}
    for i in (0, 2, 3):
        g_in[i] = nc.declare_dram_parameter(f"g{i}", [D], BF16, isOutput=False)
        be_in[i] = nc.declare_dram_parameter(f"be{i}", [D], BF16, isOutput=False)
    out_dram = nc.declare_dram_parameter("out", [N, D], F32, isOutput=True)

    with tile.TileContext(nc) as tc, ExitStack() as ctx:
        consts = ctx.enter_context(tc.tile_pool(name="consts", bufs=1))
        persist = ctx.enter_context(tc.tile_pool(name="persist", bufs=1))
        small = ctx.enter_context(tc.tile_pool(name="small", bufs=12))
        stg = ctx.enter_context(tc.tile_pool(name="stg", bufs=3))
        # 2-bank 512-wide PSUM pool: open through B/C, reused in E via scope.
        ps512 = ctx.enter_context(
            tc.tile_pool(name="ps512", bufs=2, space=bass.MemorySpace.PSUM))

        # ---- constants -------------------------------------------------
        identb = consts.tile([P, P], BF16)
        make_identity(nc, identb)
        omib = consts.tile([P, P], BF16)  # 1 - I
        nc.gpsimd.memset(omib, 1.0)
        nc.gpsimd.affine_select(
            out=omib, in_=omib, compare_op=OP.not_equal, fill=0.0,
            base=0, pattern=[[-1, P]], channel_multiplier=1)
        # fp8 DoubleRow stationary: two slots of I/32 (moving is bias*16).
        id2 = consts.tile([P, 2, P], F8)
        nc.gpsimd.memset(id2, 0.0)
        for s in range(2):
            nc.gpsimd.affine_select(
                out=id2[:, s, :], in_=id2[:, s, :], compare_op=OP.not_equal,
                fill=1.0 / 32.0, base=0, pattern=[[-1, P]], channel_multiplier=1)
        nc.scalar.add_instruction(
            mybir.InstLoadActFuncSet(
                name=nc.get_next_instruction_name(), ins=[], outs=[],
                act_func_set_id=6))
        eps_t = consts.tile([P, 1], F32)
        nc.vector.memset(eps_t, EPS)
        onesb = consts.tile([1, P], BF16)
        nc.vector.memset(onesb, 1.0)
        onescol = consts.tile([P, 1], BF16)
        nc.vector.memset(onescol, 1.0)

        gb = {}
        beb = {}
        for i in (0, 2, 3):
            gb[i] = consts.tile([P, D], BF16, name=f"g{i}b", tag=f"g{i}b")
            _bcast_load(nc, gb[i], g_in[i][:])
            beb[i] = consts.tile([P, D], BF16, name=f"be{i}b", tag=f"be{i}b")
            _bcast_load(nc, beb[i], be_in[i][:])
        bvb = consts.tile([P, HD], F32)
        _bcast_load(nc, bvb, bv_in[:])
        bqr = consts.tile([P, NT], F32)
        nc.sync.dma_start(out=bqr, in_=bqr_in[:, :])
        bkr = consts.tile([P, NT], F32)
        nc.sync.dma_start(out=bkr, in_=bkr_in[:, :])
        wq_sb = consts.tile([P, HD], BF16)
        nc.sync.dma_start(out=wq_sb, in_=wq_in[:, :])
        wk_sb = consts.tile([P, HD], BF16)
        nc.sync.dma_start(out=wk_sb, in_=wk_in[:, :])
        wv_sb = consts.tile([P, HD], BF16)
        nc.sync.dma_start(out=wv_sb, in_=wv_in[:, :])
        # W_O as [p=dv-within-head, h, d]
        wo_sb = consts.tile([P, HEADS, D], BF16)
        nc.sync.dma_start(
            out=wo_sb, in_=wo_in.rearrange("(hh p) d -> p hh d", p=P))
        bo_sb = consts.tile([1, D], BF16)
        nc.sync.dma_start(out=bo_sb, in_=bo_in[:, :])
        w1_sb = consts.tile([P, D], BF16)
        nc.sync.dma_start(out=w1_sb, in_=w1_in[:, :])
        w2_sb = consts.tile([P, D], BF16)
        nc.sync.dma_start(out=w2_sb, in_=w2_in[:, :])
        b1_sb = consts.tile([P, 1], F32)
        nc.sync.dma_start(out=b1_sb, in_=b1_in[:, :])
        b2_sb = consts.tile([P, 1], F32)
        nc.sync.dma_start(out=b2_sb, in_=b2_in[:, :])

        # ---- persistent tensors ---------------------------------------
        h_sb = persist.tile([P, NT, D], BF16, tag="h")
        nc.sync.dma_start(out=h_sb, in_=h_in.rearrange("(t p) d -> p t d", p=P))
        hn_t = persist.tile([P, NT, D], BF16, tag="hnt")
        ats = persist.tile([P, NT, N], BF16, tag="ats")  # A^T -> adj_norm^T
        disb = persist.tile([P, N], BF16, tag="disb")
        dis_tok = persist.tile([P, NT], F32, tag="distok")
        adjHnT = persist.tile([P, N], BF16, tag="adjhnt")
        qT = persist.tile([P, HEADS, N], BF16, tag="qT")
        kT = persist.tile([P, HEADS, N], BF16, tag="kT")
        vna = persist.tile([P, NT, HEADS, DV + 1], BF16, tag="v")
        nc.vector.memset(vna[:, :, :, DV:DV + 1], 1.0)
        mhcT = persist.tile([P, HEADS, N], BF16, tag="mhcT")

        # ---- phase A: H LayerNorm (token-major) ------------------------
        for i in range(NT):
            s6 = small.tile([P, 6], F32, tag="s6")
            mv = small.tile([P, 2], F32, tag="mv")
            nc.vector.bn_stats(out=s6, in_=h_sb[:, i, :])
            nc.vector.bn_aggr(out=mv, in_=s6)
            lnv = small.tile([P, 1], F32, tag="lnv")
            nc.scalar.activation(out=lnv, in_=mv[:, 1:2], func=AF.Ln, bias=eps_t)
            rstd = small.tile([P, 1], F32, tag="rstd")
            nc.scalar.activation(out=rstd, in_=lnv, func=AF.Exp, scale=-0.5)
            hw = stg.tile([P, D], BF16, name="hw", tag="hw")
            nc.vector.tensor_scalar(
                out=hw, in0=h_sb[:, i, :], scalar1=mv[:, 0:1], scalar2=rstd,
                op0=OP.subtract, op1=OP.mult)
            nc.vector.tensor_mul(out=hw, in0=hw, in1=gb[0])
            nc.vector.tensor_add(out=hn_t[:, i, :], in0=hw, in1=beb[0])

        # ---- phase B: adjacency prep ------------------------------------
        with tc.tile_pool(name="psR", bufs=1, space=bass.MemorySpace.PSUM) as psR, \
             tc.tile_pool(name="psTb", bufs=2, space=bass.MemorySpace.PSUM) as psTb:
            rs_ps = [psR.tile([1, 512], F32, name=f"rsps{c}", tag=f"rsps{c}")
                     for c in range(2)]
            for j in range(NT):
                nc.sync.dma_start(
                    out=ats[:, j, :],
                    in_=at_in[:, :].rearrange("(t p) n -> p t n", p=P)[:, j, :])
                db = ats[:, j, j * P:(j + 1) * P]
                nc.vector.tensor_mul(out=db, in0=db, in1=omib)
                nc.vector.tensor_add(out=db, in0=db, in1=identb)
                for c in range(2):
                    nc.tensor.matmul(
                        rs_ps[c], onescol, ats[:, j, c * 512:(c + 1) * 512],
                        start=(j == 0), stop=(j == NT - 1))
            rs_sb = small.tile([1, N], F32, tag="rssb", bufs=1)
            for c in range(2):
                nc.vector.tensor_scalar_max(
                    out=rs_sb[:, c * 512:(c + 1) * 512], in0=rs_ps[c], scalar1=1.0)
            lnr = small.tile([1, N], F32, tag="lnr", bufs=1)
            nc.scalar.activation(out=lnr, in_=rs_sb, func=AF.Ln)
            disrow = small.tile([1, N], BF16, tag="disrow", bufs=1)
            nc.scalar.activation(out=disrow, in_=lnr, func=AF.Exp, scale=-0.5)
            # broadcast di over partitions: disb[q, n] = di_n
            for c in range(2):
                psd = ps512.tile([P, 512], F32, tag="ps512")
                nc.tensor.matmul(
                    psd, onesb, disrow[:, c * 512:(c + 1) * 512],
                    start=True, stop=True)
                nc.vector.tensor_copy(
                    out=disb[:, c * 512:(c + 1) * 512], in_=psd)
            # dis_tok[p, j] = di_{j*P+p} via PE transpose of disb chunks
            for j in range(NT):
                ptb = psTb.tile([P, P], BF16, tag="ptb")
                nc.tensor.transpose(ptb, disb[:, j * P:(j + 1) * P], identb)
                nc.vector.tensor_copy(out=dis_tok[:, j:j + 1], in_=ptb[:, 0:1])
            # fold both degree scalings into A^T (in place):
            # ats[m, n] = At[m, n] * di_m * di_n
            for j in range(NT):
                nc.vector.scalar_tensor_tensor(
                    out=ats[:, j, :], in0=ats[:, j, :],
                    scalar=dis_tok[:, j:j + 1], in1=disb,
                    op0=OP.mult, op1=OP.mult)

        # ---- phase C: shared GCN trunk + Q/K/V --------------------------
        # adjHnT[d, n] = sum_m hn[m, d] * adjn^T[m, n]
        for c in range(2):
            psc = ps512.tile([P, 512], F32, tag="ps512")
            for j in range(NT):
                nc.tensor.matmul(
                    psc, hn_t[:, j, :], ats[:, j, c * 512:(c + 1) * 512],
                    start=(j == 0), stop=(j == NT - 1))
            nc.vector.tensor_copy(out=adjHnT[:, c * 512:(c + 1) * 512], in_=psc)
        # V first so heads 0-3 unblock early
        for c in range(2):
            for i in range(NT):
                psc = ps512.tile([P, 512], F32, tag="ps512")
                nc.tensor.matmul(
                    psc, adjHnT[:, i * P:(i + 1) * P],
                    wv_sb[:, c * 512:(c + 1) * 512], start=True, stop=True)
                nc.vector.tensor_add(
                    out=vna[:, i, c * 4:(c + 1) * 4, 0:DV],
                    in0=psc.rearrange("p (a b) -> p a b", a=4),
                    in1=bvb[:, c * 512:(c + 1) * 512].rearrange(
                        "p (a b) -> p a b", a=4))
        for hh in range(HEADS):
            for c in range(2):
                psc = ps512.tile([P, 512], F32, tag="ps512")
                nc.tensor.matmul(
                    psc, wq_sb[:, hh * P:(hh + 1) * P],
                    adjHnT[:, c * 512:(c + 1) * 512], start=True, stop=True)
                nc.vector.tensor_scalar_add(
                    out=qT[:, hh, c * 512:(c + 1) * 512], in0=psc,
                    scalar1=bqr[:, hh:hh + 1])
                psc = ps512.tile([P, 512], F32, tag="ps512")
                nc.tensor.matmul(
                    psc, wk_sb[:, hh * P:(hh + 1) * P],
                    adjHnT[:, c * 512:(c + 1) * 512], start=True, stop=True)
                nc.vector.tensor_scalar_add(
                    out=kT[:, hh, c * 512:(c + 1) * 512], in0=psc,
                    scalar1=bkr[:, hh:hh + 1])

        # ---- phase D: attention per head --------------------------------
        with tc.tile_pool(name="psE", bufs=2, space=bass.MemorySpace.PSUM) as psE, \
             tc.tile_pool(name="psPM", bufs=2, space=bass.MemorySpace.PSUM) as psPM, \
             tc.tile_pool(name="etp", bufs=2) as etp, \
             tc.tile_pool(name="btp", bufs=2) as btp, \
             tc.tile_pool(name="mhp", bufs=2) as mhp:
            for hh in range(HEADS):
                btile = btp.tile([P, NT, N], F8, tag="bt")
                nc.sync.dma_start(
                    out=btile,
                    in_=bt_in[hh].rearrange("(t p) n -> p t n", p=P))
                et = etp.tile([P, NT, N], BF16, tag="et")
                for j in range(NT):
                    pse = psE.tile([P, N], F32, tag="pse")
                    for c in range(2):
                        nc.tensor.matmul(
                            pse[:, c * 512:(c + 1) * 512],
                            kT[:, hh, j * P:(j + 1) * P],
                            qT[:, hh, c * 512:(c + 1) * 512],
                            start=True, stop=False)
                        nc.tensor.matmul(
                            pse[:, c * 512:(c + 1) * 512],
                            id2, _dup2(btile[:, j, c * 512:(c + 1) * 512]),
                            start=False, stop=True, perf_mode=PM.DoubleRow)
                    nc.scalar.activation(out=et[:, j, :], in_=pse, func=AF.Exp)
                mh_head = mhp.tile([P, NT, DV], BF16, tag="mh")
                # PV in pairs of query tiles; LN rstd batched per pair.
                for i0 in range(0, NT, 2):
                    pms = []
                    mvs = small.tile([P, 2, 2], F32, tag="mvs", bufs=4)
                    vee = small.tile([P, 2], F32, tag="vee", bufs=4)
                    for k in range(2):
                        i = i0 + k
                        pm = psPM.tile([P, DV + 1], F32, tag="pm")
                        pms.append(pm)
                        for j in range(NT):
                            nc.tensor.matmul(
                                pm, et[:, j, i * P:(i + 1) * P],
                                vna[:, j, hh, :],
                                start=(j == 0), stop=(j == NT - 1))
                        s6 = small.tile([P, 6], F32, tag="s6")
                        nc.vector.bn_stats(out=s6, in_=pm[:, 0:DV])
                        nc.vector.bn_aggr(out=mvs[:, k, :], in_=s6)
                        # vee = var + eps * rowsum^2
                        t = small.tile([P, 1], F32, tag="t")
                        nc.vector.tensor_scalar(
                            out=t, in0=pm[:, DV:DV + 1],
                            scalar1=pm[:, DV:DV + 1], scalar2=EPS,
                            op0=OP.mult, op1=OP.mult)
                        nc.vector.tensor_add(
                            out=vee[:, k:k + 1], in0=t, in1=mvs[:, k, 1:2])
                    lnv2 = small.tile([P, 2], F32, tag="lnv2", bufs=4)
                    nc.scalar.activation(out=lnv2, in_=vee, func=AF.Ln)
                    rstd2 = small.tile([P, 2], F32, tag="rstd2", bufs=4)
                    nc.scalar.activation(out=rstd2, in_=lnv2, func=AF.Exp,
                                         scale=-0.5)
                    for k in range(2):
                        i = i0 + k
                        nc.vector.tensor_scalar(
                            out=mh_head[:, i, :], in0=pms[k][:, 0:DV],
                            scalar1=mvs[:, k, 0:1], scalar2=rstd2[:, k:k + 1],
                            op0=OP.subtract, op1=OP.mult)
                # transpose all 8 [128,128] tiles via DMA XBAR
                nc.sync.dma_start(
                    out=mhcT[:, hh, :].rearrange("p (t f) -> p t f", t=NT),
                    in_=mh_head, transpose=True)

        # ---- phase E: output projection + MLP ---------------------------
        o_ln = persist.tile([P, NT, D], BF16, tag="oln")
        orows = persist.tile([P, NT, D], F32, tag="orows")
        mvE = small.tile([P, NT, 2], F32, tag="mvE", bufs=1)
        veE = small.tile([P, NT], F32, tag="veE", bufs=1)
        with tc.tile_pool(name="psO", bufs=2, space=bass.MemorySpace.PSUM) as psO, \
             tc.tile_pool(name="psTe", bufs=2, space=bass.MemorySpace.PSUM) as psTe:
            for i in range(NT):
                pso = psO.tile([P, D], F32, tag="pso")
                for hh in range(HEADS):
                    nc.tensor.matmul(
                        pso, mhcT[:, hh, i * P:(i + 1) * P], wo_sb[:, hh, :],
                        start=(hh == 0), stop=False)
                nc.tensor.matmul(pso, onesb, bo_sb, start=False, stop=True)
                nc.vector.tensor_add(
                    out=orows[:, i, :], in0=pso, in1=h_sb[:, i, :])
                s6 = small.tile([P, 6], F32, tag="s6")
                nc.vector.bn_stats(out=s6, in_=orows[:, i, :])
                nc.vector.bn_aggr(out=mvE[:, i, :], in_=s6)
                nc.vector.tensor_scalar_add(
                    out=veE[:, i:i + 1], in0=mvE[:, i, 1:2], scalar1=EPS)
            lnE = small.tile([P, NT], F32, tag="lnE", bufs=1)
            nc.scalar.activation(out=lnE, in_=veE, func=AF.Ln)
            rstdE = small.tile([P, NT], F32, tag="rstdE", bufs=1)
            nc.scalar.activation(out=rstdE, in_=lnE, func=AF.Exp, scale=-0.5)
            oT = persist.tile([P, N], BF16, tag="oT")
            for i in range(NT):
                nc.vector.tensor_scalar(
                    out=o_ln[:, i, :], in0=orows[:, i, :],
                    scalar1=mvE[:, i, 0:1], scalar2=rstdE[:, i:i + 1],
                    op0=OP.subtract, op1=OP.mult)
                nc.vector.tensor_mul(out=o_ln[:, i, :], in0=o_ln[:, i, :],
                                     in1=gb[2])
                nc.vector.tensor_add(out=o_ln[:, i, :], in0=o_ln[:, i, :],
                                     in1=beb[2])
                ptb = psTe.tile([P, P], BF16, tag="pte")
                nc.tensor.transpose(ptb, o_ln[:, i, :], identb)
                nc.vector.tensor_copy(out=oT[:, i * P:(i + 1) * P], in_=ptb)

            r1T = persist.tile([P, N], BF16, tag="r1T")
            for c in range(2):
                psc = ps512.tile([P, 512], F32, tag="ps512")
                nc.tensor.matmul(
                    psc, w1_sb, oT[:, c * 512:(c + 1) * 512],
                    start=True, stop=True)
                nc.scalar.activation(
                    out=r1T[:, c * 512:(c + 1) * 512], in_=psc, func=AF.Relu,
                    bias=b1_sb)
            r2T = persist.tile([P, N], BF16, tag="r2T")
            for c in range(2):
                psc = ps512.tile([P, 512], F32, tag="ps512")
                nc.tensor.matmul(
                    psc, w2_sb, r1T[:, c * 512:(c + 1) * 512],
                    start=True, stop=True)
                nc.scalar.activation(
                    out=r2T[:, c * 512:(c + 1) * 512], in_=psc, func=AF.Relu,
                    bias=b2_sb)

            r2tok = persist.tile([P, NT, D], BF16, tag="r2tok")
            mvR = small.tile([P, NT, 2], F32, tag="mvR", bufs=1)
            veR = small.tile([P, NT], F32, tag="veR", bufs=1)
            for i in range(NT):
                ptb = psTe.tile([P, P], BF16, tag="pte")
                nc.tensor.transpose(ptb, r2T[:, i * P:(i + 1) * P], identb)
                nc.vector.tensor_copy(out=r2tok[:, i, :], in_=ptb)
                s6 = small.tile([P, 6], F32, tag="s6")
                nc.vector.bn_stats(out=s6, in_=r2tok[:, i, :])
                nc.vector.bn_aggr(out=mvR[:, i, :], in_=s6)
                nc.vector.tensor_scalar_add(
                    out=veR[:, i:i + 1], in0=mvR[:, i, 1:2], scalar1=EPS)
            lnR = small.tile([P, NT], F32, tag="lnR", bufs=1)
            nc.scalar.activation(out=lnR, in_=veR, func=AF.Ln)
            rstdR = small.tile([P, NT], F32, tag="rstdR", bufs=1)
            nc.scalar.activation(out=rstdR, in_=lnR, func=AF.Exp, scale=-0.5)
            out_sb = persist.tile([P, NT, D], F32, tag="osb")
            for i in range(NT):
                ro = stg.tile([P, D], BF16, name="ro", tag="ro")
                nc.vector.tensor_scalar(
                    out=ro, in0=r2tok[:, i, :], scalar1=mvR[:, i, 0:1],
                    scalar2=rstdR[:, i:i + 1], op0=OP.subtract, op1=OP.mult)
                nc.vector.tensor_mul(out=ro, in0=ro, in1=gb[3])
                nc.vector.tensor_add(out=ro, in0=ro, in1=beb[3])
                nc.vector.tensor_add(out=out_sb[:, i, :], in0=o_ln[:, i, :],
                                     in1=ro)
            nc.sync.dma_start(
                out=out_dram.rearrange("(t p) d -> p t d", p=P), in_=out_sb)

    nc.compile()
    return nc


def _get_program():
    if "nc" not in _prog_cache:
        _prog_cache["nc"] = _build_program()
    return _prog_cache["nc"]


def kernel(**inputs):
    nc = _get_program()
    f32 = np.float32
    bf16 = ml_dtypes.bfloat16
    f8 = ml_dtypes.float8_e4m3fn

    H = np.asarray(inputs["H"], dtype=f32)
    A = np.asarray(inputs["A"], dtype=f32)
    g1 = np.asarray(inputs["g1"], dtype=f32)
    be1 = np.asarray(inputs["be1"], dtype=f32)
    WO = np.asarray(inputs["W_O"], dtype=f32)
    # fold LN1 gain/bias into the output projection
    WO_fold = WO * np.tile(g1, HEADS)[:, None]
    bO = np.tile(be1, HEADS) @ WO

    BT = np.asarray(inputs["B_bias"], dtype=f32).transpose(0, 2, 1)
    base = {
        "bt": np.ascontiguousarray(BT * 16.0).astype(f8),
        "wq": (np.asarray(inputs["W_Q"], dtype=f32) * DK).astype(bf16),
        "wk": np.asarray(inputs["W_K"], dtype=f32).astype(bf16),
        "wv": np.asarray(inputs["W_V"], dtype=f32).astype(bf16),
        "bqr": np.ascontiguousarray(
            (np.asarray(inputs["b_Q"], dtype=f32) * DK).reshape(NT, P).T),
        "bkr": np.ascontiguousarray(
            np.asarray(inputs["b_K"], dtype=f32).reshape(NT, P).T),
        "bv": np.asarray(inputs["b_V"], dtype=f32),
        "wo": WO_fold.astype(bf16),
        "bo": bO.reshape(1, D).astype(bf16),
        "w1": np.asarray(inputs["W1"], dtype=f32).astype(bf16),
        "w2": np.asarray(inputs["W2"], dtype=f32).astype(bf16),
        "b1": np.asarray(inputs["b1"], dtype=f32).reshape(D, 1),
        "b2": np.asarray(inputs["b2"], dtype=f32).reshape(D, 1),
    }
    for i in (0, 2, 3):
        base[f"g{i}"] = np.asarray(inputs[f"g{i}"], dtype=f32).astype(bf16)
        base[f"be{i}"] = np.asarray(inputs[f"be{i}"], dtype=f32).astype(bf16)

    in_maps = []
    for c in range(B):
        m = dict(base)
        m["h"] = H[c].astype(bf16)
        m["at"] = np.ascontiguousarray(A[c].T).astype(bf16)
        in_maps.append(m)

    res = run_bass_kernel_spmd(nc, in_maps, list(range(B)))
    out = np.stack([res.results[c]["out"] for c in range(B)], axis=0)
    return out.astype(np.float32)


if __name__ == "__main__":
    nc = _get_program()
    print("program built ok")
    from concourse.timeline_sim import TimelineSim
    ns = TimelineSim(nc, trace=False).simulate()
    print(f"TimelineSim: {ns:.0f} ns")


# revision 7
# speedup vs baseline: 2.0041x; 1.0188x over previous
"""Trainium2 Bass kernel for nn_GRIC_31550829756424 (GCN-attention block).

Data-parallel over batch: 8 batches -> 8 NeuronCores, one full batch per core.

Key structure (v2):
- GCN reassociated: adjHnT = Hn^T @ adj_norm^T computed once (shared by
  Q/K/V), then Q/K/V are single-step K=128 matmuls.  A is host-transposed
  and bf16; both degree scalings fold into A^T via one scalar_tensor_tensor.
- All rsqrt computed as exp(-0.5*ln(x)) so the ONLY activation table used is
  natural_log_exp_and_others (Exp/Ln/Relu/Copy/Identity) -> 1 table load.
- Attention bias B added into the QK PSUM via an fp8 DoubleRow matmul
  (identity/32 stationary, bias*16 moving) instead of DVE adds.
- LN1 gain/bias folded into W_O on the host; MH transposed via DMA XBAR.

Self-contained: hardcodes all shapes; imports only the in-container concourse
stack.
"""

import sys

sys.path.insert(0, "/opt/trn_rl_repo")

import numpy as np
import ml_dtypes
from contextlib import ExitStack

import concourse.bass as bass
import concourse.tile as tile
from concourse import bacc
from concourse import mybir
from concourse.bass_utils import run_bass_kernel_spmd
from concourse.masks import make_identity

F32 = mybir.dt.float32
BF16 = mybir.dt.bfloat16
F8 = mybir.dt.float8e4
AF = mybir.ActivationFunctionType
OP = mybir.AluOpType
PM = mybir.MatmulPerfMode

B = 8
N = 1024
D = 128
HEADS = 8
DV = 128
HD = HEADS * DV  # 1024
P = 128
NT = N // P  # 8 tiles of 128 rows
DK = 1.0 / float(np.sqrt(np.float32(D)))
EPS = 1e-5

_prog_cache = {}


def _bcast_load(nc, dst, src):
    """DMA-load 1D DRAM vector src [W] replicated across all P partitions of
    dst [P, W] (same dtype)."""
    rep = bass.AP(tensor=src.tensor, offset=src.offset, ap=[[0, P]] + list(src.ap))
    nc.gpsimd.dma_start(out=dst, in_=rep)


def _dup2(ap):
    """View a [P, W] AP as [P, 2, W] with a stride-0 middle dim (DoubleRow
    moving operand reading the same data in both slots)."""
    return bass.AP(
        tensor=ap.tensor, offset=ap.offset,
        ap=[list(ap.ap[0]), [0, 2]] + [list(a) for a in ap.ap[1:]],
    )


def _build_program():
    nc = bacc.Bacc(None)

    h_in = nc.declare_dram_parameter("h", [N, D], BF16, isOutput=False)
    at_in = nc.declare_dram_parameter("at", [N, N], BF16, isOutput=False)
    bt_in = nc.declare_dram_parameter("bt", [HEADS, N, N], F8, isOutput=False)
    wq_in = nc.declare_dram_parameter("wq", [D, HD], BF16, isOutput=False)
    wk_in = nc.declare_dram_parameter("wk", [D, HD], BF16, isOutput=False)
    wv_in = nc.declare_dram_parameter("wv", [D, HD], BF16, isOutput=False)
    bqr_in = nc.declare_dram_parameter("bqr", [P, NT], F32, isOutput=False)
    bkr_in = nc.declare_dram_parameter("bkr", [P, NT], F32, isOutput=False)
    bv_in = nc.declare_dram_parameter("bv", [HD], F32, isOutput=False)
    wo_in = nc.declare_dram_parameter("wo", [HD, D], BF16, isOutput=False)
    bo_in = nc.declare_dram_parameter("bo", [1, D], BF16, isOutput=False)
    w1_in = nc.declare_dram_parameter("w1", [D, D], BF16, isOutput=False)
    w2_in = nc.declare_dram_parameter("w2", [D, D], BF16, isOutput=False)
    b1_in = nc.declare_dram_parameter("b1", [D, 1], F32, isOutput=False)
    b2_in = nc.declare_dram_parameter("b2", [D, 1], F32, isOutput=False)
    g_in = {}
    be_in = {}
    for i in (0, 2, 3):
        g_in[i] = nc.declare_dram_parameter(f"g{i}", [D], BF16, isOutput=False)
        be_in[i] = nc.declare_dram_parameter(f"be{i}", [D], BF16, isOutput=False)
    out_dram = nc.declare_dram_parameter("out", [N, D], F32, isOutput=True)

    with tile.TileContext(nc) as tc, ExitStack() as ctx:
        consts = ctx.enter_context(tc.tile_pool(name="consts", bufs=1))
        persist = ctx.enter_context(tc.tile_pool(name="persist", bufs=1))
        small = ctx.enter_context(tc.tile_pool(name="small", bufs=12))
        stg = ctx.enter_context(tc.tile_pool(name="stg", bufs=3))
        # 2-bank 512-wide PSUM pool: open through B/C, reused in E via scope.
        ps512 = ctx.enter_context(
            tc.tile_pool(name="ps512", bufs=2, space=bass.MemorySpace.PSUM))

        # ---- constants -------------------------------------------------
        identb = consts.tile([P, P], BF16)
        make_identity(nc, identb)
        omib = consts.tile([P, P], BF16)  # 1 - I
        nc.gpsimd.memset(omib, 1.0)
        nc.gpsimd.affine_select(
            out=omib, in_=omib, compare_op=OP.not_equal, fill=0.0,
            base=0, pattern=[[-1, P]], channel_multiplier=1)
        # fp8 DoubleRow stationary: two slots of I/32 (moving is bias*16).
        id2 = consts.tile([P, 2, P], F8)
        nc.gpsimd.memset(id2, 0.0)
        for s in range(2):
            nc.gpsimd.affine_select(
                out=id2[:, s, :], in_=id2[:, s, :], compare_op=OP.not_equal,
                fill=1.0 / 32.0, base=0, pattern=[[-1, P]], channel_multiplier=1)
        nc.scalar.add_instruction(
            mybir.InstLoadActFuncSet(
                name=nc.get_next_instruction_name(), ins=[], outs=[],
                act_func_set_id=6))
        eps_t = consts.tile([P, 1], F32)
        nc.vector.memset(eps_t, EPS)
        onesb = consts.tile([1, P], BF16)
        nc.vector.memset(onesb, 1.0)
        onescol = consts.tile([P, 1], BF16)
        nc.vector.memset(onescol, 1.0)

        gb = {}
        beb = {}
        for i in (0, 2, 3):
            gb[i] = consts.tile([P, D], BF16, name=f"g{i}b", tag=f"g{i}b")
            _bcast_load(nc, gb[i], g_in[i][:])
            beb[i] = consts.tile([P, D], BF16, name=f"be{i}b", tag=f"be{i}b")
            _bcast_load(nc, beb[i], be_in[i][:])
        bvb = consts.tile([P, HD], F32)
        _bcast_load(nc, bvb, bv_in[:])
        bqr = consts.tile([P, NT], F32)
        nc.gpsimd.dma_start(out=bqr, in_=bqr_in[:, :])
        bkr = consts.tile([P, NT], F32)
        nc.gpsimd.dma_start(out=bkr, in_=bkr_in[:, :])
        wq_sb = consts.tile([P, HD], BF16)
        nc.gpsimd.dma_start(out=wq_sb, in_=wq_in[:, :])
        wk_sb = consts.tile([P, HD], BF16)
        nc.gpsimd.dma_start(out=wk_sb, in_=wk_in[:, :])
        wv_sb = consts.tile([P, HD], BF16)
        nc.gpsimd.dma_start(out=wv_sb, in_=wv_in[:, :])
        # W_O as [p=dv-within-head, h, d]
        wo_sb = consts.tile([P, HEADS, D], BF16)
        nc.sync.dma_start(
            out=wo_sb, in_=wo_in.rearrange("(hh p) d -> p hh d", p=P))
        bo_sb = consts.tile([1, D], BF16)
        nc.gpsimd.dma_start(out=bo_sb, in_=bo_in[:, :])
        w1_sb = consts.tile([P, D], BF16)
        nc.gpsimd.dma_start(out=w1_sb, in_=w1_in[:, :])
        w2_sb = consts.tile([P, D], BF16)
        nc.gpsimd.dma_start(out=w2_sb, in_=w2_in[:, :])
        b1_sb = consts.tile([P, 1], F32)
        nc.gpsimd.dma_start(out=b1_sb, in_=b1_in[:, :])
        b2_sb = consts.tile([P, 1], F32)
        nc.gpsimd.dma_start(out=b2_sb, in_=b2_in[:, :])

        # ---- persistent tensors ---------------------------------------
        h_sb = persist.tile([P, NT, D], BF16, tag="h")
        nc.sync.dma_start(out=h_sb, in_=h_in.rearrange("(t p) d -> p t d", p=P))
        hn_t = persist.tile([P, NT, D], BF16, tag="hnt")
        ats = persist.tile([P, NT, N], BF16, tag="ats")  # A^T -> adj_norm^T
        disb = persist.tile([P, N], BF16, tag="disb")
        dis_tok = persist.tile([P, NT], F32, tag="distok")
        adjHnT = persist.tile([P, N], BF16, tag="adjhnt")
        qT = persist.tile([P, HEADS, N], BF16, tag="qT")
        kT = persist.tile([P, HEADS, N], BF16, tag="kT")
        vna = persist.tile([P, NT, HEADS, DV + 1], BF16, tag="v")
        nc.vector.memset(vna[:, :, :, DV:DV + 1], 1.0)
        mhcT = persist.tile([P, HEADS, N], BF16, tag="mhcT")

        # ---- phase A: H LayerNorm (token-major) ------------------------
        for i in range(NT):
            s6 = small.tile([P, 6], F32, tag="s6")
            mv = small.tile([P, 2], F32, tag="mv")
            nc.vector.bn_stats(out=s6, in_=h_sb[:, i, :])
            nc.vector.bn_aggr(out=mv, in_=s6)
            lnv = small.tile([P, 1], F32, tag="lnv")
            nc.scalar.activation(out=lnv, in_=mv[:, 1:2], func=AF.Ln, bias=eps_t)
            rstd = small.tile([P, 1], F32, tag="rstd")
            nc.scalar.activation(out=rstd, in_=lnv, func=AF.Exp, scale=-0.5)
            hw = stg.tile([P, D], BF16, name="hw", tag="hw")
            nc.vector.tensor_scalar(
                out=hw, in0=h_sb[:, i, :], scalar1=mv[:, 0:1], scalar2=rstd,
                op0=OP.subtract, op1=OP.mult)
            nc.vector.tensor_mul(out=hw, in0=hw, in1=gb[0])
            nc.vector.tensor_add(out=hn_t[:, i, :], in0=hw, in1=beb[0])

        # ---- phase B: adjacency prep ------------------------------------
        with tc.tile_pool(name="psR", bufs=1, space=bass.MemorySpace.PSUM) as psR, \
             tc.tile_pool(name="psTb", bufs=2, space=bass.MemorySpace.PSUM) as psTb:
            rs_ps = [psR.tile([1, 512], F32, name=f"rsps{c}", tag=f"rsps{c}")
                     for c in range(2)]
            for j in range(NT):
                nc.sync.dma_start(
                    out=ats[:, j, :],
                    in_=at_in[:, :].rearrange("(t p) n -> p t n", p=P)[:, j, :])
                db = ats[:, j, j * P:(j + 1) * P]
                nc.vector.tensor_mul(out=db, in0=db, in1=omib)
                nc.vector.tensor_add(out=db, in0=db, in1=identb)
                for c in range(2):
                    nc.tensor.matmul(
                        rs_ps[c], onescol, ats[:, j, c * 512:(c + 1) * 512],
                        start=(j == 0), stop=(j == NT - 1))
            rs_sb = small.tile([1, N], F32, tag="rssb", bufs=1)
            for c in range(2):
                nc.vector.tensor_scalar_max(
                    out=rs_sb[:, c * 512:(c + 1) * 512], in0=rs_ps[c], scalar1=1.0)
            lnr = small.tile([1, N], F32, tag="lnr", bufs=1)
            nc.scalar.activation(out=lnr, in_=rs_sb, func=AF.Ln)
            disrow = small.tile([1, N], BF16, tag="disrow", bufs=1)
            nc.scalar.activation(out=disrow, in_=lnr, func=AF.Exp, scale=-0.5)
            # broadcast di over partitions: disb[q, n] = di_n
            for c in range(2):
                psd = ps512.tile([P, 512], F32, tag="ps512")
                nc.tensor.matmul(
                    psd, onesb, disrow[:, c * 512:(c + 1) * 512],
                    start=True, stop=True)
                nc.vector.tensor_copy(
                    out=disb[:, c * 512:(c + 1) * 512], in_=psd)
            # dis_tok[p, j] = di_{j*P+p} via PE transpose of disb chunks
            for j in range(NT):
                ptb = psTb.tile([P, P], BF16, tag="ptb")
                nc.tensor.transpose(ptb, disb[:, j * P:(j + 1) * P], identb)
                nc.vector.tensor_copy(out=dis_tok[:, j:j + 1], in_=ptb[:, 0:1])
            # fold both degree scalings into A^T (in place):
            # ats[m, n] = At[m, n] * di_m * di_n
            for j in range(NT):
                nc.vector.scalar_tensor_tensor(
                    out=ats[:, j, :], in0=ats[:, j, :],
                    scalar=dis_tok[:, j:j + 1], in1=disb,
                    op0=OP.mult, op1=OP.mult)

        # ---- phase C: shared GCN trunk + Q/K/V --------------------------
        # adjHnT[d, n] = sum_m hn[m, d] * adjn^T[m, n]
        for c in range(2):
            psc = ps512.tile([P, 512], F32, tag="ps512")
            for j in range(NT):
                nc.tensor.matmul(
                    psc, hn_t[:, j, :], ats[:, j, c * 512:(c + 1) * 512],
                    start=(j == 0), stop=(j == NT - 1))
            nc.vector.tensor_copy(out=adjHnT[:, c * 512:(c + 1) * 512], in_=psc)
        # V first so heads 0-3 unblock early
        for c in range(2):
            for i in range(NT):
                psc = ps512.tile([P, 512], F32, tag="ps512")
                nc.tensor.matmul(
                    psc, adjHnT[:, i * P:(i + 1) * P],
                    wv_sb[:, c * 512:(c + 1) * 512], start=True, stop=True)
                nc.vector.tensor_add(
                    out=vna[:, i, c * 4:(c + 1) * 4, 0:DV],
                    in0=psc.rearrange("p (a b) -> p a b", a=4),
                    in1=bvb[:, c * 512:(c + 1) * 512].rearrange(
                        "p (a b) -> p a b", a=4))
        for hh in range(HEADS):
            for c in range(2):
                psc = ps512.tile([P, 512], F32, tag="ps512")
                nc.tensor.matmul(
                    psc, wq_sb[:, hh * P:(hh + 1) * P],
                    adjHnT[:, c * 512:(c + 1) * 512], start=True, stop=True)
                nc.scalar.activation(
                    out=qT[:, hh, c * 512:(c + 1) * 512], in_=psc,
                    func=AF.Identity, bias=bqr[:, hh:hh + 1])
                psc = ps512.tile([P, 512], F32, tag="ps512")
                nc.tensor.matmul(
                    psc, wk_sb[:, hh * P:(hh + 1) * P],
                    adjHnT[:, c * 512:(c + 1) * 512], start=True, stop=True)
                nc.scalar.activation(
                    out=kT[:, hh, c * 512:(c + 1) * 512], in_=psc,
                    func=AF.Identity, bias=bkr[:, hh:hh + 1])

        # ---- phase D: attention per head --------------------------------
        with tc.tile_pool(name="psE", bufs=2, space=bass.MemorySpace.PSUM) as psE, \
             tc.tile_pool(name="psPM", bufs=2, space=bass.MemorySpace.PSUM) as psPM, \
             tc.tile_pool(name="etp", bufs=2) as etp, \
             tc.tile_pool(name="btp", bufs=2) as btp, \
             tc.tile_pool(name="mhp", bufs=2) as mhp:
            for hh in range(HEADS):
                btile = btp.tile([P, NT, N], F8, tag="bt")
                nc.sync.dma_start(
                    out=btile,
                    in_=bt_in[hh].rearrange("(t p) n -> p t n", p=P))
                et = etp.tile([P, NT, N], BF16, tag="et")
                for j in range(NT):
                    pse = psE.tile([P, N], F32, tag="pse")
                    for c in range(2):
                        nc.tensor.matmul(
                            pse[:, c * 512:(c + 1) * 512],
                            kT[:, hh, j * P:(j + 1) * P],
                            qT[:, hh, c * 512:(c + 1) * 512],
                            start=True, stop=False)
                        nc.tensor.matmul(
                            pse[:, c * 512:(c + 1) * 512],
                            id2, _dup2(btile[:, j, c * 512:(c + 1) * 512]),
                            start=False, stop=True, perf_mode=PM.DoubleRow)
                    nc.scalar.activation(out=et[:, j, :], in_=pse, func=AF.Exp)
                mh_head = mhp.tile([P, NT, DV], BF16, tag="mh")
                # PV in pairs of query tiles; LN rstd batched per pair.
                for i0 in range(0, NT, 2):
                    pms = []
                    mvs = small.tile([P, 2, 2], F32, tag="mvs", bufs=4)
                    vee = small.tile([P, 2], F32, tag="vee", bufs=4)
                    for k in range(2):
                        i = i0 + k
                        pm = psPM.tile([P, DV + 1], F32, tag="pm")
                        pms.append(pm)
                        for j in range(NT):
                            nc.tensor.matmul(
                                pm, et[:, j, i * P:(i + 1) * P],
                                vna[:, j, hh, :],
                                start=(j == 0), stop=(j == NT - 1))
                        s6 = small.tile([P, 6], F32, tag="s6")
                        nc.vector.bn_stats(out=s6, in_=pm[:, 0:DV])
                        nc.vector.bn_aggr(out=mvs[:, k, :], in_=s6)
                        # vee = var + eps * rowsum^2
                        t = small.tile([P, 1], F32, tag="t")
                        nc.vector.tensor_scalar(
                            out=t, in0=pm[:, DV:DV + 1],
                            scalar1=pm[:, DV:DV + 1], scalar2=EPS,
                            op0=OP.mult, op1=OP.mult)
                        nc.vector.tensor_add(
                            out=vee[:, k:k + 1], in0=t, in1=mvs[:, k, 1:2])
                    lnv2 = small.tile([P, 2], F32, tag="lnv2", bufs=4)
                    nc.scalar.activation(out=lnv2, in_=vee, func=AF.Ln)
                    rstd2 = small.tile([P, 2], F32, tag="rstd2", bufs=4)
                    nc.scalar.activation(out=rstd2, in_=lnv2, func=AF.Exp,
                                         scale=-0.5)
                    for k in range(2):
                        i = i0 + k
                        nc.vector.tensor_scalar(
                            out=mh_head[:, i, :], in0=pms[k][:, 0:DV],
                            scalar1=mvs[:, k, 0:1], scalar2=rstd2[:, k:k + 1],
                            op0=OP.subtract, op1=OP.mult)
                # transpose all 8 [128,128] tiles via DMA XBAR
                nc.sync.dma_start(
                    out=mhcT[:, hh, :].rearrange("p (t f) -> p t f", t=NT),
                    in_=mh_head, transpose=True)

        # ---- phase E: output projection + MLP ---------------------------
        o_ln = persist.tile([P, NT, D], BF16, tag="oln")
        with tc.tile_pool(name="psO", bufs=2, space=bass.MemorySpace.PSUM) as psO, \
             tc.tile_pool(name="psTe", bufs=2, space=bass.MemorySpace.PSUM) as psTe:
            oT = persist.tile([P, N], BF16, tag="oT")
            for i in range(NT):
                pso = psO.tile([P, D], F32, tag="pso")
                for hh in range(HEADS):
                    nc.tensor.matmul(
                        pso, mhcT[:, hh, i * P:(i + 1) * P], wo_sb[:, hh, :],
                        start=(hh == 0), stop=False)
                nc.tensor.matmul(pso, onesb, bo_sb, start=False, stop=True)
                orow = stg.tile([P, D], F32, name="orow", tag="orow")
                nc.vector.tensor_add(out=orow, in0=pso, in1=h_sb[:, i, :])
                s6 = small.tile([P, 6], F32, tag="s6")
                mv = small.tile([P, 2], F32, tag="mv")
                nc.vector.bn_stats(out=s6, in_=orow)
                nc.vector.bn_aggr(out=mv, in_=s6)
                lnv = small.tile([P, 1], F32, tag="lnv")
                nc.scalar.activation(out=lnv, in_=mv[:, 1:2], func=AF.Ln,
                                     bias=eps_t)
                rstd = small.tile([P, 1], F32, tag="rstd")
                nc.scalar.activation(out=rstd, in_=lnv, func=AF.Exp, scale=-0.5)
                nc.vector.tensor_scalar(
                    out=o_ln[:, i, :], in0=orow,
                    scalar1=mv[:, 0:1], scalar2=rstd,
                    op0=OP.subtract, op1=OP.mult)
                nc.vector.tensor_mul(out=o_ln[:, i, :], in0=o_ln[:, i, :],
                                     in1=gb[2])
                nc.vector.tensor_add(out=o_ln[:, i, :], in0=o_ln[:, i, :],
                                     in1=beb[2])
                ptb = psTe.tile([P, P], BF16, tag="pte")
                nc.tensor.transpose(ptb, o_ln[:, i, :], identb)
                nc.vector.tensor_copy(out=oT[:, i * P:(i + 1) * P], in_=ptb)

            r1T = persist.tile([P, N], BF16, tag="r1T")
            for c in range(2):
                psc = ps512.tile([P, 512], F32, tag="ps512")
                nc.tensor.matmul(
                    psc, w1_sb, oT[:, c * 512:(c + 1) * 512],
                    start=True, stop=True)
                nc.scalar.activation(
                    out=r1T[:, c * 512:(c + 1) * 512], in_=psc, func=AF.Relu,
                    bias=b1_sb)
            r2T = persist.tile([P, N], BF16, tag="r2T")
            for c in range(2):
                psc = ps512.tile([P, 512], F32, tag="ps512")
                nc.tensor.matmul(
                    psc, w2_sb, r1T[:, c * 512:(c + 1) * 512],
                    start=True, stop=True)
                nc.scalar.activation(
                    out=r2T[:, c * 512:(c + 1) * 512], in_=psc, func=AF.Relu,
                    bias=b2_sb)

            out_sb = persist.tile([P, NT, D], F32, tag="osb")
            for i in range(NT):
                ptb = psTe.tile([P, P], BF16, tag="pte")
                nc.tensor.transpose(ptb, r2T[:, i * P:(i + 1) * P], identb)
                r2t = stg.tile([P, D], BF16, name="r2t", tag="r2t")
                nc.vector.tensor_copy(out=r2t, in_=ptb)
                s6 = small.tile([P, 6], F32, tag="s6")
                mv = small.tile([P, 2], F32, tag="mv")
                nc.vector.bn_stats(out=s6, in_=r2t)
                nc.vector.bn_aggr(out=mv, in_=s6)
                lnv = small.tile([P, 1], F32, tag="lnv")
                nc.scalar.activation(out=lnv, in_=mv[:, 1:2], func=AF.Ln,
                                     bias=eps_t)
                rstd = small.tile([P, 1], F32, tag="rstd")
                nc.scalar.activation(out=rstd, in_=lnv, func=AF.Exp, scale=-0.5)
                ro = stg.tile([P, D], BF16, name="ro", tag="ro")
                nc.vector.tensor_scalar(
                    out=ro, in0=r2t, scalar1=mv[:, 0:1],
                    scalar2=rstd, op0=OP.subtract, op1=OP.mult)
                nc.vector.tensor_mul(out=ro, in0=ro, in1=gb[3])
                nc.vector.tensor_add(out=ro, in0=ro, in1=beb[3])
                nc.vector.tensor_add(out=out_sb[:, i, :], in0=o_ln[:, i, :],
                                     in1=ro)
                nc.sync.dma_start(
                    out=out_dram.rearrange("(t p) d -> p t d", p=P)[:, i, :],
                    in_=out_sb[:, i, :])

    nc.compile()
    return nc


def _get_program():
    if "nc" not in _prog_cache:
        _prog_cache["nc"] = _build_program()
    return _prog_cache["nc"]


def kernel(**inputs):
    nc = _get_program()
    f32 = np.float32
    bf16 = ml_dtypes.bfloat16
    f8 = ml_dtypes.float8_e4m3fn

    H = np.asarray(inputs["H"], dtype=f32)
    A = np.asarray(inputs["A"], dtype=f32)
    g1 = np.asarray(inputs["g1"], dtype=f32)
    be1 = np.asarray(inputs["be1"], dtype=f32)
    WO = np.asarray(inputs["W_O"], dtype=f32)
    # fold LN1 gain/bias into the output projection
    WO_fold = WO * np.tile(g1, HEADS)[:, None]
    bO = np.tile(be1, HEADS) @ WO

    BT = np.asarray(inputs["B_bias"], dtype=f32).transpose(0, 2, 1)
    base = {
        "bt": np.ascontiguousarray(BT * 16.0).astype(f8),
        "wq": (np.asarray(inputs["W_Q"], dtype=f32) * DK).astype(bf16),
        "wk": np.asarray(inputs["W_K"], dtype=f32).astype(bf16),
        "wv": np.asarray(inputs["W_V"], dtype=f32).astype(bf16),
        "bqr": np.ascontiguousarray(
            (np.asarray(inputs["b_Q"], dtype=f32) * DK).reshape(NT, P).T),
        "bkr": np.ascontiguousarray(
            np.asarray(inputs["b_K"], dtype=f32).reshape(NT, P).T),
        "bv": np.asarray(inputs["b_V"], dtype=f32),
        "wo": WO_fold.astype(bf16),
        "bo": bO.reshape(1, D).astype(bf16),
        "w1": np.asarray(inputs["W1"], dtype=f32).astype(bf16),
        "w2": np.asarray(inputs["W2"], dtype=f32).astype(bf16),
        "b1": np.asarray(inputs["b1"], dtype=f32).reshape(D, 1),
        "b2": np.asarray(inputs["b2"], dtype=f32).reshape(D, 1),
    }
    for i in (0, 2, 3):
        base[f"g{i}"] = np.asarray(inputs[f"g{i}"], dtype=f32).astype(bf16)
        base[f"be{i}"] = np.asarray(inputs[f"be{i}"], dtype=f32).astype(bf16)

    in_maps = []
    for c in range(B):
        m = dict(base)
        m["h"] = H[c].astype(bf16)
        m["at"] = np.ascontiguousarray(A[c].T).astype(bf16)
        in_maps.append(m)

    res = run_bass_kernel_spmd(nc, in_maps, list(range(B)))
    out = np.stack([res.results[c]["out"] for c in range(B)], axis=0)
    return out.astype(np.float32)


if __name__ == "__main__":
    nc = _get_program()
    print("program built ok")
    from concourse.timeline_sim import TimelineSim
    ns = TimelineSim(nc, trace=False).simulate()
    print(f"TimelineSim: {ns:.0f} ns")


# revision 8
# speedup vs baseline: 2.6795x; 1.3370x over previous
"""Trainium2 Bass kernel for nn_GRIC_31550829756424 (GCN-attention block).

Data-parallel over batch: 8 batches -> 8 NeuronCores, one full batch per core.

Key structure (v2):
- GCN reassociated: adjHnT = Hn^T @ adj_norm^T computed once (shared by
  Q/K/V), then Q/K/V are single-step K=128 matmuls.  A is host-transposed
  and bf16; both degree scalings fold into A^T via one scalar_tensor_tensor.
- All rsqrt computed as exp(-0.5*ln(x)) so the ONLY activation table used is
  natural_log_exp_and_others (Exp/Ln/Relu/Copy/Identity) -> 1 table load.
- Attention bias B added into the QK PSUM via an fp8 DoubleRow matmul
  (identity/32 stationary, bias*16 moving) instead of DVE adds.
- LN1 gain/bias folded into W_O on the host; MH transposed via DMA XBAR.

Self-contained: hardcodes all shapes; imports only the in-container concourse
stack.
"""

import sys

sys.path.insert(0, "/opt/trn_rl_repo")

import numpy as np
import ml_dtypes
from contextlib import ExitStack

import concourse.bass as bass
import concourse.tile as tile
from concourse import bacc
from concourse import mybir
from concourse.bass_utils import run_bass_kernel_spmd
from concourse.masks import make_identity

F32 = mybir.dt.float32
BF16 = mybir.dt.bfloat16
F8 = mybir.dt.float8e4
AF = mybir.ActivationFunctionType
OP = mybir.AluOpType
PM = mybir.MatmulPerfMode

B = 8
N = 1024
D = 128
HEADS = 8
DV = 128
HD = HEADS * DV  # 1024
P = 128
NT = N // P  # 8 tiles of 128 rows
DK = 1.0 / float(np.sqrt(np.float32(D)))
EPS = 1e-5

_prog_cache = {}


def _bcast_load(nc, dst, src):
    """DMA-load 1D DRAM vector src [W] replicated across all P partitions of
    dst [P, W] (same dtype)."""
    rep = bass.AP(tensor=src.tensor, offset=src.offset, ap=[[0, P]] + list(src.ap))
    nc.gpsimd.dma_start(out=dst, in_=rep)


def _dup2(ap):
    """View a [P, W] AP as [P, 2, W] with a stride-0 middle dim (DoubleRow
    moving operand reading the same data in both slots)."""
    return bass.AP(
        tensor=ap.tensor, offset=ap.offset,
        ap=[list(ap.ap[0]), [0, 2]] + [list(a) for a in ap.ap[1:]],
    )


def _build_program():
    nc = bacc.Bacc(None)

    h_in = nc.declare_dram_parameter("h", [N, D], BF16, isOutput=False)
    at_in = nc.declare_dram_parameter("at", [N, N], BF16, isOutput=False)
    bt_in = nc.declare_dram_parameter("bt", [HEADS, N, N], F8, isOutput=False)
    wq_in = nc.declare_dram_parameter("wq", [D, HD], BF16, isOutput=False)
    wk_in = nc.declare_dram_parameter("wk", [D, HD], BF16, isOutput=False)
    wv_in = nc.declare_dram_parameter("wv", [D, HD], BF16, isOutput=False)
    bqr_in = nc.declare_dram_parameter("bqr", [P, NT], F32, isOutput=False)
    bkr_in = nc.declare_dram_parameter("bkr", [P, NT], F32, isOutput=False)
    bv_in = nc.declare_dram_parameter("bv", [HD], F32, isOutput=False)
    wo_in = nc.declare_dram_parameter("wo", [HD, D], BF16, isOutput=False)
    bo_in = nc.declare_dram_parameter("bo", [1, D], BF16, isOutput=False)
    w1_in = nc.declare_dram_parameter("w1", [D, D], BF16, isOutput=False)
    w2_in = nc.declare_dram_parameter("w2", [D, D], BF16, isOutput=False)
    b1_in = nc.declare_dram_parameter("b1", [D, 1], F32, isOutput=False)
    b2_in = nc.declare_dram_parameter("b2", [D, 1], F32, isOutput=False)
    g_in = {}
    be_in = {}
    for i in (0, 2, 3):
        g_in[i] = nc.declare_dram_parameter(f"g{i}", [D], BF16, isOutput=False)
        be_in[i] = nc.declare_dram_parameter(f"be{i}", [D], BF16, isOutput=False)
    out_dram = nc.declare_dram_parameter("out", [N, D], F32, isOutput=True)

    with tile.TileContext(nc) as tc, ExitStack() as ctx:
        consts = ctx.enter_context(tc.tile_pool(name="consts", bufs=1))
        persist = ctx.enter_context(tc.tile_pool(name="persist", bufs=1))
        small = ctx.enter_context(tc.tile_pool(name="small", bufs=12))
        stg = ctx.enter_context(tc.tile_pool(name="stg", bufs=3))
        # 2-bank 512-wide PSUM pool: open through B/C, reused in E via scope.
        ps512 = ctx.enter_context(
            tc.tile_pool(name="ps512", bufs=2, space=bass.MemorySpace.PSUM))

        # ---- constants -------------------------------------------------
        identb = consts.tile([P, P], BF16)
        make_identity(nc, identb)
        omib = consts.tile([P, P], BF16)  # 1 - I
        nc.gpsimd.memset(omib, 1.0)
        nc.gpsimd.affine_select(
            out=omib, in_=omib, compare_op=OP.not_equal, fill=0.0,
            base=0, pattern=[[-1, P]], channel_multiplier=1)
        # fp8 DoubleRow stationary: two slots of I/32 (moving is bias*16).
        id2 = consts.tile([P, 2, P], F8)
        nc.gpsimd.memset(id2, 0.0)
        for s in range(2):
            nc.gpsimd.affine_select(
                out=id2[:, s, :], in_=id2[:, s, :], compare_op=OP.not_equal,
                fill=1.0 / 32.0, base=0, pattern=[[-1, P]], channel_multiplier=1)
        nc.scalar.add_instruction(
            mybir.InstLoadActFuncSet(
                name=nc.get_next_instruction_name(), ins=[], outs=[],
                act_func_set_id=6))
        eps_t = consts.tile([P, 1], F32)
        nc.vector.memset(eps_t, EPS)
        onesb = consts.tile([1, P], BF16)
        nc.vector.memset(onesb, 1.0)
        onescol = consts.tile([P, 1], BF16)
        nc.vector.memset(onescol, 1.0)

        gb = {}
        beb = {}
        for i in (0, 2, 3):
            gb[i] = consts.tile([P, D], BF16, name=f"g{i}b", tag=f"g{i}b")
            _bcast_load(nc, gb[i], g_in[i][:])
            beb[i] = consts.tile([P, D], BF16, name=f"be{i}b", tag=f"be{i}b")
            _bcast_load(nc, beb[i], be_in[i][:])
        bvb = consts.tile([P, HD], F32)
        _bcast_load(nc, bvb, bv_in[:])
        bqr = consts.tile([P, NT], F32)
        nc.gpsimd.dma_start(out=bqr, in_=bqr_in[:, :])
        bkr = consts.tile([P, NT], F32)
        nc.gpsimd.dma_start(out=bkr, in_=bkr_in[:, :])
        wq_sb = consts.tile([P, HD], BF16)
        nc.gpsimd.dma_start(out=wq_sb, in_=wq_in[:, :])
        wk_sb = consts.tile([P, HD], BF16)
        nc.gpsimd.dma_start(out=wk_sb, in_=wk_in[:, :])
        wv_sb = consts.tile([P, HD], BF16)
        nc.gpsimd.dma_start(out=wv_sb, in_=wv_in[:, :])
        # W_O as [p=dv-within-head, h, d]
        wo_sb = consts.tile([P, HEADS, D], BF16)
        nc.sync.dma_start(
            out=wo_sb, in_=wo_in.rearrange("(hh p) d -> p hh d", p=P))
        bo_sb = consts.tile([1, D], BF16)
        nc.gpsimd.dma_start(out=bo_sb, in_=bo_in[:, :])
        w1_sb = consts.tile([P, D], BF16)
        nc.gpsimd.dma_start(out=w1_sb, in_=w1_in[:, :])
        w2_sb = consts.tile([P, D], BF16)
        nc.gpsimd.dma_start(out=w2_sb, in_=w2_in[:, :])
        b1_sb = consts.tile([P, 1], F32)
        nc.gpsimd.dma_start(out=b1_sb, in_=b1_in[:, :])
        b2_sb = consts.tile([P, 1], F32)
        nc.gpsimd.dma_start(out=b2_sb, in_=b2_in[:, :])

        # ---- persistent tensors ---------------------------------------
        h_sb = persist.tile([P, NT, D], BF16, tag="h")
        nc.sync.dma_start(out=h_sb, in_=h_in.rearrange("(t p) d -> p t d", p=P))
        hn_t = persist.tile([P, NT, D], BF16, tag="hnt")
        ats = persist.tile([P, NT, N], BF16, tag="ats")  # A^T -> adj_norm^T
        disb = persist.tile([P, N], BF16, tag="disb")
        dis_tok = persist.tile([P, NT], F32, tag="distok")
        adjHnT = persist.tile([P, N], BF16, tag="adjhnt")
        qT = persist.tile([P, HEADS, N], BF16, tag="qT")
        kT = persist.tile([P, HEADS, N], BF16, tag="kT")
        vna = persist.tile([P, NT, HEADS, DV + 1], BF16, tag="v")
        nc.vector.memset(vna[:, :, :, DV:DV + 1], 1.0)
        mhcT = persist.tile([P, HEADS, N], BF16, tag="mhcT")

        # ---- phase A: H LayerNorm (token-major) ------------------------
        for i in range(NT):
            s6 = small.tile([P, 6], F32, tag="s6")
            mv = small.tile([P, 2], F32, tag="mv")
            nc.vector.bn_stats(out=s6, in_=h_sb[:, i, :])
            nc.vector.bn_aggr(out=mv, in_=s6)
            lnv = small.tile([P, 1], F32, tag="lnv")
            nc.scalar.activation(out=lnv, in_=mv[:, 1:2], func=AF.Ln, bias=eps_t)
            rstd = small.tile([P, 1], F32, tag="rstd")
            nc.scalar.activation(out=rstd, in_=lnv, func=AF.Exp, scale=-0.5)
            hw = stg.tile([P, D], BF16, name="hw", tag="hw")
            nc.vector.tensor_scalar(
                out=hw, in0=h_sb[:, i, :], scalar1=mv[:, 0:1], scalar2=rstd,
                op0=OP.subtract, op1=OP.mult)
            nc.vector.tensor_mul(out=hw, in0=hw, in1=gb[0])
            nc.vector.tensor_add(out=hn_t[:, i, :], in0=hw, in1=beb[0])

        # ---- phase B: adjacency prep ------------------------------------
        with tc.tile_pool(name="psR", bufs=1, space=bass.MemorySpace.PSUM) as psR, \
             tc.tile_pool(name="psTb", bufs=2, space=bass.MemorySpace.PSUM) as psTb:
            rs_ps = [psR.tile([1, 512], F32, name=f"rsps{c}", tag=f"rsps{c}")
                     for c in range(2)]
            for j in range(NT):
                nc.sync.dma_start(
                    out=ats[:, j, :],
                    in_=at_in[:, :].rearrange("(t p) n -> p t n", p=P)[:, j, :])
                db = ats[:, j, j * P:(j + 1) * P]
                nc.vector.tensor_mul(out=db, in0=db, in1=omib)
                nc.vector.tensor_add(out=db, in0=db, in1=identb)
                for c in range(2):
                    nc.tensor.matmul(
                        rs_ps[c], onescol, ats[:, j, c * 512:(c + 1) * 512],
                        start=(j == 0), stop=(j == NT - 1))
            rs_sb = small.tile([1, N], F32, tag="rssb", bufs=1)
            for c in range(2):
                nc.vector.tensor_scalar_max(
                    out=rs_sb[:, c * 512:(c + 1) * 512], in0=rs_ps[c], scalar1=1.0)
            lnr = small.tile([1, N], F32, tag="lnr", bufs=1)
            nc.scalar.activation(out=lnr, in_=rs_sb, func=AF.Ln)
            disrow = small.tile([1, N], BF16, tag="disrow", bufs=1)
            nc.scalar.activation(out=disrow, in_=lnr, func=AF.Exp, scale=-0.5)
            # broadcast di over partitions: disb[q, n] = di_n
            for c in range(2):
                psd = ps512.tile([P, 512], F32, tag="ps512")
                nc.tensor.matmul(
                    psd, onesb, disrow[:, c * 512:(c + 1) * 512],
                    start=True, stop=True)
                nc.vector.tensor_copy(
                    out=disb[:, c * 512:(c + 1) * 512], in_=psd)
            # dis_tok[p, j] = di_{j*P+p} via PE transpose of disb chunks
            for j in range(NT):
                ptb = psTb.tile([P, P], BF16, tag="ptb")
                nc.tensor.transpose(ptb, disb[:, j * P:(j + 1) * P], identb)
                nc.vector.tensor_copy(out=dis_tok[:, j:j + 1], in_=ptb[:, 0:1])
            # fold both degree scalings into A^T (in place):
            # ats[m, n] = At[m, n] * di_m * di_n
            for j in range(NT):
                nc.vector.scalar_tensor_tensor(
                    out=ats[:, j, :], in0=ats[:, j, :],
                    scalar=dis_tok[:, j:j + 1], in1=disb,
                    op0=OP.mult, op1=OP.mult)

        # ---- phase C: shared GCN trunk ----------------------------------
        # adjHnT[d, n] = sum_m hn[m, d] * adjn^T[m, n]
        for c in range(2):
            psc = ps512.tile([P, 512], F32, tag="ps512")
            for j in range(NT):
                nc.tensor.matmul(
                    psc, hn_t[:, j, :], ats[:, j, c * 512:(c + 1) * 512],
                    start=(j == 0), stop=(j == NT - 1))
            nc.vector.tensor_copy(out=adjHnT[:, c * 512:(c + 1) * 512], in_=psc)

        def emit_qk(hh):
            for c in range(2):
                psc = ps512.tile([P, 512], F32, tag="ps512")
                nc.tensor.matmul(
                    psc, wq_sb[:, hh * P:(hh + 1) * P],
                    adjHnT[:, c * 512:(c + 1) * 512], start=True, stop=True)
                nc.vector.tensor_scalar_add(
                    out=qT[:, hh, c * 512:(c + 1) * 512], in0=psc,
                    scalar1=bqr[:, hh:hh + 1])
                psc = ps512.tile([P, 512], F32, tag="ps512")
                nc.tensor.matmul(
                    psc, wk_sb[:, hh * P:(hh + 1) * P],
                    adjHnT[:, c * 512:(c + 1) * 512], start=True, stop=True)
                nc.vector.tensor_scalar_add(
                    out=kT[:, hh, c * 512:(c + 1) * 512], in0=psc,
                    scalar1=bkr[:, hh:hh + 1])

        emit_qk(0)
        # V after head-0 Q/K so attention stage 0 can start immediately
        for c in range(2):
            for i in range(NT):
                psc = ps512.tile([P, 512], F32, tag="ps512")
                nc.tensor.matmul(
                    psc, adjHnT[:, i * P:(i + 1) * P],
                    wv_sb[:, c * 512:(c + 1) * 512], start=True, stop=True)
                nc.vector.tensor_add(
                    out=vna[:, i, c * 4:(c + 1) * 4, 0:DV],
                    in0=psc.rearrange("p (a b) -> p a b", a=4),
                    in1=bvb[:, c * 512:(c + 1) * 512].rearrange(
                        "p (a b) -> p a b", a=4))

        # ---- phase D: attention, software-pipelined over heads ----------
        with tc.tile_pool(name="psE", bufs=2, space=bass.MemorySpace.PSUM) as psE, \
             tc.tile_pool(name="psPM", bufs=2, space=bass.MemorySpace.PSUM) as psPM, \
             tc.tile_pool(name="etp", bufs=2) as etp, \
             tc.tile_pool(name="btp", bufs=2) as btp, \
             tc.tile_pool(name="mhp", bufs=2) as mhp:
            ets = {}
            mhs = {}
            mvss = {}
            vees = {}
            bt0 = btp.tile([P, NT, N], F8, tag="bt", name="bt0")
            nc.sync.dma_start(
                out=bt0, in_=bt_in[0].rearrange("(t p) n -> p t n", p=P))
            bts = {0: bt0}
            for stage in range(HEADS + 1):
                hh = stage
                if hh < HEADS:
                    if hh + 1 < HEADS:
                        btn = btp.tile([P, NT, N], F8, tag="bt",
                                       name=f"bt{hh + 1}")
                        nc.sync.dma_start(
                            out=btn,
                            in_=bt_in[hh + 1].rearrange("(t p) n -> p t n", p=P))
                        bts[hh + 1] = btn
                    if hh + 1 < HEADS:
                        emit_qk(hh + 1)
                    et = etp.tile([P, NT, N], BF16, tag="et")
                    ets[hh] = et
                    btile = bts[hh]
                    for j in range(NT):
                        pse = psE.tile([P, N], F32, tag="pse")
                        for c in range(2):
                            nc.tensor.matmul(
                                pse[:, c * 512:(c + 1) * 512],
                                kT[:, hh, j * P:(j + 1) * P],
                                qT[:, hh, c * 512:(c + 1) * 512],
                                start=True, stop=False)
                            nc.tensor.matmul(
                                pse[:, c * 512:(c + 1) * 512],
                                id2, _dup2(btile[:, j, c * 512:(c + 1) * 512]),
                                start=False, stop=True, perf_mode=PM.DoubleRow)
                        nc.scalar.activation(out=et[:, j, :], in_=pse,
                                             func=AF.Exp)
                if stage >= 1:
                    ph = stage - 1  # head whose PV/LN we process now
                    et = ets.pop(ph)
                    mh_sub = mhp.tile([P, NT, DV], BF16, tag="mh")
                    mhs[ph] = mh_sub
                    mvs = small.tile([P, NT, 2], F32, tag="mvs", bufs=2)
                    mvss[ph] = mvs
                    vee = small.tile([P, NT], F32, tag="vee", bufs=2)
                    vees[ph] = vee
                    for i in range(NT):
                        pm = psPM.tile([P, DV + 1], F32, tag="pm")
                        for j in range(NT):
                            nc.tensor.matmul(
                                pm, et[:, j, i * P:(i + 1) * P],
                                vna[:, j, ph, :],
                                start=(j == 0), stop=(j == NT - 1))
                        s6 = small.tile([P, 6], F32, tag="s6")
                        nc.vector.bn_stats(out=s6, in_=pm[:, 0:DV])
                        nc.vector.bn_aggr(out=mvs[:, i, :], in_=s6)
                        t = small.tile([P, 1], F32, tag="t")
                        nc.vector.tensor_scalar(
                            out=t, in0=pm[:, DV:DV + 1],
                            scalar1=pm[:, DV:DV + 1], scalar2=EPS,
                            op0=OP.mult, op1=OP.mult)
                        # mean-subtract now (frees pm); rstd scale later
                        nc.vector.tensor_scalar(
                            out=mh_sub[:, i, :], in0=pm[:, 0:DV],
                            scalar1=mvs[:, i, 0:1], scalar2=None,
                            op0=OP.subtract)
                        nc.gpsimd.tensor_add(
                            out=vee[:, i:i + 1], in0=t, in1=mvs[:, i, 1:2])
                    # batched rstd for the whole head (after next head's Exps
                    # in ACT program order -> no convoy)
                    lnv8 = small.tile([P, NT], F32, tag="lnv8", bufs=2)
                    nc.scalar.activation(out=lnv8, in_=vee, func=AF.Ln)
                    rstd8 = small.tile([P, NT], F32, tag="rstd8", bufs=2)
                    nc.scalar.activation(out=rstd8, in_=lnv8, func=AF.Exp,
                                         scale=-0.5)
                    for i in range(NT):
                        nc.vector.tensor_scalar_mul(
                            out=mh_sub[:, i, :], in0=mh_sub[:, i, :],
                            scalar1=rstd8[:, i:i + 1])
                    nc.sync.dma_start(
                        out=mhcT[:, ph, :].rearrange("p (t f) -> p t f", t=NT),
                        in_=mh_sub, transpose=True)

        # ---- phase E: output projection + MLP ---------------------------
        o_ln = persist.tile([P, NT, D], BF16, tag="oln")
        orows = persist.tile([P, NT, D], F32, tag="orows")
        mvE = small.tile([P, NT, 2], F32, tag="mvE", bufs=1)
        rstdE = small.tile([P, NT], F32, tag="rstdE", bufs=1)
        with tc.tile_pool(name="psO", bufs=2, space=bass.MemorySpace.PSUM) as psO, \
             tc.tile_pool(name="psTe", bufs=2, space=bass.MemorySpace.PSUM) as psTe:
            oT = persist.tile([P, N], BF16, tag="oT")
            for i in range(NT):
                pso = psO.tile([P, D], F32, tag="pso")
                for hh in range(HEADS):
                    nc.tensor.matmul(
                        pso, mhcT[:, hh, i * P:(i + 1) * P], wo_sb[:, hh, :],
                        start=(hh == 0), stop=False)
                nc.tensor.matmul(pso, onesb, bo_sb, start=False, stop=True)
                nc.vector.tensor_add(
                    out=orows[:, i, :], in0=pso, in1=h_sb[:, i, :])
                s6 = small.tile([P, 6], F32, tag="s6")
                nc.vector.bn_stats(out=s6, in_=orows[:, i, :])
                nc.vector.bn_aggr(out=mvE[:, i, :], in_=s6)
                lnv = small.tile([P, 1], F32, tag="lnv")
                nc.scalar.activation(out=lnv, in_=mvE[:, i, 1:2], func=AF.Ln,
                                     bias=eps_t)
                nc.scalar.activation(out=rstdE[:, i:i + 1], in_=lnv,
                                     func=AF.Exp, scale=-0.5)
            for i in range(NT):
                nc.vector.tensor_scalar(
                    out=o_ln[:, i, :], in0=orows[:, i, :],
                    scalar1=mvE[:, i, 0:1], scalar2=rstdE[:, i:i + 1],
                    op0=OP.subtract, op1=OP.mult)
                nc.vector.tensor_mul(out=o_ln[:, i, :], in0=o_ln[:, i, :],
                                     in1=gb[2])
                nc.vector.tensor_add(out=o_ln[:, i, :], in0=o_ln[:, i, :],
                                     in1=beb[2])
                ptb = psTe.tile([P, P], BF16, tag="pte")
                nc.tensor.transpose(ptb, o_ln[:, i, :], identb)
                nc.vector.tensor_copy(out=oT[:, i * P:(i + 1) * P], in_=ptb)

            r1T = persist.tile([P, N], BF16, tag="r1T")
            for c in range(2):
                psc = ps512.tile([P, 512], F32, tag="ps512")
                nc.tensor.matmul(
                    psc, w1_sb, oT[:, c * 512:(c + 1) * 512],
                    start=True, stop=True)
                nc.scalar.activation(
                    out=r1T[:, c * 512:(c + 1) * 512], in_=psc, func=AF.Relu,
                    bias=b1_sb)
            r2T = persist.tile([P, N], BF16, tag="r2T")
            for c in range(2):
                psc = ps512.tile([P, 512], F32, tag="ps512")
                nc.tensor.matmul(
                    psc, w2_sb, r1T[:, c * 512:(c + 1) * 512],
                    start=True, stop=True)
                nc.scalar.activation(
                    out=r2T[:, c * 512:(c + 1) * 512], in_=psc, func=AF.Relu,
                    bias=b2_sb)

            r2tok = persist.tile([P, NT, D], BF16, tag="r2tok")
            mvR = small.tile([P, NT, 2], F32, tag="mvR", bufs=1)
            rstdR = small.tile([P, NT], F32, tag="rstdR", bufs=1)
            out_sb = persist.tile([P, NT, D], F32, tag="osb")
            for i in range(NT):
                ptb = psTe.tile([P, P], BF16, tag="pte")
                nc.tensor.transpose(ptb, r2T[:, i * P:(i + 1) * P], identb)
                nc.vector.tensor_copy(out=r2tok[:, i, :], in_=ptb)
                s6 = small.tile([P, 6], F32, tag="s6")
                nc.vector.bn_stats(out=s6, in_=r2tok[:, i, :])
                nc.vector.bn_aggr(out=mvR[:, i, :], in_=s6)
                lnv = small.tile([P, 1], F32, tag="lnv")
                nc.scalar.activation(out=lnv, in_=mvR[:, i, 1:2], func=AF.Ln,
                                     bias=eps_t)
                nc.scalar.activation(out=rstdR[:, i:i + 1], in_=lnv,
                                     func=AF.Exp, scale=-0.5)
            for i in range(NT):
                ro = stg.tile([P, D], BF16, name="ro", tag="ro")
                nc.vector.tensor_scalar(
                    out=ro, in0=r2tok[:, i, :], scalar1=mvR[:, i, 0:1],
                    scalar2=rstdR[:, i:i + 1], op0=OP.subtract, op1=OP.mult)
                nc.vector.tensor_mul(out=ro, in0=ro, in1=gb[3])
                nc.vector.tensor_add(out=ro, in0=ro, in1=beb[3])
                nc.vector.tensor_add(out=out_sb[:, i, :], in0=o_ln[:, i, :],
                                     in1=ro)
                nc.sync.dma_start(
                    out=out_dram.rearrange("(t p) d -> p t d", p=P)[:, i, :],
                    in_=out_sb[:, i, :])

    nc.compile()
    return nc


def _get_program():
    if "nc" not in _prog_cache:
        _prog_cache["nc"] = _build_program()
    return _prog_cache["nc"]


def kernel(**inputs):
    nc = _get_program()
    f32 = np.float32
    bf16 = ml_dtypes.bfloat16
    f8 = ml_dtypes.float8_e4m3fn

    H = np.asarray(inputs["H"], dtype=f32)
    A = np.asarray(inputs["A"], dtype=f32)
    g1 = np.asarray(inputs["g1"], dtype=f32)
    be1 = np.asarray(inputs["be1"], dtype=f32)
    WO = np.asarray(inputs["W_O"], dtype=f32)
    # fold LN1 gain/bias into the output projection
    WO_fold = WO * np.tile(g1, HEADS)[:, None]
    bO = np.tile(be1, HEADS) @ WO

    BT = np.asarray(inputs["B_bias"], dtype=f32).transpose(0, 2, 1)
    base = {
        "bt": np.ascontiguousarray(BT * 16.0).astype(f8),
        "wq": (np.asarray(inputs["W_Q"], dtype=f32) * DK).astype(bf16),
        "wk": np.asarray(inputs["W_K"], dtype=f32).astype(bf16),
        "wv": np.asarray(inputs["W_V"], dtype=f32).astype(bf16),
        "bqr": np.ascontiguousarray(
            (np.asarray(inputs["b_Q"], dtype=f32) * DK).reshape(NT, P).T),
        "bkr": np.ascontiguousarray(
            np.asarray(inputs["b_K"], dtype=f32).reshape(NT, P).T),
        "bv": np.asarray(inputs["b_V"], dtype=f32),
        "wo": WO_fold.astype(bf16),
        "bo": bO.reshape(1, D).astype(bf16),
        "w1": np.asarray(inputs["W1"], dtype=f32).astype(bf16),
        "w2": np.asarray(inputs["W2"], dtype=f32).astype(bf16),
        "b1": np.asarray(inputs["b1"], dtype=f32).reshape(D, 1),
        "b2": np.asarray(inputs["b2"], dtype=f32).reshape(D, 1),
    }
    for i in (0, 2, 3):
        base[f"g{i}"] = np.asarray(inputs[f"g{i}"], dtype=f32).astype(bf16)
        base[f"be{i}"] = np.asarray(inputs[f"be{i}"], dtype=f32).astype(bf16)

    in_maps = []
    for c in range(B):
        m = dict(base)
        m["h"] = H[c].astype(bf16)
        m["at"] = np.ascontiguousarray(A[c].T).astype(bf16)
        in_maps.append(m)

    res = run_bass_kernel_spmd(nc, in_maps, list(range(B)))
    out = np.stack([res.results[c]["out"] for c in range(B)], axis=0)
    return out.astype(np.float32)


if __name__ == "__main__":
    nc = _get_program()
    print("program built ok")
    from concourse.timeline_sim import TimelineSim
    ns = TimelineSim(nc, trace=False).simulate()
    print(f"TimelineSim: {ns:.0f} ns")


# revision 20
# speedup vs baseline: 2.8002x; 1.0450x over previous
"""Trainium2 Bass kernel for nn_GRIC_31550829756424 (GCN-attention block).

Data-parallel over batch: 8 batches -> 8 NeuronCores, one full batch per core.

Key structure (v2):
- GCN reassociated: adjHnT = Hn^T @ adj_norm^T computed once (shared by
  Q/K/V), then Q/K/V are single-step K=128 matmuls.  A is host-transposed
  and bf16; both degree scalings fold into A^T via one scalar_tensor_tensor.
- All rsqrt computed as exp(-0.5*ln(x)) so the ONLY activation table used is
  natural_log_exp_and_others (Exp/Ln/Relu/Copy/Identity) -> 1 table load.
- Attention bias B added into the QK PSUM via an fp8 DoubleRow matmul
  (identity/32 stationary, bias*16 moving) instead of DVE adds.
- LN1 gain/bias folded into W_O on the host; MH transposed via DMA XBAR.

Self-contained: hardcodes all shapes; imports only the in-container concourse
stack.
"""

import sys

sys.path.insert(0, "/opt/trn_rl_repo")

import numpy as np
import ml_dtypes
from contextlib import ExitStack

import concourse.bass as bass
import concourse.tile as tile
from concourse import bacc
from concourse import mybir
from concourse.bass_utils import run_bass_kernel_spmd
from concourse.masks import make_identity

F32 = mybir.dt.float32
BF16 = mybir.dt.bfloat16
F8 = mybir.dt.float8e4
AF = mybir.ActivationFunctionType
OP = mybir.AluOpType
PM = mybir.MatmulPerfMode

B = 8
N = 1024
D = 128
HEADS = 8
DV = 128
HD = HEADS * DV  # 1024
P = 128
NT = N // P  # 8 tiles of 128 rows
DK = 1.0 / float(np.sqrt(np.float32(D)))
EPS = 1e-5

_prog_cache = {}


def _bcast_load(nc, dst, src):
    """DMA-load 1D DRAM vector src [W] replicated across all P partitions of
    dst [P, W] (same dtype)."""
    rep = bass.AP(tensor=src.tensor, offset=src.offset, ap=[[0, P]] + list(src.ap))
    nc.gpsimd.dma_start(out=dst, in_=rep)


def _dup2(ap):
    """View a [P, W] AP as [P, 2, W] with a stride-0 middle dim (DoubleRow
    moving operand reading the same data in both slots)."""
    return bass.AP(
        tensor=ap.tensor, offset=ap.offset,
        ap=[list(ap.ap[0]), [0, 2]] + [list(a) for a in ap.ap[1:]],
    )


def _build_program():
    nc = bacc.Bacc(None)

    h_in = nc.declare_dram_parameter("h", [N, D], BF16, isOutput=False)
    at_in = nc.declare_dram_parameter("at", [N, N], BF16, isOutput=False)
    bt_in = nc.declare_dram_parameter("bt", [HEADS, N, N], F8, isOutput=False)
    wq_in = nc.declare_dram_parameter("wq", [D, HD], BF16, isOutput=False)
    wk_in = nc.declare_dram_parameter("wk", [D, HD], BF16, isOutput=False)
    wv_in = nc.declare_dram_parameter("wv", [D, HD], BF16, isOutput=False)
    bqr_in = nc.declare_dram_parameter("bqr", [P, NT], F32, isOutput=False)
    bkr_in = nc.declare_dram_parameter("bkr", [P, NT], F32, isOutput=False)
    bv_in = nc.declare_dram_parameter("bv", [HD], F32, isOutput=False)
    wo_in = nc.declare_dram_parameter("wo", [HD, D], BF16, isOutput=False)
    bo_in = nc.declare_dram_parameter("bo", [1, D], BF16, isOutput=False)
    w1_in = nc.declare_dram_parameter("w1", [D, D], BF16, isOutput=False)
    w2_in = nc.declare_dram_parameter("w2", [D, D], BF16, isOutput=False)
    b1_in = nc.declare_dram_parameter("b1", [D, 1], F32, isOutput=False)
    b2_in = nc.declare_dram_parameter("b2", [D, 1], F32, isOutput=False)
    g_in = {}
    be_in = {}
    for i in (0, 2, 3):
        g_in[i] = nc.declare_dram_parameter(f"g{i}", [D], BF16, isOutput=False)
        be_in[i] = nc.declare_dram_parameter(f"be{i}", [D], BF16, isOutput=False)
    out_dram = nc.declare_dram_parameter("out", [N, D], F32, isOutput=True)

    with tile.TileContext(nc) as tc, ExitStack() as ctx:
        consts = ctx.enter_context(tc.tile_pool(name="consts", bufs=1))
        persist = ctx.enter_context(tc.tile_pool(name="persist", bufs=1))
        small = ctx.enter_context(tc.tile_pool(name="small", bufs=12))
        stg = ctx.enter_context(tc.tile_pool(name="stg", bufs=3))
        # 2-bank 512-wide PSUM pool: open through B/C, reused in E via scope.
        ps512 = ctx.enter_context(
            tc.tile_pool(name="ps512", bufs=2, space=bass.MemorySpace.PSUM))

        # ---- constants -------------------------------------------------
        identb = consts.tile([P, P], BF16)
        make_identity(nc, identb)
        omib = consts.tile([P, P], BF16)  # 1 - I
        nc.gpsimd.memset(omib, 1.0)
        nc.gpsimd.affine_select(
            out=omib, in_=omib, compare_op=OP.not_equal, fill=0.0,
            base=0, pattern=[[-1, P]], channel_multiplier=1)
        # fp8 DoubleRow stationary: two slots of I/32 (moving is bias*16).
        id2 = consts.tile([P, 2, P], F8)
        nc.gpsimd.memset(id2, 0.0)
        for s in range(2):
            nc.gpsimd.affine_select(
                out=id2[:, s, :], in_=id2[:, s, :], compare_op=OP.not_equal,
                fill=1.0 / 32.0, base=0, pattern=[[-1, P]], channel_multiplier=1)
        nc.scalar.add_instruction(
            mybir.InstLoadActFuncSet(
                name=nc.get_next_instruction_name(), ins=[], outs=[],
                act_func_set_id=6))
        eps_t = consts.tile([P, 1], F32)
        nc.vector.memset(eps_t, EPS)
        onesb = consts.tile([1, P], BF16)
        nc.vector.memset(onesb, 1.0)
        onescol = consts.tile([P, 1], BF16)
        nc.vector.memset(onescol, 1.0)

        gb = {}
        beb = {}
        for i in (0, 2, 3):
            gb[i] = consts.tile([P, D], BF16, name=f"g{i}b", tag=f"g{i}b")
            _bcast_load(nc, gb[i], g_in[i][:])
            beb[i] = consts.tile([P, D], BF16, name=f"be{i}b", tag=f"be{i}b")
            _bcast_load(nc, beb[i], be_in[i][:])
        bvb = consts.tile([P, HD], F32)
        _bcast_load(nc, bvb, bv_in[:])
        bqr = consts.tile([P, NT], F32)
        nc.gpsimd.dma_start(out=bqr, in_=bqr_in[:, :])
        bkr = consts.tile([P, NT], F32)
        nc.gpsimd.dma_start(out=bkr, in_=bkr_in[:, :])
        wq_sb = consts.tile([P, HD], BF16)
        nc.gpsimd.dma_start(out=wq_sb, in_=wq_in[:, :])
        wk_sb = consts.tile([P, HD], BF16)
        nc.gpsimd.dma_start(out=wk_sb, in_=wk_in[:, :])
        wv_sb = consts.tile([P, HD], BF16)
        nc.gpsimd.dma_start(out=wv_sb, in_=wv_in[:, :])
        # W_O as [p=dv-within-head, h, d]
        wo_sb = consts.tile([P, HEADS, D], BF16)
        nc.sync.dma_start(
            out=wo_sb, in_=wo_in.rearrange("(hh p) d -> p hh d", p=P))
        bo_sb = consts.tile([1, D], BF16)
        nc.gpsimd.dma_start(out=bo_sb, in_=bo_in[:, :])
        w1_sb = consts.tile([P, D], BF16)
        nc.gpsimd.dma_start(out=w1_sb, in_=w1_in[:, :])
        w2_sb = consts.tile([P, D], BF16)
        nc.gpsimd.dma_start(out=w2_sb, in_=w2_in[:, :])
        b1_sb = consts.tile([P, 1], F32)
        nc.gpsimd.dma_start(out=b1_sb, in_=b1_in[:, :])
        b2_sb = consts.tile([P, 1], F32)
        nc.gpsimd.dma_start(out=b2_sb, in_=b2_in[:, :])

        # ---- persistent tensors ---------------------------------------
        h_sb = persist.tile([P, NT, D], BF16, tag="h")
        hn_t = persist.tile([P, NT, D], BF16, tag="hnt")
        ats = persist.tile([P, NT, N], BF16, tag="ats")  # A^T -> adj_norm^T
        for j in range(NT):
            nc.sync.dma_start(
                out=ats[:, j, :],
                in_=at_in[:, :].rearrange("(t p) n -> p t n", p=P)[:, j, :])
        nc.sync.dma_start(out=h_sb, in_=h_in.rearrange("(t p) d -> p t d", p=P))
        disb = persist.tile([P, N], BF16, tag="disb")
        dis_tok = persist.tile([P, NT], F32, tag="distok")
        adjHnT = persist.tile([P, N], BF16, tag="adjhnt")
        qT = persist.tile([P, HEADS, N], BF16, tag="qT")
        kT = persist.tile([P, HEADS, N], BF16, tag="kT")
        vna = persist.tile([P, NT, HEADS, DV + 1], BF16, tag="v")
        nc.vector.memset(vna[:, :, :, DV:DV + 1], 1.0)
        mhcT = persist.tile([P, HEADS, N], BF16, tag="mhcT")

        # ---- phase A: H LayerNorm (token-major) ------------------------
        for i in range(NT):
            s6 = small.tile([P, 6], F32, tag="s6")
            mv = small.tile([P, 2], F32, tag="mv")
            nc.vector.bn_stats(out=s6, in_=h_sb[:, i, :])
            nc.vector.bn_aggr(out=mv, in_=s6)
            lnv = small.tile([P, 1], F32, tag="lnv")
            nc.scalar.activation(out=lnv, in_=mv[:, 1:2], func=AF.Ln, bias=eps_t)
            rstd = small.tile([P, 1], F32, tag="rstd")
            nc.scalar.activation(out=rstd, in_=lnv, func=AF.Exp, scale=-0.5)
            hw = stg.tile([P, D], BF16, name="hw", tag="hw")
            nc.vector.tensor_scalar(
                out=hw, in0=h_sb[:, i, :], scalar1=mv[:, 0:1], scalar2=rstd,
                op0=OP.subtract, op1=OP.mult)
            nc.vector.tensor_mul(out=hw, in0=hw, in1=gb[0])
            nc.vector.tensor_add(out=hn_t[:, i, :], in0=hw, in1=beb[0])

        # ---- phase B: adjacency prep ------------------------------------
        with tc.tile_pool(name="psR", bufs=1, space=bass.MemorySpace.PSUM) as psR, \
             tc.tile_pool(name="psTb", bufs=2, space=bass.MemorySpace.PSUM) as psTb:
            rs_ps = [psR.tile([1, 512], F32, name=f"rsps{c}", tag=f"rsps{c}")
                     for c in range(2)]
            for j in range(NT):
                db = ats[:, j, j * P:(j + 1) * P]
                nc.vector.tensor_mul(out=db, in0=db, in1=omib)
                nc.vector.tensor_add(out=db, in0=db, in1=identb)
                for c in range(2):
                    nc.tensor.matmul(
                        rs_ps[c], onescol, ats[:, j, c * 512:(c + 1) * 512],
                        start=(j == 0), stop=(j == NT - 1))
            rs_sb = small.tile([1, N], F32, tag="rssb", bufs=1)
            for c in range(2):
                nc.vector.tensor_scalar_max(
                    out=rs_sb[:, c * 512:(c + 1) * 512], in0=rs_ps[c], scalar1=1.0)
            lnr = small.tile([1, N], F32, tag="lnr", bufs=1)
            nc.scalar.activation(out=lnr, in_=rs_sb, func=AF.Ln)
            disrow = small.tile([1, N], BF16, tag="disrow", bufs=1)
            nc.scalar.activation(out=disrow, in_=lnr, func=AF.Exp, scale=-0.5)
            # broadcast di over partitions: disb[q, n] = di_n
            for c in range(2):
                psd = ps512.tile([P, 512], F32, tag="ps512")
                nc.tensor.matmul(
                    psd, onesb, disrow[:, c * 512:(c + 1) * 512],
                    start=True, stop=True)
                nc.scalar.activation(
                    out=disb[:, c * 512:(c + 1) * 512], in_=psd, func=AF.Copy)
            # dis_tok[p, j] = di_{j*P+p} via PE transpose of disb chunks
            for j in range(NT):
                ptb = psTb.tile([P, P], BF16, tag="ptb")
                nc.tensor.transpose(ptb, disb[:, j * P:(j + 1) * P], identb)
                nc.scalar.activation(out=dis_tok[:, j:j + 1], in_=ptb[:, 0:1],
                                     func=AF.Copy)
            # fold both degree scalings into A^T (in place):
            # ats[m, n] = At[m, n] * di_m * di_n
            for j in range(NT):
                nc.vector.scalar_tensor_tensor(
                    out=ats[:, j, :], in0=ats[:, j, :],
                    scalar=dis_tok[:, j:j + 1], in1=disb,
                    op0=OP.mult, op1=OP.mult)

        # ---- phase C: shared GCN trunk ----------------------------------
        # adjHnT[d, n] = sum_m hn[m, d] * adjn^T[m, n]
        for c in range(2):
            psc = ps512.tile([P, 512], F32, tag="ps512")
            for j in range(NT):
                nc.tensor.matmul(
                    psc, hn_t[:, j, :], ats[:, j, c * 512:(c + 1) * 512],
                    start=(j == 0), stop=(j == NT - 1))
            nc.vector.tensor_copy(out=adjHnT[:, c * 512:(c + 1) * 512], in_=psc)

        def emit_qk(hh, q_on_act=False):
            for c in range(2):
                psc = ps512.tile([P, 512], F32, tag="ps512")
                nc.tensor.matmul(
                    psc, wq_sb[:, hh * P:(hh + 1) * P],
                    adjHnT[:, c * 512:(c + 1) * 512], start=True, stop=True)
                if q_on_act:
                    nc.scalar.activation(
                        out=qT[:, hh, c * 512:(c + 1) * 512], in_=psc,
                        func=AF.Identity, bias=bqr[:, hh:hh + 1])
                else:
                    nc.vector.tensor_scalar_add(
                        out=qT[:, hh, c * 512:(c + 1) * 512], in0=psc,
                        scalar1=bqr[:, hh:hh + 1])
                psc = ps512.tile([P, 512], F32, tag="ps512")
                nc.tensor.matmul(
                    psc, wk_sb[:, hh * P:(hh + 1) * P],
                    adjHnT[:, c * 512:(c + 1) * 512], start=True, stop=True)
                nc.vector.tensor_scalar_add(
                    out=kT[:, hh, c * 512:(c + 1) * 512], in0=psc,
                    scalar1=bkr[:, hh:hh + 1])

        emit_qk(0, q_on_act=True)

        def emit_v(c, i0=0, i1=NT):
            for i in range(i0, i1):
                psc = ps512.tile([P, 512], F32, tag="ps512")
                nc.tensor.matmul(
                    psc, adjHnT[:, i * P:(i + 1) * P],
                    wv_sb[:, c * 512:(c + 1) * 512], start=True, stop=True)
                nc.vector.tensor_add(
                    out=vna[:, i, c * 4:(c + 1) * 4, 0:DV],
                    in0=psc.rearrange("p (a b) -> p a b", a=4),
                    in1=bvb[:, c * 512:(c + 1) * 512].rearrange(
                        "p (a b) -> p a b", a=4))

        emit_v(0)

        # ---- phase D: attention, software-pipelined over heads ----------
        with tc.tile_pool(name="psE", bufs=2, space=bass.MemorySpace.PSUM) as psE, \
             tc.tile_pool(name="psPM", bufs=2, space=bass.MemorySpace.PSUM) as psPM, \
             tc.tile_pool(name="etp", bufs=2) as etp, \
             tc.tile_pool(name="btp", bufs=2) as btp, \
             tc.tile_pool(name="mhp", bufs=2) as mhp:
            ets = {}
            mhs = {}
            mvss = {}
            vees = {}
            bt0 = btp.tile([P, NT, N], F8, tag="bt", name="bt0")
            nc.sync.dma_start(
                out=bt0, in_=bt_in[0].rearrange("(t p) n -> p t n", p=P))
            bts = {0: bt0}
            for stage in range(HEADS + 1):
                hh = stage
                if hh < HEADS:
                    et = etp.tile([P, NT, N], BF16, tag="et")
                    ets[hh] = et
                    btile = bts[hh]
                    for j in range(NT):
                        pse = psE.tile([P, N], F32, tag="pse")
                        for c in range(2):
                            nc.tensor.matmul(
                                pse[:, c * 512:(c + 1) * 512],
                                kT[:, hh, j * P:(j + 1) * P],
                                qT[:, hh, c * 512:(c + 1) * 512],
                                start=True, stop=False)
                            nc.tensor.matmul(
                                pse[:, c * 512:(c + 1) * 512],
                                id2, _dup2(btile[:, j, c * 512:(c + 1) * 512]),
                                start=False, stop=True, perf_mode=PM.DoubleRow)
                        nc.scalar.activation(out=et[:, j, :], in_=pse,
                                             func=AF.Exp)
                    if hh + 1 < HEADS:
                        btn = btp.tile([P, NT, N], F8, tag="bt",
                                       name=f"bt{hh + 1}")
                        nc.sync.dma_start(
                            out=btn,
                            in_=bt_in[hh + 1].rearrange("(t p) n -> p t n", p=P))
                        bts[hh + 1] = btn
                        emit_qk(hh + 1)
                if 1 <= stage <= 4:
                    emit_v(1, (stage - 1) * 2, stage * 2)
                if stage >= 1:
                    ph = stage - 1  # head whose PV/LN we process now
                    et = ets.pop(ph)
                    mh_sub = mhp.tile([P, NT, DV], BF16, tag="mh")
                    mhs[ph] = mh_sub
                    mvs = small.tile([P, NT, 2], F32, tag="mvs", bufs=2)
                    mvss[ph] = mvs
                    vee = small.tile([P, NT], F32, tag="vee", bufs=2)
                    vees[ph] = vee
                    for i in range(NT):
                        pm = psPM.tile([P, DV + 1], F32, tag="pm")
                        for j in range(NT):
                            nc.tensor.matmul(
                                pm, et[:, j, i * P:(i + 1) * P],
                                vna[:, j, ph, :],
                                start=(j == 0), stop=(j == NT - 1))
                        s6 = small.tile([P, 6], F32, tag="s6")
                        nc.vector.bn_stats(out=s6, in_=pm[:, 0:DV])
                        nc.vector.bn_aggr(out=mvs[:, i, :], in_=s6)
                        t = small.tile([P, 1], F32, tag="t")
                        nc.vector.tensor_scalar(
                            out=t, in0=pm[:, DV:DV + 1],
                            scalar1=pm[:, DV:DV + 1], scalar2=EPS,
                            op0=OP.mult, op1=OP.mult)
                        # mean-subtract now (frees pm); rstd scale later
                        nc.vector.tensor_scalar(
                            out=mh_sub[:, i, :], in0=pm[:, 0:DV],
                            scalar1=mvs[:, i, 0:1], scalar2=None,
                            op0=OP.subtract)
                        nc.gpsimd.tensor_add(
                            out=vee[:, i:i + 1], in0=t, in1=mvs[:, i, 1:2])
                    # batched rstd for the whole head (after next head's Exps
                    # in ACT program order -> no convoy)
                    lnv8 = small.tile([P, NT], F32, tag="lnv8", bufs=2)
                    nc.scalar.activation(out=lnv8, in_=vee, func=AF.Ln)
                    rstd8 = small.tile([P, NT], F32, tag="rstd8", bufs=2)
                    nc.scalar.activation(out=rstd8, in_=lnv8, func=AF.Exp,
                                         scale=-0.5)
                    for i in range(NT):
                        nc.vector.tensor_scalar_mul(
                            out=mh_sub[:, i, :], in0=mh_sub[:, i, :],
                            scalar1=rstd8[:, i:i + 1])
                    nc.sync.dma_start(
                        out=mhcT[:, ph, :].rearrange("p (t f) -> p t f", t=NT),
                        in_=mh_sub, transpose=True)

        # ---- phase E: output projection + MLP ---------------------------
        o_ln = persist.tile([P, NT, D], BF16, tag="oln")
        obe = persist.tile([P, NT, D], BF16, tag="obe")
        orows = persist.tile([P, NT, D], F32, tag="orows")
        mvE = small.tile([P, NT, 2], F32, tag="mvE", bufs=1)
        rstdE = small.tile([P, NT], F32, tag="rstdE", bufs=1)
        with tc.tile_pool(name="psO", bufs=2, space=bass.MemorySpace.PSUM) as psO, \
             tc.tile_pool(name="psTe", bufs=4, space=bass.MemorySpace.PSUM) as psTe:
            oT = persist.tile([P, NT, P], BF16, tag="oT")
            for i in range(NT):
                pso = psO.tile([P, D], F32, tag="pso")
                for hh in range(HEADS):
                    nc.tensor.matmul(
                        pso, mhcT[:, hh, i * P:(i + 1) * P], wo_sb[:, hh, :],
                        start=(hh == 0), stop=False)
                nc.tensor.matmul(pso, onesb, bo_sb, start=False, stop=True)
                nc.vector.tensor_add(
                    out=orows[:, i, :], in0=pso, in1=h_sb[:, i, :])
                s6 = small.tile([P, 6], F32, tag="s6")
                nc.vector.bn_stats(out=s6, in_=orows[:, i, :])
                nc.vector.bn_aggr(out=mvE[:, i, :], in_=s6)
                lnv = small.tile([P, 1], F32, tag="lnv")
                nc.scalar.activation(out=lnv, in_=mvE[:, i, 1:2], func=AF.Ln,
                                     bias=eps_t)
                nc.scalar.activation(out=rstdE[:, i:i + 1], in_=lnv,
                                     func=AF.Exp, scale=-0.5)
            for i in range(NT):
                nc.vector.tensor_scalar(
                    out=o_ln[:, i, :], in0=orows[:, i, :],
                    scalar1=mvE[:, i, 0:1], scalar2=rstdE[:, i:i + 1],
                    op0=OP.subtract, op1=OP.mult)
                nc.vector.tensor_mul(out=o_ln[:, i, :], in0=o_ln[:, i, :],
                                     in1=gb[2])
                nc.vector.tensor_add(out=o_ln[:, i, :], in0=o_ln[:, i, :],
                                     in1=beb[2])
                nc.gpsimd.tensor_add(out=obe[:, i, :], in0=o_ln[:, i, :],
                                     in1=beb[3])
                ptb = psTe.tile([P, P], BF16, tag="pte")
                nc.tensor.transpose(ptb, o_ln[:, i, :], identb)
                nc.scalar.activation(
                    out=oT[:, i, :], in_=ptb, func=AF.Copy)

            r1T = persist.tile([P, N], BF16, tag="r1T")
            for c in range(2):
                psc = ps512.tile([P, 512], F32, tag="ps512")
                nc.tensor.matmul(
                    psc, w1_sb,
                    oT[:, 4 * c:4 * (c + 1), :].rearrange("p t f -> p (t f)"),
                    start=True, stop=True)
                nc.scalar.activation(
                    out=r1T[:, c * 512:(c + 1) * 512], in_=psc, func=AF.Relu,
                    bias=b1_sb)
            r2T = persist.tile([P, N], BF16, tag="r2T")
            for c in range(2):
                psc = ps512.tile([P, 512], F32, tag="ps512")
                nc.tensor.matmul(
                    psc, w2_sb, r1T[:, c * 512:(c + 1) * 512],
                    start=True, stop=True)
                nc.scalar.activation(
                    out=r2T[:, c * 512:(c + 1) * 512], in_=psc, func=AF.Relu,
                    bias=b2_sb)

            r2tok = persist.tile([P, NT, D], BF16, tag="r2tok")
            mvR = small.tile([P, NT, 2], F32, tag="mvR", bufs=1)
            rstdR = small.tile([P, NT], F32, tag="rstdR", bufs=1)
            out_sb = persist.tile([P, NT, D], F32, tag="osb")
            for i in range(NT):
                ptb = psTe.tile([P, P], BF16, tag="pte")
                nc.tensor.transpose(ptb, r2T[:, i * P:(i + 1) * P], identb)
                nc.scalar.activation(out=r2tok[:, i, :], in_=ptb, func=AF.Copy)
                s6 = small.tile([P, 6], F32, tag="s6")
                nc.vector.bn_stats(out=s6, in_=r2tok[:, i, :])
                nc.vector.bn_aggr(out=mvR[:, i, :], in_=s6)
                lnv = small.tile([P, 1], F32, tag="lnv")
                nc.scalar.activation(out=lnv, in_=mvR[:, i, 1:2], func=AF.Ln,
                                     bias=eps_t)
                nc.scalar.activation(out=rstdR[:, i:i + 1], in_=lnv,
                                     func=AF.Exp, scale=-0.5)
            for i in range(NT):
                ro = stg.tile([P, D], BF16, name="ro", tag="ro")
                nc.vector.tensor_scalar(
                    out=ro, in0=r2tok[:, i, :], scalar1=mvR[:, i, 0:1],
                    scalar2=rstdR[:, i:i + 1], op0=OP.subtract, op1=OP.mult)
                nc.vector.tensor_mul(out=ro, in0=ro, in1=gb[3])
                nc.vector.tensor_add(out=out_sb[:, i, :], in0=obe[:, i, :],
                                     in1=ro)
                nc.sync.dma_start(
                    out=out_dram.rearrange("(t p) d -> p t d", p=P)[:, i, :],
                    in_=out_sb[:, i, :])

    nc.compile()
    return nc


def _get_program():
    if "nc" not in _prog_cache:
        _prog_cache["nc"] = _build_program()
    return _prog_cache["nc"]


def kernel(**inputs):
    nc = _get_program()
    f32 = np.float32
    bf16 = ml_dtypes.bfloat16
    f8 = ml_dtypes.float8_e4m3fn

    H = np.asarray(inputs["H"], dtype=f32)
    A = np.asarray(inputs["A"], dtype=f32)
    g1 = np.asarray(inputs["g1"], dtype=f32)
    be1 = np.asarray(inputs["be1"], dtype=f32)
    WO = np.asarray(inputs["W_O"], dtype=f32)
    # fold LN1 gain/bias into the output projection
    WO_fold = WO * np.tile(g1, HEADS)[:, None]
    bO = np.tile(be1, HEADS) @ WO

    BT = np.asarray(inputs["B_bias"], dtype=f32).transpose(0, 2, 1)
    base = {
        "bt": np.ascontiguousarray(BT * 16.0).astype(f8),
        "wq": (np.asarray(inputs["W_Q"], dtype=f32) * DK).astype(bf16),
        "wk": np.asarray(inputs["W_K"], dtype=f32).astype(bf16),
        "wv": np.asarray(inputs["W_V"], dtype=f32).astype(bf16),
        "bqr": np.ascontiguousarray(
            (np.asarray(inputs["b_Q"], dtype=f32) * DK).reshape(NT, P).T),
        "bkr": np.ascontiguousarray(
            np.asarray(inputs["b_K"], dtype=f32).reshape(NT, P).T),
        "bv": np.asarray(inputs["b_V"], dtype=f32),
        "wo": WO_fold.astype(bf16),
        "bo": bO.reshape(1, D).astype(bf16),
        "w1": np.asarray(inputs["W1"], dtype=f32).astype(bf16),
        "w2": np.asarray(inputs["W2"], dtype=f32).astype(bf16),
        "b1": np.asarray(inputs["b1"], dtype=f32).reshape(D, 1),
        "b2": np.asarray(inputs["b2"], dtype=f32).reshape(D, 1),
    }
    for i in (0, 2, 3):
        base[f"g{i}"] = np.asarray(inputs[f"g{i}"], dtype=f32).astype(bf16)
        base[f"be{i}"] = np.asarray(inputs[f"be{i}"], dtype=f32).astype(bf16)

    in_maps = []
    for c in range(B):
        m = dict(base)
        m["h"] = H[c].astype(bf16)
        m["at"] = np.ascontiguousarray(A[c].T).astype(bf16)
        in_maps.append(m)

    res = run_bass_kernel_spmd(nc, in_maps, list(range(B)))
    out = np.stack([res.results[c]["out"] for c in range(B)], axis=0)
    return out.astype(np.float32)


if __name__ == "__main__":
    nc = _get_program()
    print("program built ok")
    from concourse.timeline_sim import TimelineSim
    ns = TimelineSim(nc, trace=False).simulate()
    print(f"TimelineSim: {ns:.0f} ns")


# revision 25
# speedup vs baseline: 2.9280x; 1.0457x over previous
"""Trainium2 Bass kernel for nn_GRIC_31550829756424 (GCN-attention block).

Data-parallel over batch: 8 batches -> 8 NeuronCores, one full batch per core.

Key structure (v2):
- GCN reassociated: adjHnT = Hn^T @ adj_norm^T computed once (shared by
  Q/K/V), then Q/K/V are single-step K=128 matmuls.  A is host-transposed
  and bf16; both degree scalings fold into A^T via one scalar_tensor_tensor.
- All rsqrt computed as exp(-0.5*ln(x)) so the ONLY activation table used is
  natural_log_exp_and_others (Exp/Ln/Relu/Copy/Identity) -> 1 table load.
- Attention bias B added into the QK PSUM via an fp8 DoubleRow matmul
  (identity/32 stationary, bias*16 moving) instead of DVE adds.
- LN1 gain/bias folded into W_O on the host; MH transposed via DMA XBAR.

Self-contained: hardcodes all shapes; imports only the in-container concourse
stack.
"""

import sys

sys.path.insert(0, "/opt/trn_rl_repo")

import numpy as np
import ml_dtypes
from contextlib import ExitStack

import concourse.bass as bass
import concourse.tile as tile
from concourse import bacc
from concourse import mybir
from concourse.bass_utils import run_bass_kernel_spmd
from concourse.masks import make_identity

F32 = mybir.dt.float32
BF16 = mybir.dt.bfloat16
F8 = mybir.dt.float8e4
AF = mybir.ActivationFunctionType
OP = mybir.AluOpType
PM = mybir.MatmulPerfMode

B = 8
N = 1024
D = 128
HEADS = 8
DV = 128
HD = HEADS * DV  # 1024
P = 128
NT = N // P  # 8 tiles of 128 rows
DK = 1.0 / float(np.sqrt(np.float32(D)))
EPS = 1e-5

_prog_cache = {}


def _bcast_load(nc, dst, src):
    """DMA-load 1D DRAM vector src [W] replicated across all P partitions of
    dst [P, W] (same dtype)."""
    rep = bass.AP(tensor=src.tensor, offset=src.offset, ap=[[0, P]] + list(src.ap))
    nc.gpsimd.dma_start(out=dst, in_=rep)


def _dup2(ap):
    """View a [P, W] AP as [P, 2, W] with a stride-0 middle dim (DoubleRow
    moving operand reading the same data in both slots)."""
    return bass.AP(
        tensor=ap.tensor, offset=ap.offset,
        ap=[list(ap.ap[0]), [0, 2]] + [list(a) for a in ap.ap[1:]],
    )


def _build_program():
    nc = bacc.Bacc(None)

    h_in = nc.declare_dram_parameter("h", [N, D], BF16, isOutput=False)
    at_in = nc.declare_dram_parameter("at", [N, N], BF16, isOutput=False)
    bt_in = nc.declare_dram_parameter("bt", [HEADS, N, N], F8, isOutput=False)
    wq_in = nc.declare_dram_parameter("wq", [D, HD], BF16, isOutput=False)
    wk_in = nc.declare_dram_parameter("wk", [D, HD], BF16, isOutput=False)
    wv_in = nc.declare_dram_parameter("wv", [D, HD], BF16, isOutput=False)
    bqr_in = nc.declare_dram_parameter("bqr", [P, NT], F32, isOutput=False)
    bkr_in = nc.declare_dram_parameter("bkr", [P, NT], F32, isOutput=False)
    bv_in = nc.declare_dram_parameter("bv", [HD], F32, isOutput=False)
    wo_in = nc.declare_dram_parameter("wo", [HD, D], BF16, isOutput=False)
    bo_in = nc.declare_dram_parameter("bo", [1, D], BF16, isOutput=False)
    w1_in = nc.declare_dram_parameter("w1", [D, D], BF16, isOutput=False)
    w2_in = nc.declare_dram_parameter("w2", [D, D], BF16, isOutput=False)
    b1_in = nc.declare_dram_parameter("b1", [D, 1], F32, isOutput=False)
    b2_in = nc.declare_dram_parameter("b2", [D, 1], F32, isOutput=False)
    g_in = {}
    be_in = {}
    for i in (0, 2, 3):
        g_in[i] = nc.declare_dram_parameter(f"g{i}", [D], BF16, isOutput=False)
        be_in[i] = nc.declare_dram_parameter(f"be{i}", [D], BF16, isOutput=False)
    out_dram = nc.declare_dram_parameter("out", [N, D], F32, isOutput=True)

    with tile.TileContext(nc) as tc, ExitStack() as ctx:
        consts = ctx.enter_context(tc.tile_pool(name="consts", bufs=1))
        persist = ctx.enter_context(tc.tile_pool(name="persist", bufs=1))
        small = ctx.enter_context(tc.tile_pool(name="small", bufs=12))
        stg = ctx.enter_context(tc.tile_pool(name="stg", bufs=3))
        # 2-bank 512-wide PSUM pool: open through B/C, reused in E via scope.
        ps512 = ctx.enter_context(
            tc.tile_pool(name="ps512", bufs=2, space=bass.MemorySpace.PSUM))

        # ---- constants -------------------------------------------------
        identb = consts.tile([P, P], BF16)
        make_identity(nc, identb)
        omib = consts.tile([P, P], BF16)  # 1 - I
        nc.gpsimd.memset(omib, 1.0)
        nc.gpsimd.affine_select(
            out=omib, in_=omib, compare_op=OP.not_equal, fill=0.0,
            base=0, pattern=[[-1, P]], channel_multiplier=1)
        # fp8 DoubleRow stationary: two slots of I/32 (moving is bias*16).
        id2 = consts.tile([P, 2, P], F8)
        nc.gpsimd.memset(id2, 0.0)
        for s in range(2):
            nc.gpsimd.affine_select(
                out=id2[:, s, :], in_=id2[:, s, :], compare_op=OP.not_equal,
                fill=1.0 / 32.0, base=0, pattern=[[-1, P]], channel_multiplier=1)
        nc.scalar.add_instruction(
            mybir.InstLoadActFuncSet(
                name=nc.get_next_instruction_name(), ins=[], outs=[],
                act_func_set_id=6))
        eps_t = consts.tile([P, 1], F32)
        nc.vector.memset(eps_t, EPS)
        onesb = consts.tile([1, P], BF16)
        nc.vector.memset(onesb, 1.0)
        onescol = consts.tile([P, 1], BF16)
        nc.vector.memset(onescol, 1.0)

        gb = {}
        beb = {}
        for i in (0, 2, 3):
            gb[i] = consts.tile([P, D], BF16, name=f"g{i}b", tag=f"g{i}b")
            _bcast_load(nc, gb[i], g_in[i][:])
            beb[i] = consts.tile([P, D], BF16, name=f"be{i}b", tag=f"be{i}b")
            _bcast_load(nc, beb[i], be_in[i][:])
        bvb = consts.tile([P, HD], F32)
        _bcast_load(nc, bvb, bv_in[:])
        bqr = consts.tile([P, NT], F32)
        nc.gpsimd.dma_start(out=bqr, in_=bqr_in[:, :])
        bkr = consts.tile([P, NT], F32)
        nc.gpsimd.dma_start(out=bkr, in_=bkr_in[:, :])
        wq_sb = consts.tile([P, HD], BF16)
        nc.gpsimd.dma_start(out=wq_sb, in_=wq_in[:, :])
        wk_sb = consts.tile([P, HD], BF16)
        nc.gpsimd.dma_start(out=wk_sb, in_=wk_in[:, :])
        wv_sb = consts.tile([P, HD], BF16)
        nc.gpsimd.dma_start(out=wv_sb, in_=wv_in[:, :])
        # W_O as [p=dv-within-head, h, d]
        wo_sb = consts.tile([P, HEADS, D], BF16)
        nc.sync.dma_start(
            out=wo_sb, in_=wo_in.rearrange("(hh p) d -> p hh d", p=P))
        bo_sb = consts.tile([1, D], BF16)
        nc.gpsimd.dma_start(out=bo_sb, in_=bo_in[:, :])
        w1_sb = consts.tile([P, D], BF16)
        nc.gpsimd.dma_start(out=w1_sb, in_=w1_in[:, :])
        w2_sb = consts.tile([P, D], BF16)
        nc.gpsimd.dma_start(out=w2_sb, in_=w2_in[:, :])
        b1_sb = consts.tile([P, 1], F32)
        nc.gpsimd.dma_start(out=b1_sb, in_=b1_in[:, :])
        b2_sb = consts.tile([P, 1], F32)
        nc.gpsimd.dma_start(out=b2_sb, in_=b2_in[:, :])

        # ---- persistent tensors ---------------------------------------
        h_sb = persist.tile([P, NT, D], BF16, tag="h")
        hn_t = persist.tile([P, NT, D], BF16, tag="hnt")
        ats = persist.tile([P, NT, N], BF16, tag="ats")  # A^T -> adj_norm^T
        nc.sync.dma_start(out=h_sb, in_=h_in.rearrange("(t p) d -> p t d", p=P))
        for j in range(NT):
            nc.sync.dma_start(
                out=ats[:, j, :],
                in_=at_in[:, :].rearrange("(t p) n -> p t n", p=P)[:, j, :])
        disb = persist.tile([P, N], BF16, tag="disb")
        dis_tok = persist.tile([P, NT], F32, tag="distok")
        adjHnT = persist.tile([P, N], BF16, tag="adjhnt")
        qT = persist.tile([P, HEADS, N], BF16, tag="qT")
        kT = persist.tile([P, HEADS, N], BF16, tag="kT")
        vna = persist.tile([P, NT, HEADS, DV + 1], BF16, tag="v")
        nc.vector.memset(vna[:, :, :, DV:DV + 1], 1.0)
        mhcT = persist.tile([P, HEADS, N], BF16, tag="mhcT")

        # ---- phase B part 1: diagonal fix + rowsums ---------------------
        with tc.tile_pool(name="psR", bufs=1, space=bass.MemorySpace.PSUM) as psR, \
             tc.tile_pool(name="psTb", bufs=2, space=bass.MemorySpace.PSUM) as psTb:
            rs_ps = [psR.tile([1, 512], F32, name=f"rsps{c}", tag=f"rsps{c}")
                     for c in range(2)]
            for j in range(NT):
                db = ats[:, j, j * P:(j + 1) * P]
                nc.vector.tensor_mul(out=db, in0=db, in1=omib)
                nc.vector.tensor_add(out=db, in0=db, in1=identb)
                for c in range(2):
                    nc.tensor.matmul(
                        rs_ps[c], onescol, ats[:, j, c * 512:(c + 1) * 512],
                        start=(j == 0), stop=(j == NT - 1))
            # -- phase A: H LayerNorm (interleaved) --
            for i in range(NT):
                s6 = small.tile([P, 6], F32, tag="s6")
                mv = small.tile([P, 2], F32, tag="mv")
                nc.vector.bn_stats(out=s6, in_=h_sb[:, i, :])
                nc.vector.bn_aggr(out=mv, in_=s6)
                lnv = small.tile([P, 1], F32, tag="lnv")
                nc.scalar.activation(out=lnv, in_=mv[:, 1:2], func=AF.Ln, bias=eps_t)
                rstd = small.tile([P, 1], F32, tag="rstd")
                nc.scalar.activation(out=rstd, in_=lnv, func=AF.Exp, scale=-0.5)
                hw = stg.tile([P, D], BF16, name="hw", tag="hw")
                nc.vector.tensor_scalar(
                    out=hw, in0=h_sb[:, i, :], scalar1=mv[:, 0:1], scalar2=rstd,
                    op0=OP.subtract, op1=OP.mult)
                nc.vector.tensor_mul(out=hw, in0=hw, in1=gb[0])
                nc.vector.tensor_add(out=hn_t[:, i, :], in0=hw, in1=beb[0])

            rs_sb = small.tile([1, N], F32, tag="rssb", bufs=1)
            for c in range(2):
                nc.vector.tensor_scalar_max(
                    out=rs_sb[:, c * 512:(c + 1) * 512], in0=rs_ps[c], scalar1=1.0)
            lnr = small.tile([1, N], F32, tag="lnr", bufs=1)
            nc.scalar.activation(out=lnr, in_=rs_sb, func=AF.Ln)
            disrow = small.tile([1, N], BF16, tag="disrow", bufs=1)
            nc.scalar.activation(out=disrow, in_=lnr, func=AF.Exp, scale=-0.5)
            # broadcast di over partitions: disb[q, n] = di_n
            for c in range(2):
                psd = ps512.tile([P, 512], F32, tag="ps512")
                nc.tensor.matmul(
                    psd, onesb, disrow[:, c * 512:(c + 1) * 512],
                    start=True, stop=True)
                nc.scalar.activation(
                    out=disb[:, c * 512:(c + 1) * 512], in_=psd, func=AF.Copy)
            # dis_tok[p, j] = di_{j*P+p} via PE transpose of disb chunks
            for j in range(NT):
                ptb = psTb.tile([P, P], BF16, tag="ptb")
                nc.tensor.transpose(ptb, disb[:, j * P:(j + 1) * P], identb)
                nc.scalar.activation(out=dis_tok[:, j:j + 1], in_=ptb[:, 0:1],
                                     func=AF.Copy)
            # fold di_m into hn_t (per-partition) instead of scaling ats
            for j in range(NT):
                nc.vector.tensor_scalar_mul(
                    out=hn_t[:, j, :], in0=hn_t[:, j, :],
                    scalar1=dis_tok[:, j:j + 1])

        # ---- phase C: shared GCN trunk ----------------------------------
        # adjHnT[d, n] = sum_m hn[m, d] * adjn^T[m, n]
        for c in range(2):
            psc = ps512.tile([P, 512], F32, tag="ps512")
            for j in range(NT):
                nc.tensor.matmul(
                    psc, hn_t[:, j, :], ats[:, j, c * 512:(c + 1) * 512],
                    start=(j == 0), stop=(j == NT - 1))
            nc.vector.tensor_mul(
                out=adjHnT[:, c * 512:(c + 1) * 512], in0=psc,
                in1=disb[:, c * 512:(c + 1) * 512])

        def emit_qk(hh, q_on_act=False):
            for c in range(2):
                psc = ps512.tile([P, 512], F32, tag="ps512")
                nc.tensor.matmul(
                    psc, wq_sb[:, hh * P:(hh + 1) * P],
                    adjHnT[:, c * 512:(c + 1) * 512], start=True, stop=True)
                if q_on_act:
                    nc.scalar.activation(
                        out=qT[:, hh, c * 512:(c + 1) * 512], in_=psc,
                        func=AF.Identity, bias=bqr[:, hh:hh + 1])
                else:
                    nc.vector.tensor_scalar_add(
                        out=qT[:, hh, c * 512:(c + 1) * 512], in0=psc,
                        scalar1=bqr[:, hh:hh + 1])
                psc = ps512.tile([P, 512], F32, tag="ps512")
                nc.tensor.matmul(
                    psc, wk_sb[:, hh * P:(hh + 1) * P],
                    adjHnT[:, c * 512:(c + 1) * 512], start=True, stop=True)
                nc.vector.tensor_scalar_add(
                    out=kT[:, hh, c * 512:(c + 1) * 512], in0=psc,
                    scalar1=bkr[:, hh:hh + 1])

        emit_qk(0, q_on_act=True)

        def emit_v(c, i0=0, i1=NT):
            for i in range(i0, i1):
                psc = ps512.tile([P, 512], F32, tag="ps512")
                nc.tensor.matmul(
                    psc, adjHnT[:, i * P:(i + 1) * P],
                    wv_sb[:, c * 512:(c + 1) * 512], start=True, stop=True)
                nc.vector.tensor_add(
                    out=vna[:, i, c * 4:(c + 1) * 4, 0:DV],
                    in0=psc.rearrange("p (a b) -> p a b", a=4),
                    in1=bvb[:, c * 512:(c + 1) * 512].rearrange(
                        "p (a b) -> p a b", a=4))

        emit_v(0)

        # ---- phase D: attention, software-pipelined over heads ----------
        with tc.tile_pool(name="psE", bufs=2, space=bass.MemorySpace.PSUM) as psE, \
             tc.tile_pool(name="psPM", bufs=2, space=bass.MemorySpace.PSUM) as psPM, \
             tc.tile_pool(name="etp", bufs=2) as etp, \
             tc.tile_pool(name="btp", bufs=2) as btp, \
             tc.tile_pool(name="mhp", bufs=2) as mhp:
            ets = {}
            mhs = {}
            mvss = {}
            vees = {}
            bt0 = btp.tile([P, NT, N], F8, tag="bt", name="bt0")
            nc.sync.dma_start(
                out=bt0, in_=bt_in[0].rearrange("(t p) n -> p t n", p=P))
            bts = {0: bt0}
            for stage in range(HEADS + 1):
                hh = stage
                if hh < HEADS:
                    et = etp.tile([P, NT, N], BF16, tag="et")
                    ets[hh] = et
                    btile = bts[hh]
                    for j in range(NT):
                        pse = psE.tile([P, N], F32, tag="pse")
                        for c in range(2):
                            nc.tensor.matmul(
                                pse[:, c * 512:(c + 1) * 512],
                                kT[:, hh, j * P:(j + 1) * P],
                                qT[:, hh, c * 512:(c + 1) * 512],
                                start=True, stop=False)
                            nc.tensor.matmul(
                                pse[:, c * 512:(c + 1) * 512],
                                id2, _dup2(btile[:, j, c * 512:(c + 1) * 512]),
                                start=False, stop=True, perf_mode=PM.DoubleRow)
                        nc.scalar.activation(out=et[:, j, :], in_=pse,
                                             func=AF.Exp)
                    if hh + 1 < HEADS:
                        btn = btp.tile([P, NT, N], F8, tag="bt",
                                       name=f"bt{hh + 1}")
                        nc.sync.dma_start(
                            out=btn,
                            in_=bt_in[hh + 1].rearrange("(t p) n -> p t n", p=P))
                        bts[hh + 1] = btn
                        emit_qk(hh + 1)
                if 1 <= stage <= 4:
                    emit_v(1, (stage - 1) * 2, stage * 2)
                if stage >= 1:
                    ph = stage - 1  # head whose PV/LN we process now
                    et = ets.pop(ph)
                    mh_sub = mhp.tile([P, NT, DV], BF16, tag="mh")
                    mhs[ph] = mh_sub
                    mvs = small.tile([P, NT, 2], F32, tag="mvs", bufs=2)
                    mvss[ph] = mvs
                    vee = small.tile([P, NT], F32, tag="vee", bufs=2)
                    vees[ph] = vee
                    for i in range(NT):
                        pm = psPM.tile([P, DV + 1], F32, tag="pm")
                        for j in range(NT):
                            nc.tensor.matmul(
                                pm, et[:, j, i * P:(i + 1) * P],
                                vna[:, j, ph, :],
                                start=(j == 0), stop=(j == NT - 1))
                        s6 = small.tile([P, 6], F32, tag="s6")
                        nc.vector.bn_stats(out=s6, in_=pm[:, 0:DV])
                        nc.vector.bn_aggr(out=mvs[:, i, :], in_=s6)
                        t = small.tile([P, 1], F32, tag="t")
                        nc.vector.tensor_scalar(
                            out=t, in0=pm[:, DV:DV + 1],
                            scalar1=pm[:, DV:DV + 1], scalar2=EPS,
                            op0=OP.mult, op1=OP.mult)
                        # mean-subtract now (frees pm); rstd scale later
                        if ph >= 6 and i % 2 == 1:
                            negm = small.tile([P, 1], F32, tag="negm", bufs=4)
                            nc.vector.tensor_scalar_mul(
                                out=negm, in0=mvs[:, i, 0:1], scalar1=-1.0)
                            nc.scalar.activation(
                                out=mh_sub[:, i, :], in_=pm[:, 0:DV],
                                func=AF.Identity, bias=negm)
                        else:
                            nc.vector.tensor_scalar(
                                out=mh_sub[:, i, :], in0=pm[:, 0:DV],
                                scalar1=mvs[:, i, 0:1], scalar2=None,
                                op0=OP.subtract)
                        nc.gpsimd.tensor_add(
                            out=vee[:, i:i + 1], in0=t, in1=mvs[:, i, 1:2])
                    # batched rstd for the whole head (after next head's Exps
                    # in ACT program order -> no convoy)
                    lnv8 = small.tile([P, NT], F32, tag="lnv8", bufs=2)
                    nc.scalar.activation(out=lnv8, in_=vee, func=AF.Ln)
                    rstd8 = small.tile([P, NT], F32, tag="rstd8", bufs=2)
                    nc.scalar.activation(out=rstd8, in_=lnv8, func=AF.Exp,
                                         scale=-0.5)
                    for i in range(NT):
                        nc.gpsimd.tensor_scalar_mul(
                            out=mh_sub[:, i, :], in0=mh_sub[:, i, :],
                            scalar1=rstd8[:, i:i + 1])
                    nc.sync.dma_start(
                        out=mhcT[:, ph, :].rearrange("p (t f) -> p t f", t=NT),
                        in_=mh_sub, transpose=True)

        # ---- phase E: output projection + MLP ---------------------------
        o_ln = persist.tile([P, NT, D], BF16, tag="oln")
        obe = persist.tile([P, NT, D], BF16, tag="obe")
        orows = persist.tile([P, NT, D], F32, tag="orows")
        mvE = small.tile([P, NT, 2], F32, tag="mvE", bufs=1)
        rstdE = small.tile([P, NT], F32, tag="rstdE", bufs=1)
        with tc.tile_pool(name="psO", bufs=2, space=bass.MemorySpace.PSUM) as psO, \
             tc.tile_pool(name="psTe", bufs=4, space=bass.MemorySpace.PSUM) as psTe:
            oT = persist.tile([P, NT, P], BF16, tag="oT")
            for i in range(NT):
                pso = psO.tile([P, D], F32, tag="pso")
                for hh in range(HEADS):
                    nc.tensor.matmul(
                        pso, mhcT[:, hh, i * P:(i + 1) * P], wo_sb[:, hh, :],
                        start=(hh == 0), stop=False)
                nc.tensor.matmul(pso, onesb, bo_sb, start=False, stop=True)
                nc.vector.tensor_add(
                    out=orows[:, i, :], in0=pso, in1=h_sb[:, i, :])
                s6 = small.tile([P, 6], F32, tag="s6")
                nc.vector.bn_stats(out=s6, in_=orows[:, i, :])
                nc.vector.bn_aggr(out=mvE[:, i, :], in_=s6)
                lnv = small.tile([P, 1], F32, tag="lnv")
                nc.scalar.activation(out=lnv, in_=mvE[:, i, 1:2], func=AF.Ln,
                                     bias=eps_t)
                nc.scalar.activation(out=rstdE[:, i:i + 1], in_=lnv,
                                     func=AF.Exp, scale=-0.5)
            for i in range(NT):
                nc.vector.tensor_scalar(
                    out=o_ln[:, i, :], in0=orows[:, i, :],
                    scalar1=mvE[:, i, 0:1], scalar2=rstdE[:, i:i + 1],
                    op0=OP.subtract, op1=OP.mult)
                nc.vector.tensor_mul(out=o_ln[:, i, :], in0=o_ln[:, i, :],
                                     in1=gb[2])
                nc.vector.tensor_add(out=o_ln[:, i, :], in0=o_ln[:, i, :],
                                     in1=beb[2])
                nc.gpsimd.tensor_add(out=obe[:, i, :], in0=o_ln[:, i, :],
                                     in1=beb[3])
                ptb = psTe.tile([P, P], BF16, tag="pte")
                nc.tensor.transpose(ptb, o_ln[:, i, :], identb)
                nc.scalar.activation(
                    out=oT[:, i, :], in_=ptb, func=AF.Copy)

            r1T = persist.tile([P, N], BF16, tag="r1T")
            for c in range(2):
                psc = ps512.tile([P, 512], F32, tag="ps512")
                nc.tensor.matmul(
                    psc, w1_sb,
                    oT[:, 4 * c:4 * (c + 1), :].rearrange("p t f -> p (t f)"),
                    start=True, stop=True)
                nc.scalar.activation(
                    out=r1T[:, c * 512:(c + 1) * 512], in_=psc, func=AF.Relu,
                    bias=b1_sb)
            r2T = persist.tile([P, N], BF16, tag="r2T")
            for c in range(2):
                psc = ps512.tile([P, 512], F32, tag="ps512")
                nc.tensor.matmul(
                    psc, w2_sb, r1T[:, c * 512:(c + 1) * 512],
                    start=True, stop=True)
                nc.scalar.activation(
                    out=r2T[:, c * 512:(c + 1) * 512], in_=psc, func=AF.Relu,
                    bias=b2_sb)

            r2tok = persist.tile([P, NT, D], BF16, tag="r2tok")
            mvR = small.tile([P, NT, 2], F32, tag="mvR", bufs=1)
            rstdR = small.tile([P, NT], F32, tag="rstdR", bufs=1)
            out_sb = persist.tile([P, NT, D], F32, tag="osb")
            for i in range(NT):
                ptb = psTe.tile([P, P], BF16, tag="pte")
                nc.tensor.transpose(ptb, r2T[:, i * P:(i + 1) * P], identb)
                nc.scalar.activation(out=r2tok[:, i, :], in_=ptb, func=AF.Copy)
                s6 = small.tile([P, 6], F32, tag="s6")
                nc.vector.bn_stats(out=s6, in_=r2tok[:, i, :])
                nc.vector.bn_aggr(out=mvR[:, i, :], in_=s6)
                lnv = small.tile([P, 1], F32, tag="lnv")
                nc.scalar.activation(out=lnv, in_=mvR[:, i, 1:2], func=AF.Ln,
                                     bias=eps_t)
                nc.scalar.activation(out=rstdR[:, i:i + 1], in_=lnv,
                                     func=AF.Exp, scale=-0.5)
            for i in range(NT):
                ro = stg.tile([P, D], BF16, name="ro", tag="ro")
                nc.vector.tensor_scalar(
                    out=ro, in0=r2tok[:, i, :], scalar1=mvR[:, i, 0:1],
                    scalar2=rstdR[:, i:i + 1], op0=OP.subtract, op1=OP.mult)
                nc.vector.tensor_mul(out=ro, in0=ro, in1=gb[3])
                nc.vector.tensor_add(out=out_sb[:, i, :], in0=obe[:, i, :],
                                     in1=ro)
                nc.sync.dma_start(
                    out=out_dram.rearrange("(t p) d -> p t d", p=P)[:, i, :],
                    in_=out_sb[:, i, :])

    nc.compile()
    return nc


def _get_program():
    if "nc" not in _prog_cache:
        _prog_cache["nc"] = _build_program()
    return _prog_cache["nc"]


def kernel(**inputs):
    nc = _get_program()
    f32 = np.float32
    bf16 = ml_dtypes.bfloat16
    f8 = ml_dtypes.float8_e4m3fn

    H = np.asarray(inputs["H"], dtype=f32)
    A = np.asarray(inputs["A"], dtype=f32)
    g1 = np.asarray(inputs["g1"], dtype=f32)
    be1 = np.asarray(inputs["be1"], dtype=f32)
    WO = np.asarray(inputs["W_O"], dtype=f32)
    # fold LN1 gain/bias into the output projection
    WO_fold = WO * np.tile(g1, HEADS)[:, None]
    bO = np.tile(be1, HEADS) @ WO

    BT = np.asarray(inputs["B_bias"], dtype=f32).transpose(0, 2, 1)
    base = {
        "bt": np.ascontiguousarray(BT * 16.0).astype(f8),
        "wq": (np.asarray(inputs["W_Q"], dtype=f32) * DK).astype(bf16),
        "wk": np.asarray(inputs["W_K"], dtype=f32).astype(bf16),
        "wv": np.asarray(inputs["W_V"], dtype=f32).astype(bf16),
        "bqr": np.ascontiguousarray(
            (np.asarray(inputs["b_Q"], dtype=f32) * DK).reshape(NT, P).T),
        "bkr": np.ascontiguousarray(
            np.asarray(inputs["b_K"], dtype=f32).reshape(NT, P).T),
        "bv": np.asarray(inputs["b_V"], dtype=f32),
        "wo": WO_fold.astype(bf16),
        "bo": bO.reshape(1, D).astype(bf16),
        "w1": np.asarray(inputs["W1"], dtype=f32).astype(bf16),
        "w2": np.asarray(inputs["W2"], dtype=f32).astype(bf16),
        "b1": np.asarray(inputs["b1"], dtype=f32).reshape(D, 1),
        "b2": np.asarray(inputs["b2"], dtype=f32).reshape(D, 1),
    }
    for i in (0, 2, 3):
        base[f"g{i}"] = np.asarray(inputs[f"g{i}"], dtype=f32).astype(bf16)
        base[f"be{i}"] = np.asarray(inputs[f"be{i}"], dtype=f32).astype(bf16)

    in_maps = []
    for c in range(B):
        m = dict(base)
        m["h"] = H[c].astype(bf16)
        m["at"] = np.ascontiguousarray(A[c].T).astype(bf16)
        in_maps.append(m)

    res = run_bass_kernel_spmd(nc, in_maps, list(range(B)))
    out = np.stack([res.results[c]["out"] for c in range(B)], axis=0)
    return out.astype(np.float32)


if __name__ == "__main__":
    nc = _get_program()
    print("program built ok")
    from concourse.timeline_sim import TimelineSim
    ns = TimelineSim(nc, trace=False).simulate()
    print(f"TimelineSim: {ns:.0f} ns")


# revision 31
# speedup vs baseline: 2.9538x; 1.0088x over previous
"""Trainium2 Bass kernel for nn_GRIC_31550829756424 (GCN-attention block).

Data-parallel over batch: 8 batches -> 8 NeuronCores, one full batch per core.

Key structure (v2):
- GCN reassociated: adjHnT = Hn^T @ adj_norm^T computed once (shared by
  Q/K/V), then Q/K/V are single-step K=128 matmuls.  A is host-transposed
  and bf16; both degree scalings fold into A^T via one scalar_tensor_tensor.
- All rsqrt computed as exp(-0.5*ln(x)) so the ONLY activation table used is
  natural_log_exp_and_others (Exp/Ln/Relu/Copy/Identity) -> 1 table load.
- Attention bias B added into the QK PSUM via an fp8 DoubleRow matmul
  (identity/32 stationary, bias*16 moving) instead of DVE adds.
- LN1 gain/bias folded into W_O on the host; MH transposed via DMA XBAR.

Self-contained: hardcodes all shapes; imports only the in-container concourse
stack.
"""

import sys

sys.path.insert(0, "/opt/trn_rl_repo")

import numpy as np
import ml_dtypes
from contextlib import ExitStack

import concourse.bass as bass
import concourse.tile as tile
from concourse import bacc
from concourse import mybir
from concourse.bass_utils import run_bass_kernel_spmd
from concourse.masks import make_identity

F32 = mybir.dt.float32
BF16 = mybir.dt.bfloat16
F8 = mybir.dt.float8e4
AF = mybir.ActivationFunctionType
OP = mybir.AluOpType
PM = mybir.MatmulPerfMode

B = 8
N = 1024
D = 128
HEADS = 8
DV = 128
HD = HEADS * DV  # 1024
P = 128
NT = N // P  # 8 tiles of 128 rows
DK = 1.0 / float(np.sqrt(np.float32(D)))
EPS = 1e-5

_prog_cache = {}


def _bcast_load(nc, dst, src):
    """DMA-load 1D DRAM vector src [W] replicated across all P partitions of
    dst [P, W] (same dtype)."""
    rep = bass.AP(tensor=src.tensor, offset=src.offset, ap=[[0, P]] + list(src.ap))
    nc.gpsimd.dma_start(out=dst, in_=rep)


def _dup2(ap):
    """View a [P, W] AP as [P, 2, W] with a stride-0 middle dim (DoubleRow
    moving operand reading the same data in both slots)."""
    return bass.AP(
        tensor=ap.tensor, offset=ap.offset,
        ap=[list(ap.ap[0]), [0, 2]] + [list(a) for a in ap.ap[1:]],
    )


def _build_program():
    nc = bacc.Bacc(None)

    h_in = nc.declare_dram_parameter("h", [N, D], BF16, isOutput=False)
    at_in = nc.declare_dram_parameter("at", [N, N], BF16, isOutput=False)
    bt_in = nc.declare_dram_parameter("bt", [HEADS, N, N], F8, isOutput=False)
    wq_in = nc.declare_dram_parameter("wq", [D, HD], BF16, isOutput=False)
    wk_in = nc.declare_dram_parameter("wk", [D, HD], BF16, isOutput=False)
    wv_in = nc.declare_dram_parameter("wv", [D, HD], BF16, isOutput=False)
    bqr_in = nc.declare_dram_parameter("bqr", [P, NT], F32, isOutput=False)
    bkr_in = nc.declare_dram_parameter("bkr", [P, NT], F32, isOutput=False)
    bv_in = nc.declare_dram_parameter("bv", [HD], F32, isOutput=False)
    wo_in = nc.declare_dram_parameter("wo", [HD, D], BF16, isOutput=False)
    bo_in = nc.declare_dram_parameter("bo", [1, D], BF16, isOutput=False)
    w1_in = nc.declare_dram_parameter("w1", [D, D], BF16, isOutput=False)
    w2_in = nc.declare_dram_parameter("w2", [D, D], BF16, isOutput=False)
    b1_in = nc.declare_dram_parameter("b1", [D, 1], F32, isOutput=False)
    b2_in = nc.declare_dram_parameter("b2", [D, 1], F32, isOutput=False)
    g_in = {}
    be_in = {}
    for i in (0, 2, 3):
        g_in[i] = nc.declare_dram_parameter(f"g{i}", [D], BF16, isOutput=False)
        be_in[i] = nc.declare_dram_parameter(f"be{i}", [D], BF16, isOutput=False)
    out_dram = nc.declare_dram_parameter("out", [N, D], F32, isOutput=True)

    with tile.TileContext(nc) as tc, ExitStack() as ctx:
        consts = ctx.enter_context(tc.tile_pool(name="consts", bufs=1))
        persist = ctx.enter_context(tc.tile_pool(name="persist", bufs=1))
        small = ctx.enter_context(tc.tile_pool(name="small", bufs=12))
        stg = ctx.enter_context(tc.tile_pool(name="stg", bufs=3))
        # 2-bank 512-wide PSUM pool: open through B/C, reused in E via scope.
        ps512 = ctx.enter_context(
            tc.tile_pool(name="ps512", bufs=2, space=bass.MemorySpace.PSUM))

        # ---- constants -------------------------------------------------
        identb = consts.tile([P, P], BF16)
        make_identity(nc, identb)
        omib = consts.tile([P, P], BF16)  # 1 - I
        nc.gpsimd.memset(omib, 1.0)
        nc.gpsimd.affine_select(
            out=omib, in_=omib, compare_op=OP.not_equal, fill=0.0,
            base=0, pattern=[[-1, P]], channel_multiplier=1)
        # fp8 DoubleRow stationary: two slots of I/32 (moving is bias*16).
        id2 = consts.tile([P, 2, P], F8)
        nc.gpsimd.memset(id2, 0.0)
        for s in range(2):
            nc.gpsimd.affine_select(
                out=id2[:, s, :], in_=id2[:, s, :], compare_op=OP.not_equal,
                fill=1.0 / 32.0, base=0, pattern=[[-1, P]], channel_multiplier=1)
        nc.scalar.add_instruction(
            mybir.InstLoadActFuncSet(
                name=nc.get_next_instruction_name(), ins=[], outs=[],
                act_func_set_id=6))
        eps_t = consts.tile([P, 1], F32)
        nc.vector.memset(eps_t, EPS)
        onesb = consts.tile([1, P], BF16)
        nc.vector.memset(onesb, 1.0)
        onescol = consts.tile([P, 1], BF16)
        nc.vector.memset(onescol, 1.0)

        gb = {}
        beb = {}
        for i in (0, 2, 3):
            gb[i] = consts.tile([P, D], BF16, name=f"g{i}b", tag=f"g{i}b")
            _bcast_load(nc, gb[i], g_in[i][:])
            beb[i] = consts.tile([P, D], BF16, name=f"be{i}b", tag=f"be{i}b")
            _bcast_load(nc, beb[i], be_in[i][:])
        bvb = consts.tile([P, HD], F32)
        _bcast_load(nc, bvb, bv_in[:])
        bqr = consts.tile([P, NT], F32)
        nc.gpsimd.dma_start(out=bqr, in_=bqr_in[:, :])
        bkr = consts.tile([P, NT], F32)
        nc.gpsimd.dma_start(out=bkr, in_=bkr_in[:, :])
        wq_sb = consts.tile([P, HD], BF16)
        nc.gpsimd.dma_start(out=wq_sb, in_=wq_in[:, :])
        wk_sb = consts.tile([P, HD], BF16)
        nc.gpsimd.dma_start(out=wk_sb, in_=wk_in[:, :])
        wv_sb = consts.tile([P, HD], BF16)
        nc.gpsimd.dma_start(out=wv_sb, in_=wv_in[:, :])
        # W_O as [p=dv-within-head, h, d]
        wo_sb = consts.tile([P, HEADS, D], BF16)
        nc.sync.dma_start(
            out=wo_sb, in_=wo_in.rearrange("(hh p) d -> p hh d", p=P))
        bo_sb = consts.tile([1, D], BF16)
        nc.gpsimd.dma_start(out=bo_sb, in_=bo_in[:, :])
        w1_sb = consts.tile([P, D], BF16)
        nc.gpsimd.dma_start(out=w1_sb, in_=w1_in[:, :])
        w2_sb = consts.tile([P, D], BF16)
        nc.gpsimd.dma_start(out=w2_sb, in_=w2_in[:, :])
        b1_sb = consts.tile([P, 1], F32)
        nc.gpsimd.dma_start(out=b1_sb, in_=b1_in[:, :])
        b2_sb = consts.tile([P, 1], F32)
        nc.gpsimd.dma_start(out=b2_sb, in_=b2_in[:, :])

        # ---- persistent tensors ---------------------------------------
        h_sb = persist.tile([P, NT, D], BF16, tag="h")
        hn_t = persist.tile([P, NT, D], BF16, tag="hnt")
        ats = persist.tile([P, NT, N], BF16, tag="ats")  # A^T -> adj_norm^T
        for j in range(NT):
            nc.sync.dma_start(
                out=ats[:, j, :],
                in_=at_in[:, :].rearrange("(t p) n -> p t n", p=P)[:, j, :])
        nc.sync.dma_start(out=h_sb, in_=h_in.rearrange("(t p) d -> p t d", p=P))
        disb = persist.tile([P, N], BF16, tag="disb")
        dis_tok = persist.tile([P, NT], F32, tag="distok")
        adjHnT = persist.tile([P, N], BF16, tag="adjhnt")
        qT = persist.tile([P, HEADS, N], BF16, tag="qT")
        kT = persist.tile([P, HEADS, N], BF16, tag="kT")
        vna = persist.tile([P, NT, HEADS, DV + 1], BF16, tag="v")
        nc.vector.memset(vna[:, :, :, DV:DV + 1], 1.0)
        mhcT = persist.tile([P, HEADS, N], BF16, tag="mhcT")

        # ---- phase B part 1: diagonal fix + rowsums ---------------------
        with tc.tile_pool(name="psR", bufs=1, space=bass.MemorySpace.PSUM) as psR, \
             tc.tile_pool(name="psTb", bufs=2, space=bass.MemorySpace.PSUM) as psTb:
            rs_ps = [psR.tile([1, 512], F32, name=f"rsps{c}", tag=f"rsps{c}")
                     for c in range(2)]
            for j in range(NT):
                db = ats[:, j, j * P:(j + 1) * P]
                nc.vector.tensor_mul(out=db, in0=db, in1=omib)
                nc.vector.tensor_add(out=db, in0=db, in1=identb)
                for c in range(2):
                    nc.tensor.matmul(
                        rs_ps[c], onescol, ats[:, j, c * 512:(c + 1) * 512],
                        start=(j == 0), stop=(j == NT - 1))
            # -- phase A: H LayerNorm (interleaved) --
            for i in range(NT):
                s6 = small.tile([P, 6], F32, tag="s6")
                mv = small.tile([P, 2], F32, tag="mv")
                nc.vector.bn_stats(out=s6, in_=h_sb[:, i, :])
                nc.vector.bn_aggr(out=mv, in_=s6)
                lnv = small.tile([P, 1], F32, tag="lnv")
                nc.scalar.activation(out=lnv, in_=mv[:, 1:2], func=AF.Ln, bias=eps_t)
                rstd = small.tile([P, 1], F32, tag="rstd")
                nc.scalar.activation(out=rstd, in_=lnv, func=AF.Exp, scale=-0.5)
                hw = stg.tile([P, D], BF16, name="hw", tag="hw")
                nc.vector.tensor_scalar(
                    out=hw, in0=h_sb[:, i, :], scalar1=mv[:, 0:1], scalar2=rstd,
                    op0=OP.subtract, op1=OP.mult)
                nc.vector.tensor_mul(out=hw, in0=hw, in1=gb[0])
                nc.vector.tensor_add(out=hn_t[:, i, :], in0=hw, in1=beb[0])

            rs_sb = small.tile([1, N], F32, tag="rssb", bufs=1)
            for c in range(2):
                nc.vector.tensor_scalar_max(
                    out=rs_sb[:, c * 512:(c + 1) * 512], in0=rs_ps[c], scalar1=1.0)
            lnr = small.tile([1, N], F32, tag="lnr", bufs=1)
            nc.scalar.activation(out=lnr, in_=rs_sb, func=AF.Ln)
            disrow = small.tile([1, N], BF16, tag="disrow", bufs=1)
            nc.scalar.activation(out=disrow, in_=lnr, func=AF.Exp, scale=-0.5)
            # broadcast di over partitions: disb[q, n] = di_n
            for c in range(2):
                psd = ps512.tile([P, 512], F32, tag="ps512")
                nc.tensor.matmul(
                    psd, onesb, disrow[:, c * 512:(c + 1) * 512],
                    start=True, stop=True)
                nc.scalar.activation(
                    out=disb[:, c * 512:(c + 1) * 512], in_=psd, func=AF.Copy)
            # dis_tok[p, j] = di_{j*P+p} via PE transpose of disb chunks
            for j in range(NT):
                ptb = psTb.tile([P, P], BF16, tag="ptb")
                nc.tensor.transpose(ptb, disb[:, j * P:(j + 1) * P], identb)
                nc.scalar.activation(out=dis_tok[:, j:j + 1], in_=ptb[:, 0:1],
                                     func=AF.Copy)
            # fold di_m into hn_t (per-partition) instead of scaling ats
            for j in range(NT):
                nc.vector.tensor_scalar_mul(
                    out=hn_t[:, j, :], in0=hn_t[:, j, :],
                    scalar1=dis_tok[:, j:j + 1])

        # ---- phase C: shared GCN trunk ----------------------------------
        # adjHnT[d, n] = sum_m hn[m, d] * adjn^T[m, n]
        for c in range(2):
            psc = ps512.tile([P, 512], F32, tag="ps512")
            for j in range(NT):
                nc.tensor.matmul(
                    psc, hn_t[:, j, :], ats[:, j, c * 512:(c + 1) * 512],
                    start=(j == 0), stop=(j == NT - 1))
            nc.vector.tensor_mul(
                out=adjHnT[:, c * 512:(c + 1) * 512], in0=psc,
                in1=disb[:, c * 512:(c + 1) * 512])

        def emit_qk(hh, q_on_act=False):
            for c in range(2):
                psc = ps512.tile([P, 512], F32, tag="ps512")
                nc.tensor.matmul(
                    psc, wq_sb[:, hh * P:(hh + 1) * P],
                    adjHnT[:, c * 512:(c + 1) * 512], start=True, stop=True)
                if q_on_act:
                    nc.scalar.activation(
                        out=qT[:, hh, c * 512:(c + 1) * 512], in_=psc,
                        func=AF.Identity, bias=bqr[:, hh:hh + 1])
                else:
                    nc.vector.tensor_scalar_add(
                        out=qT[:, hh, c * 512:(c + 1) * 512], in0=psc,
                        scalar1=bqr[:, hh:hh + 1])
                psc = ps512.tile([P, 512], F32, tag="ps512")
                nc.tensor.matmul(
                    psc, wk_sb[:, hh * P:(hh + 1) * P],
                    adjHnT[:, c * 512:(c + 1) * 512], start=True, stop=True)
                nc.vector.tensor_scalar_add(
                    out=kT[:, hh, c * 512:(c + 1) * 512], in0=psc,
                    scalar1=bkr[:, hh:hh + 1])

        emit_qk(0, q_on_act=True)

        def emit_v(c, i0=0, i1=NT):
            for i in range(i0, i1):
                psc = ps512.tile([P, 512], F32, tag="ps512")
                nc.tensor.matmul(
                    psc, adjHnT[:, i * P:(i + 1) * P],
                    wv_sb[:, c * 512:(c + 1) * 512], start=True, stop=True)
                nc.vector.tensor_add(
                    out=vna[:, i, c * 4:(c + 1) * 4, 0:DV],
                    in0=psc.rearrange("p (a b) -> p a b", a=4),
                    in1=bvb[:, c * 512:(c + 1) * 512].rearrange(
                        "p (a b) -> p a b", a=4))

        emit_v(0)

        # ---- phase D: attention, software-pipelined over heads ----------
        with tc.tile_pool(name="psE", bufs=2, space=bass.MemorySpace.PSUM) as psE, \
             tc.tile_pool(name="psPM", bufs=2, space=bass.MemorySpace.PSUM) as psPM, \
             tc.tile_pool(name="etp", bufs=3) as etp, \
             tc.tile_pool(name="btp", bufs=2) as btp, \
             tc.tile_pool(name="mhp", bufs=2) as mhp:
            ets = {}
            mhs = {}
            mvss = {}
            vees = {}
            bt0 = btp.tile([P, NT, N], F8, tag="bt", name="bt0")
            nc.sync.dma_start(
                out=bt0, in_=bt_in[0].rearrange("(t p) n -> p t n", p=P))
            bts = {0: bt0}
            for stage in range(HEADS + 1):
                hh = stage
                if hh < HEADS:
                    et = etp.tile([P, NT, N], BF16, tag="et")
                    ets[hh] = et
                    btile = bts[hh]
                    for j in range(NT):
                        pse = psE.tile([P, N], F32, tag="pse")
                        for c in range(2):
                            nc.tensor.matmul(
                                pse[:, c * 512:(c + 1) * 512],
                                kT[:, hh, j * P:(j + 1) * P],
                                qT[:, hh, c * 512:(c + 1) * 512],
                                start=True, stop=False)
                            nc.tensor.matmul(
                                pse[:, c * 512:(c + 1) * 512],
                                id2, _dup2(btile[:, j, c * 512:(c + 1) * 512]),
                                start=False, stop=True, perf_mode=PM.DoubleRow)
                        nc.scalar.activation(out=et[:, j, :], in_=pse,
                                             func=AF.Exp)
                    if hh + 1 < HEADS:
                        btn = btp.tile([P, NT, N], F8, tag="bt",
                                       name=f"bt{hh + 1}")
                        nc.sync.dma_start(
                            out=btn,
                            in_=bt_in[hh + 1].rearrange("(t p) n -> p t n", p=P))
                        bts[hh + 1] = btn
                        emit_qk(hh + 1)
                if 1 <= stage <= 4:
                    emit_v(1, (stage - 1) * 2, stage * 2)
                if stage >= 1:
                    ph = stage - 1  # head whose PV/LN we process now
                    et = ets.pop(ph)
                    mh_sub = mhp.tile([P, NT, DV], BF16, tag="mh")
                    mhs[ph] = mh_sub
                    mvs = small.tile([P, NT, 2], F32, tag="mvs", bufs=2)
                    mvss[ph] = mvs
                    vee = small.tile([P, NT], F32, tag="vee", bufs=2)
                    vees[ph] = vee
                    for i in range(NT):
                        pm = psPM.tile([P, DV + 1], F32, tag="pm")
                        for j in range(NT):
                            nc.tensor.matmul(
                                pm, et[:, j, i * P:(i + 1) * P],
                                vna[:, j, ph, :],
                                start=(j == 0), stop=(j == NT - 1))
                        s6 = small.tile([P, 6], F32, tag="s6")
                        nc.vector.bn_stats(out=s6, in_=pm[:, 0:DV])
                        nc.vector.bn_aggr(out=mvs[:, i, :], in_=s6)
                        t = small.tile([P, 1], F32, tag="t")
                        nc.vector.tensor_scalar(
                            out=t, in0=pm[:, DV:DV + 1],
                            scalar1=pm[:, DV:DV + 1], scalar2=EPS,
                            op0=OP.mult, op1=OP.mult)
                        # mean-subtract now (frees pm); rstd scale later
                        nc.vector.tensor_scalar(
                            out=mh_sub[:, i, :], in0=pm[:, 0:DV],
                            scalar1=mvs[:, i, 0:1], scalar2=None,
                            op0=OP.subtract)
                        nc.gpsimd.tensor_add(
                            out=vee[:, i:i + 1], in0=t, in1=mvs[:, i, 1:2])
                    # batched rstd for the whole head (after next head's Exps
                    # in ACT program order -> no convoy)
                    lnv8 = small.tile([P, NT], F32, tag="lnv8", bufs=2)
                    nc.scalar.activation(out=lnv8, in_=vee, func=AF.Ln)
                    rstd8 = small.tile([P, NT], F32, tag="rstd8", bufs=2)
                    nc.scalar.activation(out=rstd8, in_=lnv8, func=AF.Exp,
                                         scale=-0.5)
                    for i in range(NT):
                        nc.gpsimd.tensor_scalar_mul(
                            out=mh_sub[:, i, :], in0=mh_sub[:, i, :],
                            scalar1=rstd8[:, i:i + 1])
                    for half in range(2):
                        nc.sync.dma_start(
                            out=mhcT[:, ph, half * 512:(half + 1) * 512]
                            .rearrange("p (t f) -> p t f", t=NT // 2),
                            in_=mh_sub[:, half * 4:(half + 1) * 4, :],
                            transpose=True)

        # ---- phase E: output projection + MLP ---------------------------
        o_ln = persist.tile([P, NT, D], BF16, tag="oln")
        obe = persist.tile([P, NT, D], BF16, tag="obe")
        orows = persist.tile([P, NT, D], F32, tag="orows")
        mvE = small.tile([P, NT, 2], F32, tag="mvE", bufs=1)
        rstdE = small.tile([P, NT], F32, tag="rstdE", bufs=1)
        with tc.tile_pool(name="psO", bufs=2, space=bass.MemorySpace.PSUM) as psO, \
             tc.tile_pool(name="psTe", bufs=4, space=bass.MemorySpace.PSUM) as psTe:
            oT = persist.tile([P, NT, P], BF16, tag="oT")
            for i in range(NT):
                pso = psO.tile([P, D], F32, tag="pso")
                for hh in range(HEADS):
                    nc.tensor.matmul(
                        pso, mhcT[:, hh, i * P:(i + 1) * P], wo_sb[:, hh, :],
                        start=(hh == 0), stop=False)
                nc.tensor.matmul(pso, onesb, bo_sb, start=False, stop=True)
                nc.vector.tensor_add(
                    out=orows[:, i, :], in0=pso, in1=h_sb[:, i, :])
                s6 = small.tile([P, 6], F32, tag="s6")
                nc.vector.bn_stats(out=s6, in_=orows[:, i, :])
                nc.vector.bn_aggr(out=mvE[:, i, :], in_=s6)
                lnv = small.tile([P, 1], F32, tag="lnv")
                nc.scalar.activation(out=lnv, in_=mvE[:, i, 1:2], func=AF.Ln,
                                     bias=eps_t)
                nc.scalar.activation(out=rstdE[:, i:i + 1], in_=lnv,
                                     func=AF.Exp, scale=-0.5)
            for i in range(NT):
                nc.vector.tensor_scalar(
                    out=o_ln[:, i, :], in0=orows[:, i, :],
                    scalar1=mvE[:, i, 0:1], scalar2=rstdE[:, i:i + 1],
                    op0=OP.subtract, op1=OP.mult)
                ptb = psTe.tile([P, P], BF16, tag="pte")
                nc.tensor.transpose(ptb, o_ln[:, i, :], identb)
                nc.scalar.activation(
                    out=oT[:, i, :], in_=ptb, func=AF.Copy)
                nc.gpsimd.tensor_mul(out=obe[:, i, :], in0=o_ln[:, i, :],
                                     in1=gb[2])
                nc.gpsimd.tensor_add(out=obe[:, i, :], in0=obe[:, i, :],
                                     in1=beb[2])

            r1T = persist.tile([P, N], BF16, tag="r1T")
            for c in range(2):
                psc = ps512.tile([P, 512], F32, tag="ps512")
                nc.tensor.matmul(
                    psc, w1_sb,
                    oT[:, 4 * c:4 * (c + 1), :].rearrange("p t f -> p (t f)"),
                    start=True, stop=True)
                nc.scalar.activation(
                    out=r1T[:, c * 512:(c + 1) * 512], in_=psc, func=AF.Relu,
                    bias=b1_sb)
            r2T = persist.tile([P, N], BF16, tag="r2T")
            for c in range(2):
                psc = ps512.tile([P, 512], F32, tag="ps512")
                nc.tensor.matmul(
                    psc, w2_sb, r1T[:, c * 512:(c + 1) * 512],
                    start=True, stop=True)
                nc.scalar.activation(
                    out=r2T[:, c * 512:(c + 1) * 512], in_=psc, func=AF.Relu,
                    bias=b2_sb)

            r2tok = persist.tile([P, NT, D], BF16, tag="r2tok")
            mvR = small.tile([P, NT, 2], F32, tag="mvR", bufs=1)
            rstdR = small.tile([P, NT], F32, tag="rstdR", bufs=1)
            out_sb = persist.tile([P, NT, D], F32, tag="osb")
            for i in range(NT):
                ptb = psTe.tile([P, P], BF16, tag="pte")
                nc.tensor.transpose(ptb, r2T[:, i * P:(i + 1) * P], identb)
                nc.scalar.activation(out=r2tok[:, i, :], in_=ptb, func=AF.Copy)
                s6 = small.tile([P, 6], F32, tag="s6")
                nc.vector.bn_stats(out=s6, in_=r2tok[:, i, :])
                nc.vector.bn_aggr(out=mvR[:, i, :], in_=s6)
                lnv = small.tile([P, 1], F32, tag="lnv")
                nc.scalar.activation(out=lnv, in_=mvR[:, i, 1:2], func=AF.Ln,
                                     bias=eps_t)
                nc.scalar.activation(out=rstdR[:, i:i + 1], in_=lnv,
                                     func=AF.Exp, scale=-0.5)
            for i in range(NT):
                ro = stg.tile([P, D], BF16, name="ro", tag="ro")
                nc.vector.tensor_scalar(
                    out=ro, in0=r2tok[:, i, :], scalar1=mvR[:, i, 0:1],
                    scalar2=rstdR[:, i:i + 1], op0=OP.subtract, op1=OP.mult)
                nc.vector.tensor_mul(out=ro, in0=ro, in1=gb[3])
                nc.vector.tensor_add(out=out_sb[:, i, :], in0=obe[:, i, :],
                                     in1=ro)
                nc.sync.dma_start(
                    out=out_dram.rearrange("(t p) d -> p t d", p=P)[:, i, :],
                    in_=out_sb[:, i, :])

    nc.compile()
    return nc


def _get_program():
    if "nc" not in _prog_cache:
        _prog_cache["nc"] = _build_program()
    return _prog_cache["nc"]


def kernel(**inputs):
    nc = _get_program()
    f32 = np.float32
    bf16 = ml_dtypes.bfloat16
    f8 = ml_dtypes.float8_e4m3fn

    H = np.asarray(inputs["H"], dtype=f32)
    A = np.asarray(inputs["A"], dtype=f32)
    g1 = np.asarray(inputs["g1"], dtype=f32)
    be1 = np.asarray(inputs["be1"], dtype=f32)
    WO = np.asarray(inputs["W_O"], dtype=f32)
    # fold LN1 gain/bias into the output projection
    WO_fold = WO * np.tile(g1, HEADS)[:, None]
    bO = np.tile(be1, HEADS) @ WO

    BT = np.asarray(inputs["B_bias"], dtype=f32).transpose(0, 2, 1)
    base = {
        "bt": np.ascontiguousarray(BT * 16.0).astype(f8),
        "wq": (np.asarray(inputs["W_Q"], dtype=f32) * DK).astype(bf16),
        "wk": np.asarray(inputs["W_K"], dtype=f32).astype(bf16),
        "wv": np.asarray(inputs["W_V"], dtype=f32).astype(bf16),
        "bqr": np.ascontiguousarray(
            (np.asarray(inputs["b_Q"], dtype=f32) * DK).reshape(NT, P).T),
        "bkr": np.ascontiguousarray(
            np.asarray(inputs["b_K"], dtype=f32).reshape(NT, P).T),
        "bv": np.asarray(inputs["b_V"], dtype=f32),
        "wo": WO_fold.astype(bf16),
        "bo": bO.reshape(1, D).astype(bf16),
        "w1": (np.asarray(inputs["g2"], dtype=f32)[:, None]
               * np.asarray(inputs["W1"], dtype=f32)).astype(bf16),
        "w2": np.asarray(inputs["W2"], dtype=f32).astype(bf16),
        "b1": (np.asarray(inputs["b1"], dtype=f32)
               + np.asarray(inputs["be2"], dtype=f32)
               @ np.asarray(inputs["W1"], dtype=f32)).reshape(D, 1),
        "b2": np.asarray(inputs["b2"], dtype=f32).reshape(D, 1),
    }
    for i in (0, 2, 3):
        base[f"g{i}"] = np.asarray(inputs[f"g{i}"], dtype=f32).astype(bf16)
        base[f"be{i}"] = np.asarray(inputs[f"be{i}"], dtype=f32).astype(bf16)
    base["be2"] = (np.asarray(inputs["be2"], dtype=f32)
                   + np.asarray(inputs["be3"], dtype=f32)).astype(bf16)

    in_maps = []
    for c in range(B):
        m = dict(base)
        m["h"] = H[c].astype(bf16)
        m["at"] = np.ascontiguousarray(A[c].T).astype(bf16)
        in_maps.append(m)

    res = run_bass_kernel_spmd(nc, in_maps, list(range(B)))
    out = np.stack([res.results[c]["out"] for c in range(B)], axis=0)
    return out.astype(np.float32)


if __name__ == "__main__":
    nc = _get_program()
    print("program built ok")
    from concourse.timeline_sim import TimelineSim
    ns = TimelineSim(nc, trace=False).simulate()
    print(f"TimelineSim: {ns:.0f} ns")


# revision 37
# speedup vs baseline: 3.0520x; 1.0332x over previous
"""Trainium2 Bass kernel for nn_GRIC_31550829756424 (GCN-attention block).

Data-parallel over batch: 8 batches -> 8 NeuronCores, one full batch per core.

Key structure (v2):
- GCN reassociated: adjHnT = Hn^T @ adj_norm^T computed once (shared by
  Q/K/V), then Q/K/V are single-step K=128 matmuls.  A is host-transposed
  and bf16; both degree scalings fold into A^T via one scalar_tensor_tensor.
- All rsqrt computed as exp(-0.5*ln(x)) so the ONLY activation table used is
  natural_log_exp_and_others (Exp/Ln/Relu/Copy/Identity) -> 1 table load.
- Attention bias B added into the QK PSUM via an fp8 DoubleRow matmul
  (identity/32 stationary, bias*16 moving) instead of DVE adds.
- LN1 gain/bias folded into W_O on the host; MH transposed via DMA XBAR.

Self-contained: hardcodes all shapes; imports only the in-container concourse
stack.
"""

import sys

sys.path.insert(0, "/opt/trn_rl_repo")

import numpy as np
import ml_dtypes
from contextlib import ExitStack

import concourse.bass as bass
import concourse.tile as tile
from concourse import bacc
from concourse import mybir
from concourse.bass_utils import run_bass_kernel_spmd
from concourse.masks import make_identity

F32 = mybir.dt.float32
BF16 = mybir.dt.bfloat16
F8 = mybir.dt.float8e4
AF = mybir.ActivationFunctionType
OP = mybir.AluOpType
PM = mybir.MatmulPerfMode

B = 8
N = 1024
D = 128
HEADS = 8
DV = 128
HD = HEADS * DV  # 1024
P = 128
NT = N // P  # 8 tiles of 128 rows
DK = 1.0 / float(np.sqrt(np.float32(D)))
EPS = 1e-5

_prog_cache = {}


def _bcast_load(nc, dst, src):
    """DMA-load 1D DRAM vector src [W] replicated across all P partitions of
    dst [P, W] (same dtype)."""
    rep = bass.AP(tensor=src.tensor, offset=src.offset, ap=[[0, P]] + list(src.ap))
    nc.gpsimd.dma_start(out=dst, in_=rep)


def _dup2(ap):
    """View a [P, W] AP as [P, 2, W] with a stride-0 middle dim (DoubleRow
    moving operand reading the same data in both slots)."""
    return bass.AP(
        tensor=ap.tensor, offset=ap.offset,
        ap=[list(ap.ap[0]), [0, 2]] + [list(a) for a in ap.ap[1:]],
    )


def _build_program():
    nc = bacc.Bacc(None)

    h_in = nc.declare_dram_parameter("h", [N, D], BF16, isOutput=False)
    at_in = nc.declare_dram_parameter("at", [N, N], BF16, isOutput=False)
    bt_in = nc.declare_dram_parameter("bt", [HEADS, N, N], F8, isOutput=False)
    wq_in = nc.declare_dram_parameter("wq", [D, HD], BF16, isOutput=False)
    wk_in = nc.declare_dram_parameter("wk", [D, HD], BF16, isOutput=False)
    wv_in = nc.declare_dram_parameter("wv", [D, HD], BF16, isOutput=False)
    bqr_in = nc.declare_dram_parameter("bqr", [P, NT], F32, isOutput=False)
    bkr_in = nc.declare_dram_parameter("bkr", [P, NT], F32, isOutput=False)
    bv_in = nc.declare_dram_parameter("bv", [HD], F32, isOutput=False)
    wo_in = nc.declare_dram_parameter("wo", [HD, D], BF16, isOutput=False)
    bo_in = nc.declare_dram_parameter("bo", [1, D], BF16, isOutput=False)
    w1_in = nc.declare_dram_parameter("w1", [D, D], BF16, isOutput=False)
    w2_in = nc.declare_dram_parameter("w2", [D, D], BF16, isOutput=False)
    b1_in = nc.declare_dram_parameter("b1", [D, 1], F32, isOutput=False)
    b2_in = nc.declare_dram_parameter("b2", [D, 1], F32, isOutput=False)
    g_in = {}
    be_in = {}
    for i in (0, 2, 3):
        g_in[i] = nc.declare_dram_parameter(f"g{i}", [D], BF16, isOutput=False)
        be_in[i] = nc.declare_dram_parameter(f"be{i}", [D], BF16, isOutput=False)
    out_dram = nc.declare_dram_parameter("out", [N, D], F32, isOutput=True)

    with tile.TileContext(nc) as tc, ExitStack() as ctx:
        consts = ctx.enter_context(tc.tile_pool(name="consts", bufs=1))
        persist = ctx.enter_context(tc.tile_pool(name="persist", bufs=1))
        small = ctx.enter_context(tc.tile_pool(name="small", bufs=12))
        stg = ctx.enter_context(tc.tile_pool(name="stg", bufs=3))
        # 2-bank 512-wide PSUM pool: open through B/C, reused in E via scope.
        ps512 = ctx.enter_context(
            tc.tile_pool(name="ps512", bufs=2, space=bass.MemorySpace.PSUM))

        # ---- constants -------------------------------------------------
        identb = consts.tile([P, P], BF16)
        make_identity(nc, identb)
        omib = consts.tile([P, P], BF16)  # 1 - I
        nc.gpsimd.memset(omib, 1.0)
        nc.gpsimd.affine_select(
            out=omib, in_=omib, compare_op=OP.not_equal, fill=0.0,
            base=0, pattern=[[-1, P]], channel_multiplier=1)
        # fp8 DoubleRow stationary: two slots of I/32 (moving is bias*16).
        id2 = consts.tile([P, 2, P], F8)
        nc.gpsimd.memset(id2, 0.0)
        for s in range(2):
            nc.gpsimd.affine_select(
                out=id2[:, s, :], in_=id2[:, s, :], compare_op=OP.not_equal,
                fill=1.0 / 32.0, base=0, pattern=[[-1, P]], channel_multiplier=1)
        nc.scalar.add_instruction(
            mybir.InstLoadActFuncSet(
                name=nc.get_next_instruction_name(), ins=[], outs=[],
                act_func_set_id=6))
        eps_t = consts.tile([P, 1], F32)
        nc.vector.memset(eps_t, EPS)
        onesb = consts.tile([1, P], BF16)
        nc.vector.memset(onesb, 1.0)
        onescol = consts.tile([P, 1], BF16)
        nc.vector.memset(onescol, 1.0)

        gb = {}
        beb = {}
        for i in (0, 2, 3):
            gb[i] = consts.tile([P, D], BF16, name=f"g{i}b", tag=f"g{i}b")
            _bcast_load(nc, gb[i], g_in[i][:])
            beb[i] = consts.tile([P, D], BF16, name=f"be{i}b", tag=f"be{i}b")
            _bcast_load(nc, beb[i], be_in[i][:])
        bvb = consts.tile([P, HD], F32)
        _bcast_load(nc, bvb, bv_in[:])
        bqr = consts.tile([P, NT], F32)
        nc.gpsimd.dma_start(out=bqr, in_=bqr_in[:, :])
        bkr = consts.tile([P, NT], F32)
        nc.gpsimd.dma_start(out=bkr, in_=bkr_in[:, :])
        wq_sb = consts.tile([P, HD], BF16)
        nc.gpsimd.dma_start(out=wq_sb, in_=wq_in[:, :])
        wk_sb = consts.tile([P, HD], BF16)
        nc.gpsimd.dma_start(out=wk_sb, in_=wk_in[:, :])
        wv_sb = consts.tile([P, HD], BF16)
        nc.gpsimd.dma_start(out=wv_sb, in_=wv_in[:, :])
        # W_O as [p=dv-within-head, h, d]
        wo_sb = consts.tile([P, HEADS, D], BF16)
        nc.sync.dma_start(
            out=wo_sb, in_=wo_in.rearrange("(hh p) d -> p hh d", p=P))
        bo_sb = consts.tile([1, D], BF16)
        nc.gpsimd.dma_start(out=bo_sb, in_=bo_in[:, :])
        w1_sb = consts.tile([P, D], BF16)
        nc.gpsimd.dma_start(out=w1_sb, in_=w1_in[:, :])
        w2_sb = consts.tile([P, D], BF16)
        nc.gpsimd.dma_start(out=w2_sb, in_=w2_in[:, :])
        b1_sb = consts.tile([P, 1], F32)
        nc.gpsimd.dma_start(out=b1_sb, in_=b1_in[:, :])
        b2_sb = consts.tile([P, 1], F32)
        nc.gpsimd.dma_start(out=b2_sb, in_=b2_in[:, :])

        # ---- persistent tensors ---------------------------------------
        h_sb = persist.tile([P, NT, D], BF16, tag="h")
        hn_t = persist.tile([P, NT, D], BF16, tag="hnt")
        ats = persist.tile([P, NT, N], BF16, tag="ats")  # A^T -> adj_norm^T
        for j in range(NT):
            nc.sync.dma_start(
                out=ats[:, j, :],
                in_=at_in[:, :].rearrange("(t p) n -> p t n", p=P)[:, j, :])
        nc.sync.dma_start(out=h_sb, in_=h_in.rearrange("(t p) d -> p t d", p=P))
        disb = persist.tile([P, N], BF16, tag="disb")
        dis_tok = persist.tile([P, NT], F32, tag="distok")
        adjHnT = persist.tile([P, N], BF16, tag="adjhnt")
        qT = persist.tile([P, HEADS, N], BF16, tag="qT")
        kT = persist.tile([P, HEADS, N], BF16, tag="kT")
        vna = persist.tile([P, NT, HEADS, DV + 1], BF16, tag="v")
        nc.vector.memset(vna[:, :, :, DV:DV + 1], 1.0)
        mhcT = persist.tile([P, HEADS, N], BF16, tag="mhcT")

        # ---- phase B part 1: diagonal fix + rowsums ---------------------
        with tc.tile_pool(name="psR", bufs=1, space=bass.MemorySpace.PSUM) as psR, \
             tc.tile_pool(name="psTb", bufs=2, space=bass.MemorySpace.PSUM) as psTb:
            rs_ps = [psR.tile([1, 512], F32, name=f"rsps{c}", tag=f"rsps{c}")
                     for c in range(2)]
            for j in range(NT):
                db = ats[:, j, j * P:(j + 1) * P]
                nc.vector.tensor_mul(out=db, in0=db, in1=omib)
                nc.vector.tensor_add(out=db, in0=db, in1=identb)
                for c in range(2):
                    nc.tensor.matmul(
                        rs_ps[c], onescol, ats[:, j, c * 512:(c + 1) * 512],
                        start=(j == 0), stop=(j == NT - 1))
            # -- phase A: H LayerNorm (interleaved) --
            for i in range(NT):
                s6 = small.tile([P, 6], F32, tag="s6")
                mv = small.tile([P, 2], F32, tag="mv")
                nc.vector.bn_stats(out=s6, in_=h_sb[:, i, :])
                nc.vector.bn_aggr(out=mv, in_=s6)
                lnv = small.tile([P, 1], F32, tag="lnv")
                nc.scalar.activation(out=lnv, in_=mv[:, 1:2], func=AF.Ln, bias=eps_t)
                rstd = small.tile([P, 1], F32, tag="rstd")
                nc.scalar.activation(out=rstd, in_=lnv, func=AF.Exp, scale=-0.5)
                hw = stg.tile([P, D], BF16, name="hw", tag="hw")
                nc.vector.tensor_scalar(
                    out=hw, in0=h_sb[:, i, :], scalar1=mv[:, 0:1], scalar2=rstd,
                    op0=OP.subtract, op1=OP.mult)
                nc.vector.tensor_mul(out=hw, in0=hw, in1=gb[0])
                nc.vector.tensor_add(out=hn_t[:, i, :], in0=hw, in1=beb[0])

            rs_sb = small.tile([1, N], F32, tag="rssb", bufs=1)
            for c in range(2):
                nc.vector.tensor_scalar_max(
                    out=rs_sb[:, c * 512:(c + 1) * 512], in0=rs_ps[c], scalar1=1.0)
            lnr = small.tile([1, N], F32, tag="lnr", bufs=1)
            nc.scalar.activation(out=lnr, in_=rs_sb, func=AF.Ln)
            disrow = small.tile([1, N], BF16, tag="disrow", bufs=1)
            nc.scalar.activation(out=disrow, in_=lnr, func=AF.Exp, scale=-0.5)
            # broadcast di over partitions: disb[q, n] = di_n
            for c in range(2):
                psd = ps512.tile([P, 512], F32, tag="ps512")
                nc.tensor.matmul(
                    psd, onesb, disrow[:, c * 512:(c + 1) * 512],
                    start=True, stop=True)
                nc.scalar.activation(
                    out=disb[:, c * 512:(c + 1) * 512], in_=psd, func=AF.Copy)
            # dis_tok[p, j] = di_{j*P+p} via PE transpose of disb chunks
            for j in range(NT):
                ptb = psTb.tile([P, P], BF16, tag="ptb")
                nc.tensor.transpose(ptb, disb[:, j * P:(j + 1) * P], identb)
                nc.scalar.activation(out=dis_tok[:, j:j + 1], in_=ptb[:, 0:1],
                                     func=AF.Copy)
            # fold di_m into hn_t (per-partition) instead of scaling ats
            for j in range(NT):
                nc.vector.tensor_scalar_mul(
                    out=hn_t[:, j, :], in0=hn_t[:, j, :],
                    scalar1=dis_tok[:, j:j + 1])

        # ---- phase C: shared GCN trunk ----------------------------------
        # adjHnT[d, n] = sum_m hn[m, d] * adjn^T[m, n]
        for c in range(2):
            psc = ps512.tile([P, 512], F32, tag="ps512")
            for j in range(NT):
                nc.tensor.matmul(
                    psc, hn_t[:, j, :], ats[:, j, c * 512:(c + 1) * 512],
                    start=(j == 0), stop=(j == NT - 1))
            nc.vector.tensor_mul(
                out=adjHnT[:, c * 512:(c + 1) * 512], in0=psc,
                in1=disb[:, c * 512:(c + 1) * 512])

        def emit_qk(hh, q_on_act=False):
            for c in range(2):
                psc = ps512.tile([P, 512], F32, tag="ps512")
                nc.tensor.matmul(
                    psc, wq_sb[:, hh * P:(hh + 1) * P],
                    adjHnT[:, c * 512:(c + 1) * 512], start=True, stop=True)
                if q_on_act:
                    nc.scalar.activation(
                        out=qT[:, hh, c * 512:(c + 1) * 512], in_=psc,
                        func=AF.Identity, bias=bqr[:, hh:hh + 1])
                else:
                    nc.vector.tensor_scalar_add(
                        out=qT[:, hh, c * 512:(c + 1) * 512], in0=psc,
                        scalar1=bqr[:, hh:hh + 1])
                psc = ps512.tile([P, 512], F32, tag="ps512")
                nc.tensor.matmul(
                    psc, wk_sb[:, hh * P:(hh + 1) * P],
                    adjHnT[:, c * 512:(c + 1) * 512], start=True, stop=True)
                nc.vector.tensor_scalar_add(
                    out=kT[:, hh, c * 512:(c + 1) * 512], in0=psc,
                    scalar1=bkr[:, hh:hh + 1])

        emit_qk(0, q_on_act=True)

        def emit_v(c, i0=0, i1=NT):
            for i in range(i0, i1):
                psc = ps512.tile([P, 512], F32, tag="ps512")
                nc.tensor.matmul(
                    psc, adjHnT[:, i * P:(i + 1) * P],
                    wv_sb[:, c * 512:(c + 1) * 512], start=True, stop=True)
                nc.vector.tensor_add(
                    out=vna[:, i, c * 4:(c + 1) * 4, 0:DV],
                    in0=psc.rearrange("p (a b) -> p a b", a=4),
                    in1=bvb[:, c * 512:(c + 1) * 512].rearrange(
                        "p (a b) -> p a b", a=4))

        emit_v(0)

        # ---- phase D: attention, software-pipelined over heads ----------
        with tc.tile_pool(name="psE", bufs=2, space=bass.MemorySpace.PSUM) as psE, \
             tc.tile_pool(name="psPM", bufs=2, space=bass.MemorySpace.PSUM) as psPM, \
             tc.tile_pool(name="etp", bufs=3) as etp, \
             tc.tile_pool(name="btp", bufs=2) as btp, \
             tc.tile_pool(name="mhp", bufs=2) as mhp:
            ets = {}
            mhs = {}
            mvss = {}
            vees = {}
            bt0 = btp.tile([P, NT, N], F8, tag="bt", name="bt0")
            nc.sync.dma_start(
                out=bt0, in_=bt_in[0].rearrange("(t p) n -> p t n", p=P))
            bts = {0: bt0}
            for stage in range(HEADS + 1):
                hh = stage
                if hh < HEADS:
                    et = etp.tile([P, NT, N], BF16, tag="et")
                    ets[hh] = et
                    btile = bts[hh]
                    for j in range(NT):
                        pse = psE.tile([P, N], F32, tag="pse")
                        for c in range(2):
                            nc.tensor.matmul(
                                pse[:, c * 512:(c + 1) * 512],
                                kT[:, hh, j * P:(j + 1) * P],
                                qT[:, hh, c * 512:(c + 1) * 512],
                                start=True, stop=False)
                            nc.tensor.matmul(
                                pse[:, c * 512:(c + 1) * 512],
                                id2, _dup2(btile[:, j, c * 512:(c + 1) * 512]),
                                start=False, stop=True, perf_mode=PM.DoubleRow)
                        nc.scalar.activation(out=et[:, j, :], in_=pse,
                                             func=AF.Exp)
                    if hh + 1 < HEADS:
                        btn = btp.tile([P, NT, N], F8, tag="bt",
                                       name=f"bt{hh + 1}")
                        nc.sync.dma_start(
                            out=btn,
                            in_=bt_in[hh + 1].rearrange("(t p) n -> p t n", p=P))
                        bts[hh + 1] = btn
                        emit_qk(hh + 1)
                if 1 <= stage <= 4:
                    emit_v(1, (stage - 1) * 2, stage * 2)
                if stage >= 1:
                    ph = stage - 1  # head whose PV/LN we process now
                    et = ets.pop(ph)
                    mh_sub = mhp.tile([P, NT, DV], BF16, tag="mh")
                    mhs[ph] = mh_sub
                    mvs = small.tile([P, NT, 2], F32, tag="mvs", bufs=2)
                    mvss[ph] = mvs
                    vee = small.tile([P, NT], F32, tag="vee", bufs=2)
                    vees[ph] = vee
                    for i in range(NT):
                        pm = psPM.tile([P, DV + 1], F32, tag="pm")
                        for j in range(NT):
                            nc.tensor.matmul(
                                pm, et[:, j, i * P:(i + 1) * P],
                                vna[:, j, ph, :],
                                start=(j == 0), stop=(j == NT - 1))
                        s6 = small.tile([P, 6], F32, tag="s6")
                        nc.vector.bn_stats(out=s6, in_=pm[:, 0:DV])
                        nc.vector.bn_aggr(out=mvs[:, i, :], in_=s6)
                        t = small.tile([P, 1], F32, tag="t")
                        nc.vector.tensor_scalar(
                            out=t, in0=pm[:, DV:DV + 1],
                            scalar1=pm[:, DV:DV + 1], scalar2=EPS,
                            op0=OP.mult, op1=OP.mult)
                        # mean-subtract now (frees pm); rstd scale later
                        nc.vector.tensor_scalar(
                            out=mh_sub[:, i, :], in0=pm[:, 0:DV],
                            scalar1=mvs[:, i, 0:1], scalar2=None,
                            op0=OP.subtract)
                        nc.gpsimd.tensor_add(
                            out=vee[:, i:i + 1], in0=t, in1=mvs[:, i, 1:2])
                    # batched rstd for the whole head (after next head's Exps
                    # in ACT program order -> no convoy)
                    lnv8 = small.tile([P, NT], F32, tag="lnv8", bufs=2)
                    nc.scalar.activation(out=lnv8, in_=vee, func=AF.Ln)
                    rstd8 = small.tile([P, NT], F32, tag="rstd8", bufs=2)
                    nc.scalar.activation(out=rstd8, in_=lnv8, func=AF.Exp,
                                         scale=-0.5)
                    for i in range(NT):
                        nc.gpsimd.tensor_scalar_mul(
                            out=mh_sub[:, i, :], in0=mh_sub[:, i, :],
                            scalar1=rstd8[:, i:i + 1])
                    for half in range(2):
                        nc.sync.dma_start(
                            out=mhcT[:, ph, half * 512:(half + 1) * 512]
                            .rearrange("p (t f) -> p t f", t=NT // 2),
                            in_=mh_sub[:, half * 4:(half + 1) * 4, :],
                            transpose=True)

        # ---- phase E: output projection + MLP ---------------------------
        o_ln = persist.tile([P, NT, D], BF16, tag="oln")
        obe = persist.tile([P, NT, D], BF16, tag="obe")
        mvE = small.tile([P, NT, 2], F32, tag="mvE", bufs=1)
        rstdE = small.tile([P, NT], F32, tag="rstdE", bufs=1)
        with tc.tile_pool(name="psO", bufs=4, space=bass.MemorySpace.PSUM) as psO, \
             tc.tile_pool(name="psTe", bufs=2, space=bass.MemorySpace.PSUM) as psTe:
            oT = persist.tile([P, NT, P], BF16, tag="oT")
            for i in range(NT):
                pso = psO.tile([P, D], F32, tag="pso")
                for hh in range(HEADS):
                    nc.tensor.matmul(
                        pso, mhcT[:, hh, i * P:(i + 1) * P], wo_sb[:, hh, :],
                        start=(hh == 0), stop=False)
                nc.tensor.matmul(pso, onesb, bo_sb, start=False, stop=False)
                nc.tensor.matmul(pso, identb, h_sb[:, i, :],
                                 start=False, stop=True)
                s6 = small.tile([P, 6], F32, tag="s6")
                nc.vector.bn_stats(out=s6, in_=pso)
                nc.vector.bn_aggr(out=mvE[:, i, :], in_=s6)
                lnv = small.tile([P, 1], F32, tag="lnv")
                nc.scalar.activation(out=lnv, in_=mvE[:, i, 1:2], func=AF.Ln,
                                     bias=eps_t)
                nc.scalar.activation(out=rstdE[:, i:i + 1], in_=lnv,
                                     func=AF.Exp, scale=-0.5)
                nc.vector.tensor_scalar(
                    out=o_ln[:, i, :], in0=pso,
                    scalar1=mvE[:, i, 0:1], scalar2=rstdE[:, i:i + 1],
                    op0=OP.subtract, op1=OP.mult)
                ptb = psTe.tile([P, P], BF16, tag="pte")
                nc.tensor.transpose(ptb, o_ln[:, i, :], identb)
                nc.vector.tensor_copy(out=oT[:, i, :], in_=ptb)
                nc.gpsimd.tensor_mul(out=obe[:, i, :], in0=o_ln[:, i, :],
                                     in1=gb[2])
                nc.gpsimd.tensor_add(out=obe[:, i, :], in0=obe[:, i, :],
                                     in1=beb[2])
            r1T = persist.tile([P, N], BF16, tag="r1T")
            for c in range(2):
                psc = ps512.tile([P, 512], F32, tag="ps512")
                nc.tensor.matmul(
                    psc, w1_sb,
                    oT[:, 4 * c:4 * (c + 1), :].rearrange("p t f -> p (t f)"),
                    start=True, stop=True)
                nc.scalar.activation(
                    out=r1T[:, c * 512:(c + 1) * 512], in_=psc, func=AF.Relu,
                    bias=b1_sb)
            r2T = persist.tile([P, N], BF16, tag="r2T")
            for c in range(2):
                psc = ps512.tile([P, 512], F32, tag="ps512")
                nc.tensor.matmul(
                    psc, w2_sb, r1T[:, c * 512:(c + 1) * 512],
                    start=True, stop=True)
                nc.scalar.activation(
                    out=r2T[:, c * 512:(c + 1) * 512], in_=psc, func=AF.Relu,
                    bias=b2_sb)

            r2tok = persist.tile([P, NT, D], BF16, tag="r2tok")
            mvR = small.tile([P, NT, 2], F32, tag="mvR", bufs=1)
            rstdR = small.tile([P, NT], F32, tag="rstdR", bufs=1)
            out_sb = persist.tile([P, NT, D], F32, tag="osb")
            for i in range(NT):
                ptb = psTe.tile([P, P], BF16, tag="pte")
                nc.tensor.transpose(ptb, r2T[:, i * P:(i + 1) * P], identb)
                nc.scalar.activation(out=r2tok[:, i, :], in_=ptb, func=AF.Copy)
                s6 = small.tile([P, 6], F32, tag="s6")
                nc.vector.bn_stats(out=s6, in_=r2tok[:, i, :])
                nc.vector.bn_aggr(out=mvR[:, i, :], in_=s6)
                lnv = small.tile([P, 1], F32, tag="lnv")
                nc.scalar.activation(out=lnv, in_=mvR[:, i, 1:2], func=AF.Ln,
                                     bias=eps_t)
                nc.scalar.activation(out=rstdR[:, i:i + 1], in_=lnv,
                                     func=AF.Exp, scale=-0.5)
            for i in range(NT):
                ro = stg.tile([P, D], BF16, name="ro", tag="ro")
                nc.vector.tensor_scalar(
                    out=ro, in0=r2tok[:, i, :], scalar1=mvR[:, i, 0:1],
                    scalar2=rstdR[:, i:i + 1], op0=OP.subtract, op1=OP.mult)
                nc.vector.tensor_mul(out=ro, in0=ro, in1=gb[3])
                nc.vector.tensor_add(out=out_sb[:, i, :], in0=obe[:, i, :],
                                     in1=ro)
                if i % 4 == 3:
                    nc.sync.dma_start(
                        out=out_dram.rearrange(
                            "(t p) d -> p t d", p=P)[:, i - 3:i + 1, :],
                        in_=out_sb[:, i - 3:i + 1, :])

    nc.compile()
    return nc


def _get_program():
    if "nc" not in _prog_cache:
        _prog_cache["nc"] = _build_program()
    return _prog_cache["nc"]


def kernel(**inputs):
    nc = _get_program()
    f32 = np.float32
    bf16 = ml_dtypes.bfloat16
    f8 = ml_dtypes.float8_e4m3fn

    H = np.asarray(inputs["H"], dtype=f32)
    A = np.asarray(inputs["A"], dtype=f32)
    g1 = np.asarray(inputs["g1"], dtype=f32)
    be1 = np.asarray(inputs["be1"], dtype=f32)
    WO = np.asarray(inputs["W_O"], dtype=f32)
    # fold LN1 gain/bias into the output projection
    WO_fold = WO * np.tile(g1, HEADS)[:, None]
    bO = np.tile(be1, HEADS) @ WO

    BT = np.asarray(inputs["B_bias"], dtype=f32).transpose(0, 2, 1)
    base = {
        "bt": np.ascontiguousarray(BT * 16.0).astype(f8),
        "wq": (np.asarray(inputs["W_Q"], dtype=f32) * DK).astype(bf16),
        "wk": np.asarray(inputs["W_K"], dtype=f32).astype(bf16),
        "wv": np.asarray(inputs["W_V"], dtype=f32).astype(bf16),
        "bqr": np.ascontiguousarray(
            (np.asarray(inputs["b_Q"], dtype=f32) * DK).reshape(NT, P).T),
        "bkr": np.ascontiguousarray(
            np.asarray(inputs["b_K"], dtype=f32).reshape(NT, P).T),
        "bv": np.asarray(inputs["b_V"], dtype=f32),
        "wo": WO_fold.astype(bf16),
        "bo": bO.reshape(1, D).astype(bf16),
        "w1": (np.asarray(inputs["g2"], dtype=f32)[:, None]
               * np.asarray(inputs["W1"], dtype=f32)).astype(bf16),
        "w2": np.asarray(inputs["W2"], dtype=f32).astype(bf16),
        "b1": (np.asarray(inputs["b1"], dtype=f32)
               + np.asarray(inputs["be2"], dtype=f32)
               @ np.asarray(inputs["W1"], dtype=f32)).reshape(D, 1),
        "b2": np.asarray(inputs["b2"], dtype=f32).reshape(D, 1),
    }
    for i in (0, 2, 3):
        base[f"g{i}"] = np.asarray(inputs[f"g{i}"], dtype=f32).astype(bf16)
        base[f"be{i}"] = np.asarray(inputs[f"be{i}"], dtype=f32).astype(bf16)
    base["be2"] = (np.asarray(inputs["be2"], dtype=f32)
                   + np.asarray(inputs["be3"], dtype=f32)).astype(bf16)

    in_maps = []
    for c in range(B):
        m = dict(base)
        m["h"] = H[c].astype(bf16)
        m["at"] = np.ascontiguousarray(A[c].T).astype(bf16)
        in_maps.append(m)

    res = run_bass_kernel_spmd(nc, in_maps, list(range(B)))
    out = np.stack([res.results[c]["out"] for c in range(B)], axis=0)
    return out.astype(np.float32)


if __name__ == "__main__":
    nc = _get_program()
    print("program built ok")
    from concourse.timeline_sim import TimelineSim
    ns = TimelineSim(nc, trace=False).simulate()
    print(f"TimelineSim: {ns:.0f} ns")


# revision 51
# speedup vs baseline: 3.0725x; 1.0067x over previous
"""Trainium2 Bass kernel for nn_GRIC_31550829756424 (GCN-attention block).

Data-parallel over batch: 8 batches -> 8 NeuronCores, one full batch per core.

Key structure (v2):
- GCN reassociated: adjHnT = Hn^T @ adj_norm^T computed once (shared by
  Q/K/V), then Q/K/V are single-step K=128 matmuls.  A is host-transposed
  and bf16; both degree scalings fold into A^T via one scalar_tensor_tensor.
- All rsqrt computed as exp(-0.5*ln(x)) so the ONLY activation table used is
  natural_log_exp_and_others (Exp/Ln/Relu/Copy/Identity) -> 1 table load.
- Attention bias B added into the QK PSUM via an fp8 DoubleRow matmul
  (identity/32 stationary, bias*16 moving) instead of DVE adds.
- LN1 gain/bias folded into W_O on the host; MH transposed via DMA XBAR.

Self-contained: hardcodes all shapes; imports only the in-container concourse
stack.
"""

import sys

sys.path.insert(0, "/opt/trn_rl_repo")

import numpy as np
import ml_dtypes
from contextlib import ExitStack

import concourse.bass as bass
import concourse.tile as tile
from concourse import bacc
from concourse import mybir
from concourse.bass_utils import run_bass_kernel_spmd
from concourse.masks import make_identity

F32 = mybir.dt.float32
BF16 = mybir.dt.bfloat16
F8 = mybir.dt.float8e4
AF = mybir.ActivationFunctionType
OP = mybir.AluOpType
PM = mybir.MatmulPerfMode

B = 8
N = 1024
D = 128
HEADS = 8
DV = 128
HD = HEADS * DV  # 1024
P = 128
NT = N // P  # 8 tiles of 128 rows
DK = 1.0 / float(np.sqrt(np.float32(D)))
EPS = 1e-5

_prog_cache = {}


def _bcast_load(nc, dst, src):
    """DMA-load 1D DRAM vector src [W] replicated across all P partitions of
    dst [P, W] (same dtype)."""
    rep = bass.AP(tensor=src.tensor, offset=src.offset, ap=[[0, P]] + list(src.ap))
    nc.gpsimd.dma_start(out=dst, in_=rep)


def _dup2(ap):
    """View a [P, W] AP as [P, 2, W] with a stride-0 middle dim (DoubleRow
    moving operand reading the same data in both slots)."""
    return bass.AP(
        tensor=ap.tensor, offset=ap.offset,
        ap=[list(ap.ap[0]), [0, 2]] + [list(a) for a in ap.ap[1:]],
    )


def _build_program():
    nc = bacc.Bacc(None)

    h_in = nc.declare_dram_parameter("h", [N, D], BF16, isOutput=False)
    at_in = nc.declare_dram_parameter("at", [N, N], BF16, isOutput=False)
    bt_in = nc.declare_dram_parameter("bt", [HEADS, N, N], F8, isOutput=False)
    wq_in = nc.declare_dram_parameter("wq", [D, HD], BF16, isOutput=False)
    wk_in = nc.declare_dram_parameter("wk", [D, HD], BF16, isOutput=False)
    wv_in = nc.declare_dram_parameter("wv", [D, HD], BF16, isOutput=False)
    bqr_in = nc.declare_dram_parameter("bqr", [P, NT], F32, isOutput=False)
    bkr_in = nc.declare_dram_parameter("bkr", [P, NT], F32, isOutput=False)
    bv_in = nc.declare_dram_parameter("bv", [HD], F32, isOutput=False)
    wo_in = nc.declare_dram_parameter("wo", [HD, D], BF16, isOutput=False)
    bo_in = nc.declare_dram_parameter("bo", [1, D], BF16, isOutput=False)
    w1_in = nc.declare_dram_parameter("w1", [D, D], BF16, isOutput=False)
    w2_in = nc.declare_dram_parameter("w2", [D, D], BF16, isOutput=False)
    b1_in = nc.declare_dram_parameter("b1", [D, 1], F32, isOutput=False)
    b2_in = nc.declare_dram_parameter("b2", [D, 1], F32, isOutput=False)
    g_in = {}
    be_in = {}
    for i in (0, 2, 3):
        g_in[i] = nc.declare_dram_parameter(f"g{i}", [D], BF16, isOutput=False)
        be_in[i] = nc.declare_dram_parameter(f"be{i}", [D], BF16, isOutput=False)
    out_dram = nc.declare_dram_parameter("out", [N, D], F32, isOutput=True)

    with tile.TileContext(nc) as tc, ExitStack() as ctx:
        consts = ctx.enter_context(tc.tile_pool(name="consts", bufs=1))
        persist = ctx.enter_context(tc.tile_pool(name="persist", bufs=1))
        small = ctx.enter_context(tc.tile_pool(name="small", bufs=12))
        stg = ctx.enter_context(tc.tile_pool(name="stg", bufs=3))
        # 2-bank 512-wide PSUM pool: open through B/C, reused in E via scope.
        ps512 = ctx.enter_context(
            tc.tile_pool(name="ps512", bufs=2, space=bass.MemorySpace.PSUM))

        # ---- constants -------------------------------------------------
        identb = consts.tile([P, P], BF16)
        make_identity(nc, identb)
        omib = consts.tile([P, P], BF16)  # 1 - I
        nc.gpsimd.memset(omib, 1.0)
        nc.gpsimd.affine_select(
            out=omib, in_=omib, compare_op=OP.not_equal, fill=0.0,
            base=0, pattern=[[-1, P]], channel_multiplier=1)
        # fp8 DoubleRow stationary: two slots of I/32 (moving is bias*16).
        id2 = consts.tile([P, 2, P], F8)
        nc.gpsimd.memset(id2, 0.0)
        for s in range(2):
            nc.gpsimd.affine_select(
                out=id2[:, s, :], in_=id2[:, s, :], compare_op=OP.not_equal,
                fill=1.0 / 32.0, base=0, pattern=[[-1, P]], channel_multiplier=1)
        nc.scalar.add_instruction(
            mybir.InstLoadActFuncSet(
                name=nc.get_next_instruction_name(), ins=[], outs=[],
                act_func_set_id=6))
        eps_t = consts.tile([P, 1], F32)
        nc.vector.memset(eps_t, EPS)
        onesb = consts.tile([1, P], BF16)
        nc.vector.memset(onesb, 1.0)
        onescol = consts.tile([P, 1], BF16)
        nc.vector.memset(onescol, 1.0)

        gb = {}
        beb = {}
        for i in (0, 2, 3):
            gb[i] = consts.tile([P, D], BF16, name=f"g{i}b", tag=f"g{i}b")
            _bcast_load(nc, gb[i], g_in[i][:])
            beb[i] = consts.tile([P, D], BF16, name=f"be{i}b", tag=f"be{i}b")
            _bcast_load(nc, beb[i], be_in[i][:])
        bvb = consts.tile([P, HD], F32)
        _bcast_load(nc, bvb, bv_in[:])
        bqr = consts.tile([P, NT], F32)
        nc.gpsimd.dma_start(out=bqr, in_=bqr_in[:, :])
        bkr = consts.tile([P, NT], F32)
        nc.gpsimd.dma_start(out=bkr, in_=bkr_in[:, :])
        wq_sb = consts.tile([P, HD], BF16)
        nc.gpsimd.dma_start(out=wq_sb, in_=wq_in[:, :])
        wk_sb = consts.tile([P, HD], BF16)
        nc.gpsimd.dma_start(out=wk_sb, in_=wk_in[:, :])
        wv_sb = consts.tile([P, HD], BF16)
        nc.gpsimd.dma_start(out=wv_sb, in_=wv_in[:, :])
        # W_O as [p=dv-within-head, h, d]
        wo_sb = consts.tile([P, HEADS, D], BF16)
        nc.sync.dma_start(
            out=wo_sb, in_=wo_in.rearrange("(hh p) d -> p hh d", p=P))
        bo_sb = consts.tile([1, D], BF16)
        nc.gpsimd.dma_start(out=bo_sb, in_=bo_in[:, :])
        w1_sb = consts.tile([P, D], BF16)
        nc.gpsimd.dma_start(out=w1_sb, in_=w1_in[:, :])
        w2_sb = consts.tile([P, D], BF16)
        nc.gpsimd.dma_start(out=w2_sb, in_=w2_in[:, :])
        b1_sb = consts.tile([P, 1], F32)
        nc.gpsimd.dma_start(out=b1_sb, in_=b1_in[:, :])
        b2_sb = consts.tile([P, 1], F32)
        nc.gpsimd.dma_start(out=b2_sb, in_=b2_in[:, :])

        # ---- persistent tensors ---------------------------------------
        h_sb = persist.tile([P, NT, D], BF16, tag="h")
        hn_t = persist.tile([P, NT, D], BF16, tag="hnt")
        ats = persist.tile([P, NT, N], BF16, tag="ats")  # A^T -> adj_norm^T
        for j in range(NT):
            nc.sync.dma_start(
                out=ats[:, j, :],
                in_=at_in[:, :].rearrange("(t p) n -> p t n", p=P)[:, j, :])
        nc.sync.dma_start(out=h_sb, in_=h_in.rearrange("(t p) d -> p t d", p=P))
        disb = persist.tile([P, N], BF16, tag="disb")
        dis_tok = persist.tile([P, NT], F32, tag="distok")
        adjHnT = persist.tile([P, N], BF16, tag="adjhnt")
        qT = persist.tile([P, HEADS, N], BF16, tag="qT")
        kT = persist.tile([P, HEADS, N], BF16, tag="kT")
        vna = persist.tile([P, NT, HEADS, DV + 1], BF16, tag="v")
        nc.vector.memset(vna[:, :, :, DV:DV + 1], 1.0)
        mhcT = persist.tile([P, HEADS, N], BF16, tag="mhcT")

        # ---- phase B part 1: diagonal fix + rowsums ---------------------
        with tc.tile_pool(name="psR", bufs=1, space=bass.MemorySpace.PSUM) as psR, \
             tc.tile_pool(name="psTb", bufs=2, space=bass.MemorySpace.PSUM) as psTb:
            rs_ps = [psR.tile([1, 512], F32, name=f"rsps{c}", tag=f"rsps{c}")
                     for c in range(2)]
            for j in range(NT):
                db = ats[:, j, j * P:(j + 1) * P]
                nc.vector.tensor_mul(out=db, in0=db, in1=omib)
                nc.vector.tensor_add(out=db, in0=db, in1=identb)
                for c in range(2):
                    nc.tensor.matmul(
                        rs_ps[c], onescol, ats[:, j, c * 512:(c + 1) * 512],
                        start=(j == 0), stop=(j == NT - 1))
            # -- phase A: H LayerNorm (interleaved) --
            for i in range(NT):
                s6 = small.tile([P, 6], F32, tag="s6")
                mv = small.tile([P, 2], F32, tag="mv")
                nc.vector.bn_stats(out=s6, in_=h_sb[:, i, :])
                nc.vector.bn_aggr(out=mv, in_=s6)
                lnv = small.tile([P, 1], F32, tag="lnv")
                nc.scalar.activation(out=lnv, in_=mv[:, 1:2], func=AF.Ln, bias=eps_t)
                rstd = small.tile([P, 1], F32, tag="rstd")
                nc.scalar.activation(out=rstd, in_=lnv, func=AF.Exp, scale=-0.5)
                hw = stg.tile([P, D], BF16, name="hw", tag="hw")
                nc.vector.tensor_scalar(
                    out=hw, in0=h_sb[:, i, :], scalar1=mv[:, 0:1], scalar2=rstd,
                    op0=OP.subtract, op1=OP.mult)
                nc.vector.tensor_mul(out=hw, in0=hw, in1=gb[0])
                nc.vector.tensor_add(out=hn_t[:, i, :], in0=hw, in1=beb[0])

            rs_sb = small.tile([1, N], F32, tag="rssb", bufs=1)
            for c in range(2):
                nc.vector.tensor_scalar_max(
                    out=rs_sb[:, c * 512:(c + 1) * 512], in0=rs_ps[c], scalar1=1.0)
            lnr = small.tile([1, N], F32, tag="lnr", bufs=1)
            nc.scalar.activation(out=lnr, in_=rs_sb, func=AF.Ln)
            disrow = small.tile([1, N], BF16, tag="disrow", bufs=1)
            nc.scalar.activation(out=disrow, in_=lnr, func=AF.Exp, scale=-0.5)
            # broadcast di over partitions: disb[q, n] = di_n
            for c in range(2):
                psd = ps512.tile([P, 512], F32, tag="ps512")
                nc.tensor.matmul(
                    psd, onesb, disrow[:, c * 512:(c + 1) * 512],
                    start=True, stop=True)
                nc.scalar.activation(
                    out=disb[:, c * 512:(c + 1) * 512], in_=psd, func=AF.Copy)
            # dis_tok[p, j] = di_{j*P+p} via PE transpose of disb chunks
            for j in range(NT):
                ptb = psTb.tile([P, P], BF16, tag="ptb")
                nc.tensor.transpose(ptb, disb[:, j * P:(j + 1) * P], identb)
                nc.scalar.activation(out=dis_tok[:, j:j + 1], in_=ptb[:, 0:1],
                                     func=AF.Copy)
            # fold di_m into hn_t (per-partition) instead of scaling ats
            for j in range(NT):
                nc.vector.tensor_scalar_mul(
                    out=hn_t[:, j, :], in0=hn_t[:, j, :],
                    scalar1=dis_tok[:, j:j + 1])

        # ---- phase C: shared GCN trunk ----------------------------------
        # adjHnT[d, n] = sum_m hn[m, d] * adjn^T[m, n]
        for c in range(2):
            psc = ps512.tile([P, 512], F32, tag="ps512")
            for j in range(NT):
                nc.tensor.matmul(
                    psc, hn_t[:, j, :], ats[:, j, c * 512:(c + 1) * 512],
                    start=(j == 0), stop=(j == NT - 1))
            nc.vector.tensor_mul(
                out=adjHnT[:, c * 512:(c + 1) * 512], in0=psc,
                in1=disb[:, c * 512:(c + 1) * 512])

        def emit_qk(hh, q_on_act=False):
            for c in range(2):
                psc = ps512.tile([P, 512], F32, tag="ps512")
                nc.tensor.matmul(
                    psc, wq_sb[:, hh * P:(hh + 1) * P],
                    adjHnT[:, c * 512:(c + 1) * 512], start=True, stop=True)
                if q_on_act:
                    nc.scalar.activation(
                        out=qT[:, hh, c * 512:(c + 1) * 512], in_=psc,
                        func=AF.Identity, bias=bqr[:, hh:hh + 1])
                else:
                    nc.vector.tensor_scalar_add(
                        out=qT[:, hh, c * 512:(c + 1) * 512], in0=psc,
                        scalar1=bqr[:, hh:hh + 1])
                psc = ps512.tile([P, 512], F32, tag="ps512")
                nc.tensor.matmul(
                    psc, wk_sb[:, hh * P:(hh + 1) * P],
                    adjHnT[:, c * 512:(c + 1) * 512], start=True, stop=True)
                nc.vector.tensor_scalar_add(
                    out=kT[:, hh, c * 512:(c + 1) * 512], in0=psc,
                    scalar1=bkr[:, hh:hh + 1])

        emit_qk(0, q_on_act=True)

        def emit_v(c, i0=0, i1=NT):
            for i in range(i0, i1):
                psc = ps512.tile([P, 512], F32, tag="ps512")
                nc.tensor.matmul(
                    psc, adjHnT[:, i * P:(i + 1) * P],
                    wv_sb[:, c * 512:(c + 1) * 512], start=True, stop=True)
                nc.vector.tensor_add(
                    out=vna[:, i, c * 4:(c + 1) * 4, 0:DV],
                    in0=psc.rearrange("p (a b) -> p a b", a=4),
                    in1=bvb[:, c * 512:(c + 1) * 512].rearrange(
                        "p (a b) -> p a b", a=4))

        emit_v(0)

        # ---- phase D: attention, software-pipelined over heads ----------
        with tc.tile_pool(name="psE", bufs=2, space=bass.MemorySpace.PSUM) as psE, \
             tc.tile_pool(name="psPM", bufs=2, space=bass.MemorySpace.PSUM) as psPM, \
             tc.tile_pool(name="etp", bufs=3) as etp, \
             tc.tile_pool(name="btp", bufs=2) as btp, \
             tc.tile_pool(name="mhp", bufs=3) as mhp:
            ets = {}
            mhs = {}
            mvss = {}
            vees = {}
            bt0 = btp.tile([P, NT, N], F8, tag="bt", name="bt0")
            nc.sync.dma_start(
                out=bt0, in_=bt_in[0].rearrange("(t p) n -> p t n", p=P))
            bts = {0: bt0}
            for stage in range(HEADS + 1):
                hh = stage
                if hh < HEADS:
                    et = etp.tile([P, NT, N], BF16, tag="et")
                    ets[hh] = et
                    btile = bts[hh]
                    for j in range(NT):
                        pse = psE.tile([P, N], F32, tag="pse")
                        for c in range(2):
                            nc.tensor.matmul(
                                pse[:, c * 512:(c + 1) * 512],
                                kT[:, hh, j * P:(j + 1) * P],
                                qT[:, hh, c * 512:(c + 1) * 512],
                                start=True, stop=False)
                            nc.tensor.matmul(
                                pse[:, c * 512:(c + 1) * 512],
                                id2, _dup2(btile[:, j, c * 512:(c + 1) * 512]),
                                start=False, stop=True, perf_mode=PM.DoubleRow)
                        nc.scalar.activation(out=et[:, j, :], in_=pse,
                                             func=AF.Exp)
                    if hh + 1 < HEADS:
                        btn = btp.tile([P, NT, N], F8, tag="bt",
                                       name=f"bt{hh + 1}")
                        nc.sync.dma_start(
                            out=btn,
                            in_=bt_in[hh + 1].rearrange("(t p) n -> p t n", p=P))
                        bts[hh + 1] = btn
                        if hh + 1 < HEADS:
                            emit_qk(hh + 1)
                if 1 <= stage <= 4:
                    emit_v(1, (stage - 1) * 2, stage * 2)
                if stage >= 1:
                    ph = stage - 1  # head whose PV/LN we process now
                    et = ets.pop(ph)
                    mh_sub = mhp.tile([P, NT, DV], BF16, tag="mh")
                    mhs[ph] = mh_sub
                    mvs = small.tile([P, NT, 2], F32, tag="mvs", bufs=2)
                    mvss[ph] = mvs
                    vee = small.tile([P, NT], F32, tag="vee", bufs=2)
                    vees[ph] = vee
                    for i in range(NT):
                        pm = psPM.tile([P, DV + 1], F32, tag="pm")
                        for j in range(NT):
                            nc.tensor.matmul(
                                pm, et[:, j, i * P:(i + 1) * P],
                                vna[:, j, ph, :],
                                start=(j == 0), stop=(j == NT - 1))
                        s6 = small.tile([P, 6], F32, tag="s6")
                        nc.vector.bn_stats(out=s6, in_=pm[:, 0:DV])
                        nc.vector.bn_aggr(out=mvs[:, i, :], in_=s6)
                        t = small.tile([P, 1], F32, tag="t")
                        nc.vector.tensor_scalar(
                            out=t, in0=pm[:, DV:DV + 1],
                            scalar1=pm[:, DV:DV + 1], scalar2=EPS,
                            op0=OP.mult, op1=OP.mult)
                        nc.gpsimd.tensor_add(
                            out=vee[:, i:i + 1], in0=t, in1=mvs[:, i, 1:2])
                        # mean-subtract now (frees pm); rstd scale later
                        nc.vector.tensor_scalar(
                            out=mh_sub[:, i, :], in0=pm[:, 0:DV],
                            scalar1=mvs[:, i, 0:1], scalar2=None,
                            op0=OP.subtract)
                    # batched rstd for the whole head (after next head's Exps
                    # in ACT program order -> no convoy)
                    lnv8 = small.tile([P, NT], F32, tag="lnv8", bufs=2)
                    nc.scalar.activation(out=lnv8, in_=vee, func=AF.Ln)
                    rstd8 = small.tile([P, NT], F32, tag="rstd8", bufs=2)
                    nc.scalar.activation(out=rstd8, in_=lnv8, func=AF.Exp,
                                         scale=-0.5)
                    eng = nc.vector if ph == HEADS - 1 else nc.gpsimd
                    for i in range(NT):
                        eng.tensor_scalar_mul(
                            out=mh_sub[:, i, :], in0=mh_sub[:, i, :],
                            scalar1=rstd8[:, i:i + 1])
                    for qt in range(4):
                        nc.sync.dma_start(
                            out=mhcT[:, ph, qt * 256:(qt + 1) * 256]
                            .rearrange("p (t f) -> p t f", t=2),
                            in_=mh_sub[:, qt * 2:(qt + 1) * 2, :],
                            transpose=True)

        # ---- phase E: output projection + MLP ---------------------------
        o_ln = persist.tile([P, NT, D], BF16, tag="oln")
        obe = persist.tile([P, NT, D], BF16, tag="obe")
        mvE = small.tile([P, NT, 2], F32, tag="mvE", bufs=1)
        rstdE = small.tile([P, NT], F32, tag="rstdE", bufs=1)
        with tc.tile_pool(name="psO", bufs=4, space=bass.MemorySpace.PSUM) as psO, \
             tc.tile_pool(name="psTe", bufs=2, space=bass.MemorySpace.PSUM) as psTe:
            oT = persist.tile([P, NT, P], BF16, tag="oT")
            for i in range(NT):
                pso = psO.tile([P, D], F32, tag="pso")
                for hh in range(HEADS):
                    nc.tensor.matmul(
                        pso, mhcT[:, hh, i * P:(i + 1) * P], wo_sb[:, hh, :],
                        start=(hh == 0), stop=False)
                nc.tensor.matmul(pso, onesb, bo_sb, start=False, stop=False)
                nc.tensor.matmul(pso, identb, h_sb[:, i, :],
                                 start=False, stop=True)
                s6 = small.tile([P, 6], F32, tag="s6")
                nc.vector.bn_stats(out=s6, in_=pso)
                nc.vector.bn_aggr(out=mvE[:, i, :], in_=s6)
                lnv = small.tile([P, 1], F32, tag="lnv")
                nc.scalar.activation(out=lnv, in_=mvE[:, i, 1:2], func=AF.Ln,
                                     bias=eps_t)
                nc.scalar.activation(out=rstdE[:, i:i + 1], in_=lnv,
                                     func=AF.Exp, scale=-0.5)
                nc.vector.tensor_scalar(
                    out=o_ln[:, i, :], in0=pso,
                    scalar1=mvE[:, i, 0:1], scalar2=rstdE[:, i:i + 1],
                    op0=OP.subtract, op1=OP.mult)
                ptb = psTe.tile([P, P], BF16, tag="pte")
                nc.tensor.transpose(ptb, o_ln[:, i, :], identb)
                nc.vector.tensor_copy(out=oT[:, i, :], in_=ptb)
                nc.gpsimd.tensor_mul(out=obe[:, i, :], in0=o_ln[:, i, :],
                                     in1=gb[2])
                nc.gpsimd.tensor_add(out=obe[:, i, :], in0=obe[:, i, :],
                                     in1=beb[2])
            r1T = persist.tile([P, N], BF16, tag="r1T")
            for c in range(2):
                psc = ps512.tile([P, 512], F32, tag="ps512")
                nc.tensor.matmul(
                    psc, w1_sb,
                    oT[:, 4 * c:4 * (c + 1), :].rearrange("p t f -> p (t f)"),
                    start=True, stop=True)
                nc.scalar.activation(
                    out=r1T[:, c * 512:(c + 1) * 512], in_=psc, func=AF.Relu,
                    bias=b1_sb)
            r2T = persist.tile([P, N], BF16, tag="r2T")
            for c in range(2):
                psc = ps512.tile([P, 512], F32, tag="ps512")
                nc.tensor.matmul(
                    psc, w2_sb, r1T[:, c * 512:(c + 1) * 512],
                    start=True, stop=True)
                nc.scalar.activation(
                    out=r2T[:, c * 512:(c + 1) * 512], in_=psc, func=AF.Relu,
                    bias=b2_sb)

            r2tok = persist.tile([P, NT, D], BF16, tag="r2tok")
            mvR = small.tile([P, NT, 2], F32, tag="mvR", bufs=1)
            rstdR = small.tile([P, NT], F32, tag="rstdR", bufs=1)
            out_sb = persist.tile([P, NT, D], F32, tag="osb")
            for i in range(NT):
                ptb = psTe.tile([P, P], BF16, tag="pte")
                nc.tensor.transpose(ptb, r2T[:, i * P:(i + 1) * P], identb)
                nc.scalar.activation(out=r2tok[:, i, :], in_=ptb, func=AF.Copy)
                s6 = small.tile([P, 6], F32, tag="s6")
                nc.vector.bn_stats(out=s6, in_=r2tok[:, i, :])
                nc.vector.bn_aggr(out=mvR[:, i, :], in_=s6)
                lnv = small.tile([P, 1], F32, tag="lnv")
                nc.scalar.activation(out=lnv, in_=mvR[:, i, 1:2], func=AF.Ln,
                                     bias=eps_t)
                nc.scalar.activation(out=rstdR[:, i:i + 1], in_=lnv,
                                     func=AF.Exp, scale=-0.5)
            for i in range(NT):
                ro = stg.tile([P, D], BF16, name="ro", tag="ro")
                nc.vector.tensor_scalar(
                    out=ro, in0=r2tok[:, i, :], scalar1=mvR[:, i, 0:1],
                    scalar2=rstdR[:, i:i + 1], op0=OP.subtract, op1=OP.mult)
                nc.vector.tensor_mul(out=ro, in0=ro, in1=gb[3])
                nc.vector.tensor_add(out=out_sb[:, i, :], in0=obe[:, i, :],
                                     in1=ro)
                if i % 4 == 3:
                    nc.sync.dma_start(
                        out=out_dram.rearrange(
                            "(t p) d -> p t d", p=P)[:, i - 3:i + 1, :],
                        in_=out_sb[:, i - 3:i + 1, :])

    nc.compile()
    return nc


def _get_program():
    if "nc" not in _prog_cache:
        _prog_cache["nc"] = _build_program()
    return _prog_cache["nc"]


def kernel(**inputs):
    nc = _get_program()
    f32 = np.float32
    bf16 = ml_dtypes.bfloat16
    f8 = ml_dtypes.float8_e4m3fn

    H = np.asarray(inputs["H"], dtype=f32)
    A = np.asarray(inputs["A"], dtype=f32)
    g1 = np.asarray(inputs["g1"], dtype=f32)
    be1 = np.asarray(inputs["be1"], dtype=f32)
    WO = np.asarray(inputs["W_O"], dtype=f32)
    # fold LN1 gain/bias into the output projection
    WO_fold = WO * np.tile(g1, HEADS)[:, None]
    bO = np.tile(be1, HEADS) @ WO

    BT = np.asarray(inputs["B_bias"], dtype=f32).transpose(0, 2, 1)
    base = {
        "bt": np.ascontiguousarray(BT * 16.0).astype(f8),
        "wq": (np.asarray(inputs["W_Q"], dtype=f32) * DK).astype(bf16),
        "wk": np.asarray(inputs["W_K"], dtype=f32).astype(bf16),
        "wv": np.asarray(inputs["W_V"], dtype=f32).astype(bf16),
        "bqr": np.ascontiguousarray(
            (np.asarray(inputs["b_Q"], dtype=f32) * DK).reshape(NT, P).T),
        "bkr": np.ascontiguousarray(
            np.asarray(inputs["b_K"], dtype=f32).reshape(NT, P).T),
        "bv": np.asarray(inputs["b_V"], dtype=f32),
        "wo": WO_fold.astype(bf16),
        "bo": bO.reshape(1, D).astype(bf16),
        "w1": (np.asarray(inputs["g2"], dtype=f32)[:, None]
               * np.asarray(inputs["W1"], dtype=f32)).astype(bf16),
        "w2": np.asarray(inputs["W2"], dtype=f32).astype(bf16),
        "b1": (np.asarray(inputs["b1"], dtype=f32)
               + np.asarray(inputs["be2"], dtype=f32)
               @ np.asarray(inputs["W1"], dtype=f32)).reshape(D, 1),
        "b2": np.asarray(inputs["b2"], dtype=f32).reshape(D, 1),
    }
    for i in (0, 2, 3):
        base[f"g{i}"] = np.asarray(inputs[f"g{i}"], dtype=f32).astype(bf16)
        base[f"be{i}"] = np.asarray(inputs[f"be{i}"], dtype=f32).astype(bf16)
    base["be2"] = (np.asarray(inputs["be2"], dtype=f32)
                   + np.asarray(inputs["be3"], dtype=f32)).astype(bf16)

    in_maps = []
    for c in range(B):
        m = dict(base)
        m["h"] = H[c].astype(bf16)
        m["at"] = np.ascontiguousarray(A[c].T).astype(bf16)
        in_maps.append(m)

    res = run_bass_kernel_spmd(nc, in_maps, list(range(B)))
    out = np.stack([res.results[c]["out"] for c in range(B)], axis=0)
    return out.astype(np.float32)


if __name__ == "__main__":
    nc = _get_program()
    print("program built ok")
    from concourse.timeline_sim import TimelineSim
    ns = TimelineSim(nc, trace=False).simulate()
    print(f"TimelineSim: {ns:.0f} ns")


# revision 52
# speedup vs baseline: 3.1180x; 1.0148x over previous
"""Trainium2 Bass kernel for nn_GRIC_31550829756424 (GCN-attention block).

Data-parallel over batch: 8 batches -> 8 NeuronCores, one full batch per core.

Key structure (v2):
- GCN reassociated: adjHnT = Hn^T @ adj_norm^T computed once (shared by
  Q/K/V), then Q/K/V are single-step K=128 matmuls.  A is host-transposed
  and bf16; both degree scalings fold into A^T via one scalar_tensor_tensor.
- All rsqrt computed as exp(-0.5*ln(x)) so the ONLY activation table used is
  natural_log_exp_and_others (Exp/Ln/Relu/Copy/Identity) -> 1 table load.
- Attention bias B added into the QK PSUM via an fp8 DoubleRow matmul
  (identity/32 stationary, bias*16 moving) instead of DVE adds.
- LN1 gain/bias folded into W_O on the host; MH transposed via DMA XBAR.

Self-contained: hardcodes all shapes; imports only the in-container concourse
stack.
"""

import sys

sys.path.insert(0, "/opt/trn_rl_repo")

import numpy as np
import ml_dtypes
from contextlib import ExitStack

import concourse.bass as bass
import concourse.tile as tile
from concourse import bacc
from concourse import mybir
from concourse.bass_utils import run_bass_kernel_spmd
from concourse.masks import make_identity

F32 = mybir.dt.float32
BF16 = mybir.dt.bfloat16
F8 = mybir.dt.float8e4
AF = mybir.ActivationFunctionType
OP = mybir.AluOpType
PM = mybir.MatmulPerfMode

B = 8
N = 1024
D = 128
HEADS = 8
DV = 128
HD = HEADS * DV  # 1024
P = 128
NT = N // P  # 8 tiles of 128 rows
DK = 1.0 / float(np.sqrt(np.float32(D)))
EPS = 1e-5

_prog_cache = {}


def _bcast_load(nc, dst, src):
    """DMA-load 1D DRAM vector src [W] replicated across all P partitions of
    dst [P, W] (same dtype)."""
    rep = bass.AP(tensor=src.tensor, offset=src.offset, ap=[[0, P]] + list(src.ap))
    nc.gpsimd.dma_start(out=dst, in_=rep)


def _dup2(ap):
    """View a [P, W] AP as [P, 2, W] with a stride-0 middle dim (DoubleRow
    moving operand reading the same data in both slots)."""
    return bass.AP(
        tensor=ap.tensor, offset=ap.offset,
        ap=[list(ap.ap[0]), [0, 2]] + [list(a) for a in ap.ap[1:]],
    )


def _build_program():
    nc = bacc.Bacc(None)

    h_in = nc.declare_dram_parameter("h", [N, D], BF16, isOutput=False)
    at_in = nc.declare_dram_parameter("at", [N, N], BF16, isOutput=False)
    bt_in = nc.declare_dram_parameter("bt", [HEADS, N, N], F8, isOutput=False)
    wq_in = nc.declare_dram_parameter("wq", [D, HD], BF16, isOutput=False)
    wk_in = nc.declare_dram_parameter("wk", [D, HD], BF16, isOutput=False)
    wv_in = nc.declare_dram_parameter("wv", [D, HD], BF16, isOutput=False)
    bqr_in = nc.declare_dram_parameter("bqr", [P, NT], F32, isOutput=False)
    bkr_in = nc.declare_dram_parameter("bkr", [P, NT], F32, isOutput=False)
    bv_in = nc.declare_dram_parameter("bv", [HD], F32, isOutput=False)
    wo_in = nc.declare_dram_parameter("wo", [HD, D], BF16, isOutput=False)
    bo_in = nc.declare_dram_parameter("bo", [1, D], BF16, isOutput=False)
    w1_in = nc.declare_dram_parameter("w1", [D, D], BF16, isOutput=False)
    w2_in = nc.declare_dram_parameter("w2", [D, D], BF16, isOutput=False)
    b1_in = nc.declare_dram_parameter("b1", [D, 1], F32, isOutput=False)
    b2_in = nc.declare_dram_parameter("b2", [D, 1], F32, isOutput=False)
    g_in = {}
    be_in = {}
    for i in (0, 2, 3):
        g_in[i] = nc.declare_dram_parameter(f"g{i}", [D], BF16, isOutput=False)
        be_in[i] = nc.declare_dram_parameter(f"be{i}", [D], BF16, isOutput=False)
    out_dram = nc.declare_dram_parameter("out", [N, D], F32, isOutput=True)

    with tile.TileContext(nc) as tc, ExitStack() as ctx:
        consts = ctx.enter_context(tc.tile_pool(name="consts", bufs=1))
        persist = ctx.enter_context(tc.tile_pool(name="persist", bufs=1))
        small = ctx.enter_context(tc.tile_pool(name="small", bufs=12))
        stg = ctx.enter_context(tc.tile_pool(name="stg", bufs=3))
        # 2-bank 512-wide PSUM pool: open through B/C, reused in E via scope.
        ps512 = ctx.enter_context(
            tc.tile_pool(name="ps512", bufs=2, space=bass.MemorySpace.PSUM))

        # ---- constants -------------------------------------------------
        identb = consts.tile([P, P], BF16)
        make_identity(nc, identb)
        omib = consts.tile([P, P], BF16)  # 1 - I
        nc.gpsimd.memset(omib, 1.0)
        nc.gpsimd.affine_select(
            out=omib, in_=omib, compare_op=OP.not_equal, fill=0.0,
            base=0, pattern=[[-1, P]], channel_multiplier=1)
        # fp8 DoubleRow stationary: two slots of I/32 (moving is bias*16).
        id2 = consts.tile([P, 2, P], F8)
        nc.gpsimd.memset(id2, 0.0)
        for s in range(2):
            nc.gpsimd.affine_select(
                out=id2[:, s, :], in_=id2[:, s, :], compare_op=OP.not_equal,
                fill=1.0 / 32.0, base=0, pattern=[[-1, P]], channel_multiplier=1)
        nc.scalar.add_instruction(
            mybir.InstLoadActFuncSet(
                name=nc.get_next_instruction_name(), ins=[], outs=[],
                act_func_set_id=6))
        eps_t = consts.tile([P, 1], F32)
        nc.vector.memset(eps_t, EPS)
        onesb = consts.tile([1, P], BF16)
        nc.vector.memset(onesb, 1.0)
        onescol = consts.tile([P, 1], BF16)
        nc.vector.memset(onescol, 1.0)

        gb = {}
        beb = {}
        for i in (0, 2, 3):
            gb[i] = consts.tile([P, D], BF16, name=f"g{i}b", tag=f"g{i}b")
            _bcast_load(nc, gb[i], g_in[i][:])
            beb[i] = consts.tile([P, D], BF16, name=f"be{i}b", tag=f"be{i}b")
            _bcast_load(nc, beb[i], be_in[i][:])
        bvb = consts.tile([P, HD], F32)
        _bcast_load(nc, bvb, bv_in[:])
        bqr = consts.tile([P, NT], F32)
        nc.gpsimd.dma_start(out=bqr, in_=bqr_in[:, :])
        bkr = consts.tile([P, NT], F32)
        nc.gpsimd.dma_start(out=bkr, in_=bkr_in[:, :])
        wq_sb = consts.tile([P, HD], BF16)
        nc.gpsimd.dma_start(out=wq_sb, in_=wq_in[:, :])
        wk_sb = consts.tile([P, HD], BF16)
        nc.gpsimd.dma_start(out=wk_sb, in_=wk_in[:, :])
        wv_sb = consts.tile([P, HD], BF16)
        nc.gpsimd.dma_start(out=wv_sb, in_=wv_in[:, :])
        # W_O as [p=dv-within-head, h, d]
        wo_sb = consts.tile([P, HEADS, D], BF16)
        nc.sync.dma_start(
            out=wo_sb, in_=wo_in.rearrange("(hh p) d -> p hh d", p=P))
        bo_sb = consts.tile([1, D], BF16)
        nc.gpsimd.dma_start(out=bo_sb, in_=bo_in[:, :])
        w1_sb = consts.tile([P, D], BF16)
        nc.gpsimd.dma_start(out=w1_sb, in_=w1_in[:, :])
        w2_sb = consts.tile([P, D], BF16)
        nc.gpsimd.dma_start(out=w2_sb, in_=w2_in[:, :])
        b1_sb = consts.tile([P, 1], F32)
        nc.gpsimd.dma_start(out=b1_sb, in_=b1_in[:, :])
        b2_sb = consts.tile([P, 1], F32)
        nc.gpsimd.dma_start(out=b2_sb, in_=b2_in[:, :])

        # ---- persistent tensors ---------------------------------------
        h_sb = persist.tile([P, NT, D], BF16, tag="h")
        hn_t = persist.tile([P, NT, D], BF16, tag="hnt")
        ats = persist.tile([P, NT, N], BF16, tag="ats")  # A^T -> adj_norm^T
        for j in range(NT):
            nc.sync.dma_start(
                out=ats[:, j, :],
                in_=at_in[:, :].rearrange("(t p) n -> p t n", p=P)[:, j, :])
        nc.sync.dma_start(out=h_sb, in_=h_in.rearrange("(t p) d -> p t d", p=P))
        disb = persist.tile([P, N], BF16, tag="disb")
        dis_tok = persist.tile([P, NT], F32, tag="distok")
        adjHnT = persist.tile([P, N], BF16, tag="adjhnt")
        qT = persist.tile([P, HEADS, N], BF16, tag="qT")
        kT = persist.tile([P, HEADS, N], BF16, tag="kT")
        vna = persist.tile([P, NT, HEADS, DV + 1], BF16, tag="v")
        nc.vector.memset(vna[:, :, :, DV:DV + 1], 1.0)
        mhcT = persist.tile([P, HEADS, N], BF16, tag="mhcT")

        # ---- phase B part 1: diagonal fix + rowsums ---------------------
        with tc.tile_pool(name="psR", bufs=1, space=bass.MemorySpace.PSUM) as psR, \
             tc.tile_pool(name="psTb", bufs=2, space=bass.MemorySpace.PSUM) as psTb:
            rs_ps = [psR.tile([1, 512], F32, name=f"rsps{c}", tag=f"rsps{c}")
                     for c in range(2)]
            for j in range(NT):
                db = ats[:, j, j * P:(j + 1) * P]
                nc.vector.tensor_mul(out=db, in0=db, in1=omib)
                nc.vector.tensor_add(out=db, in0=db, in1=identb)
                for c in range(2):
                    nc.tensor.matmul(
                        rs_ps[c], onescol, ats[:, j, c * 512:(c + 1) * 512],
                        start=(j == 0), stop=(j == NT - 1))
            # -- phase A: H LayerNorm (interleaved) --
            for i in range(NT):
                s6 = small.tile([P, 6], F32, tag="s6")
                mv = small.tile([P, 2], F32, tag="mv")
                nc.vector.bn_stats(out=s6, in_=h_sb[:, i, :])
                nc.vector.bn_aggr(out=mv, in_=s6)
                lnv = small.tile([P, 1], F32, tag="lnv")
                nc.scalar.activation(out=lnv, in_=mv[:, 1:2], func=AF.Ln, bias=eps_t)
                rstd = small.tile([P, 1], F32, tag="rstd")
                nc.scalar.activation(out=rstd, in_=lnv, func=AF.Exp, scale=-0.5)
                hw = stg.tile([P, D], BF16, name="hw", tag="hw")
                nc.vector.tensor_scalar(
                    out=hw, in0=h_sb[:, i, :], scalar1=mv[:, 0:1], scalar2=rstd,
                    op0=OP.subtract, op1=OP.mult)
                nc.vector.tensor_mul(out=hw, in0=hw, in1=gb[0])
                nc.vector.tensor_add(out=hn_t[:, i, :], in0=hw, in1=beb[0])

            rs_sb = small.tile([1, N], F32, tag="rssb", bufs=1)
            for c in range(2):
                nc.vector.tensor_scalar_max(
                    out=rs_sb[:, c * 512:(c + 1) * 512], in0=rs_ps[c], scalar1=1.0)
            lnr = small.tile([1, N], F32, tag="lnr", bufs=1)
            nc.scalar.activation(out=lnr, in_=rs_sb, func=AF.Ln)
            disrow = small.tile([1, N], BF16, tag="disrow", bufs=1)
            nc.scalar.activation(out=disrow, in_=lnr, func=AF.Exp, scale=-0.5)
            # broadcast di over partitions: disb[q, n] = di_n
            for c in range(2):
                psd = ps512.tile([P, 512], F32, tag="ps512")
                nc.tensor.matmul(
                    psd, onesb, disrow[:, c * 512:(c + 1) * 512],
                    start=True, stop=True)
                nc.scalar.activation(
                    out=disb[:, c * 512:(c + 1) * 512], in_=psd, func=AF.Copy)
            # dis_tok[p, j] = di_{j*P+p} via PE transpose of disb chunks
            for j in range(NT):
                ptb = psTb.tile([P, P], BF16, tag="ptb")
                nc.tensor.transpose(ptb, disb[:, j * P:(j + 1) * P], identb)
                nc.scalar.activation(out=dis_tok[:, j:j + 1], in_=ptb[:, 0:1],
                                     func=AF.Copy)
            # fold di_m into hn_t (per-partition) instead of scaling ats
            for j in range(NT):
                nc.vector.tensor_scalar_mul(
                    out=hn_t[:, j, :], in0=hn_t[:, j, :],
                    scalar1=dis_tok[:, j:j + 1])

        # ---- phase C: shared GCN trunk ----------------------------------
        # adjHnT[d, n] = sum_m hn[m, d] * adjn^T[m, n]
        for c in range(2):
            psc = ps512.tile([P, 512], F32, tag="ps512")
            for j in range(NT):
                nc.tensor.matmul(
                    psc, hn_t[:, j, :], ats[:, j, c * 512:(c + 1) * 512],
                    start=(j == 0), stop=(j == NT - 1))
            nc.vector.tensor_mul(
                out=adjHnT[:, c * 512:(c + 1) * 512], in0=psc,
                in1=disb[:, c * 512:(c + 1) * 512])

        def emit_qk(hh, q_on_act=False):
            for c in range(2):
                psc = ps512.tile([P, 512], F32, tag="ps512")
                nc.tensor.matmul(
                    psc, wq_sb[:, hh * P:(hh + 1) * P],
                    adjHnT[:, c * 512:(c + 1) * 512], start=True, stop=True)
                if q_on_act:
                    nc.scalar.activation(
                        out=qT[:, hh, c * 512:(c + 1) * 512], in_=psc,
                        func=AF.Identity, bias=bqr[:, hh:hh + 1])
                else:
                    nc.vector.tensor_scalar_add(
                        out=qT[:, hh, c * 512:(c + 1) * 512], in0=psc,
                        scalar1=bqr[:, hh:hh + 1])
                psc = ps512.tile([P, 512], F32, tag="ps512")
                nc.tensor.matmul(
                    psc, wk_sb[:, hh * P:(hh + 1) * P],
                    adjHnT[:, c * 512:(c + 1) * 512], start=True, stop=True)
                nc.vector.tensor_scalar_add(
                    out=kT[:, hh, c * 512:(c + 1) * 512], in0=psc,
                    scalar1=bkr[:, hh:hh + 1])

        emit_qk(0, q_on_act=True)

        def emit_v(c, i0=0, i1=NT):
            for i in range(i0, i1):
                psc = ps512.tile([P, 512], F32, tag="ps512")
                nc.tensor.matmul(
                    psc, adjHnT[:, i * P:(i + 1) * P],
                    wv_sb[:, c * 512:(c + 1) * 512], start=True, stop=True)
                nc.vector.tensor_add(
                    out=vna[:, i, c * 4:(c + 1) * 4, 0:DV],
                    in0=psc.rearrange("p (a b) -> p a b", a=4),
                    in1=bvb[:, c * 512:(c + 1) * 512].rearrange(
                        "p (a b) -> p a b", a=4))

        emit_v(0)

        # ---- phase D: attention, software-pipelined over heads ----------
        with tc.tile_pool(name="psE", bufs=2, space=bass.MemorySpace.PSUM) as psE, \
             tc.tile_pool(name="psPM", bufs=2, space=bass.MemorySpace.PSUM) as psPM, \
             tc.tile_pool(name="etp", bufs=3) as etp, \
             tc.tile_pool(name="btp", bufs=2) as btp, \
             tc.tile_pool(name="mhp", bufs=3) as mhp:
            ets = {}
            mhs = {}
            mvss = {}
            vees = {}
            bt0 = btp.tile([P, NT, N], F8, tag="bt", name="bt0")
            nc.sync.dma_start(
                out=bt0, in_=bt_in[0].rearrange("(t p) n -> p t n", p=P))
            bts = {0: bt0}
            for stage in range(HEADS + 1):
                hh = stage
                if hh < HEADS:
                    et = etp.tile([P, NT, N], BF16, tag="et")
                    ets[hh] = et
                    btile = bts[hh]
                    for j in range(NT):
                        pse = psE.tile([P, N], F32, tag="pse")
                        for c in range(2):
                            nc.tensor.matmul(
                                pse[:, c * 512:(c + 1) * 512],
                                kT[:, hh, j * P:(j + 1) * P],
                                qT[:, hh, c * 512:(c + 1) * 512],
                                start=True, stop=False)
                            nc.tensor.matmul(
                                pse[:, c * 512:(c + 1) * 512],
                                id2, _dup2(btile[:, j, c * 512:(c + 1) * 512]),
                                start=False, stop=True, perf_mode=PM.DoubleRow)
                        nc.scalar.activation(out=et[:, j, :], in_=pse,
                                             func=AF.Exp)
                    if hh + 1 < HEADS:
                        btn = btp.tile([P, NT, N], F8, tag="bt",
                                       name=f"bt{hh + 1}")
                        nc.sync.dma_start(
                            out=btn,
                            in_=bt_in[hh + 1].rearrange("(t p) n -> p t n", p=P))
                        bts[hh + 1] = btn
                        if hh + 1 < HEADS:
                            emit_qk(hh + 1)
                if 1 <= stage <= 4:
                    emit_v(1, (stage - 1) * 2, stage * 2)
                if stage >= 1:
                    ph = stage - 1  # head whose PV/LN we process now
                    et = ets.pop(ph)
                    mh_sub = mhp.tile([P, NT, DV], BF16, tag="mh")
                    mhs[ph] = mh_sub
                    mvs = small.tile([P, NT, 2], F32, tag="mvs", bufs=2)
                    mvss[ph] = mvs
                    vee = small.tile([P, NT], F32, tag="vee", bufs=2)
                    vees[ph] = vee
                    for i in range(NT):
                        pm = psPM.tile([P, DV + 1], F32, tag="pm")
                        for j in range(NT):
                            nc.tensor.matmul(
                                pm, et[:, j, i * P:(i + 1) * P],
                                vna[:, j, ph, :],
                                start=(j == 0), stop=(j == NT - 1))
                        s6 = small.tile([P, 6], F32, tag="s6")
                        nc.vector.bn_stats(out=s6, in_=pm[:, 0:DV])
                        nc.vector.bn_aggr(out=mvs[:, i, :], in_=s6)
                        t = small.tile([P, 1], F32, tag="t")
                        nc.vector.tensor_scalar(
                            out=t, in0=pm[:, DV:DV + 1],
                            scalar1=pm[:, DV:DV + 1], scalar2=EPS,
                            op0=OP.mult, op1=OP.mult)
                        nc.gpsimd.tensor_add(
                            out=vee[:, i:i + 1], in0=t, in1=mvs[:, i, 1:2])
                        # mean-subtract now (frees pm); rstd scale later
                        nc.vector.tensor_scalar(
                            out=mh_sub[:, i, :], in0=pm[:, 0:DV],
                            scalar1=mvs[:, i, 0:1], scalar2=None,
                            op0=OP.subtract)
                    # rstd batched per half-head: first scales/transposes
                    # start before the second half's PV stats finish
                    lnv8 = small.tile([P, NT], F32, tag="lnv8", bufs=2)
                    rstd8 = small.tile([P, NT], F32, tag="rstd8", bufs=2)
                    eng = nc.vector if ph == HEADS - 1 else nc.gpsimd
                    for half in range(2):
                        sl = slice(half * 4, (half + 1) * 4)
                        nc.scalar.activation(out=lnv8[:, sl], in_=vee[:, sl],
                                             func=AF.Ln)
                        nc.scalar.activation(out=rstd8[:, sl], in_=lnv8[:, sl],
                                             func=AF.Exp, scale=-0.5)
                        for i in range(half * 4, (half + 1) * 4):
                            eng.tensor_scalar_mul(
                                out=mh_sub[:, i, :], in0=mh_sub[:, i, :],
                                scalar1=rstd8[:, i:i + 1])
                        for qt in range(half * 2, (half + 1) * 2):
                            nc.sync.dma_start(
                                out=mhcT[:, ph, qt * 256:(qt + 1) * 256]
                                .rearrange("p (t f) -> p t f", t=2),
                                in_=mh_sub[:, qt * 2:(qt + 1) * 2, :],
                                transpose=True)

        # ---- phase E: output projection + MLP ---------------------------
        o_ln = persist.tile([P, NT, D], BF16, tag="oln")
        obe = persist.tile([P, NT, D], BF16, tag="obe")
        mvE = small.tile([P, NT, 2], F32, tag="mvE", bufs=1)
        rstdE = small.tile([P, NT], F32, tag="rstdE", bufs=1)
        with tc.tile_pool(name="psO", bufs=4, space=bass.MemorySpace.PSUM) as psO, \
             tc.tile_pool(name="psTe", bufs=2, space=bass.MemorySpace.PSUM) as psTe:
            oT = persist.tile([P, NT, P], BF16, tag="oT")
            for i in range(NT):
                pso = psO.tile([P, D], F32, tag="pso")
                for hh in range(HEADS):
                    nc.tensor.matmul(
                        pso, mhcT[:, hh, i * P:(i + 1) * P], wo_sb[:, hh, :],
                        start=(hh == 0), stop=False)
                nc.tensor.matmul(pso, onesb, bo_sb, start=False, stop=False)
                nc.tensor.matmul(pso, identb, h_sb[:, i, :],
                                 start=False, stop=True)
                s6 = small.tile([P, 6], F32, tag="s6")
                nc.vector.bn_stats(out=s6, in_=pso)
                nc.vector.bn_aggr(out=mvE[:, i, :], in_=s6)
                lnv = small.tile([P, 1], F32, tag="lnv")
                nc.scalar.activation(out=lnv, in_=mvE[:, i, 1:2], func=AF.Ln,
                                     bias=eps_t)
                nc.scalar.activation(out=rstdE[:, i:i + 1], in_=lnv,
                                     func=AF.Exp, scale=-0.5)
                nc.vector.tensor_scalar(
                    out=o_ln[:, i, :], in0=pso,
                    scalar1=mvE[:, i, 0:1], scalar2=rstdE[:, i:i + 1],
                    op0=OP.subtract, op1=OP.mult)
                ptb = psTe.tile([P, P], BF16, tag="pte")
                nc.tensor.transpose(ptb, o_ln[:, i, :], identb)
                nc.vector.tensor_copy(out=oT[:, i, :], in_=ptb)
                nc.gpsimd.tensor_mul(out=obe[:, i, :], in0=o_ln[:, i, :],
                                     in1=gb[2])
                nc.gpsimd.tensor_add(out=obe[:, i, :], in0=obe[:, i, :],
                                     in1=beb[2])
            r1T = persist.tile([P, N], BF16, tag="r1T")
            for c in range(2):
                psc = ps512.tile([P, 512], F32, tag="ps512")
                nc.tensor.matmul(
                    psc, w1_sb,
                    oT[:, 4 * c:4 * (c + 1), :].rearrange("p t f -> p (t f)"),
                    start=True, stop=True)
                nc.scalar.activation(
                    out=r1T[:, c * 512:(c + 1) * 512], in_=psc, func=AF.Relu,
                    bias=b1_sb)
            r2T = persist.tile([P, N], BF16, tag="r2T")
            for c in range(2):
                psc = ps512.tile([P, 512], F32, tag="ps512")
                nc.tensor.matmul(
                    psc, w2_sb, r1T[:, c * 512:(c + 1) * 512],
                    start=True, stop=True)
                nc.scalar.activation(
                    out=r2T[:, c * 512:(c + 1) * 512], in_=psc, func=AF.Relu,
                    bias=b2_sb)

            r2tok = persist.tile([P, NT, D], BF16, tag="r2tok")
            mvR = small.tile([P, NT, 2], F32, tag="mvR", bufs=1)
            rstdR = small.tile([P, NT], F32, tag="rstdR", bufs=1)
            out_sb = persist.tile([P, NT, D], F32, tag="osb")
            for i in range(NT):
                ptb = psTe.tile([P, P], BF16, tag="pte")
                nc.tensor.transpose(ptb, r2T[:, i * P:(i + 1) * P], identb)
                nc.scalar.activation(out=r2tok[:, i, :], in_=ptb, func=AF.Copy)
                s6 = small.tile([P, 6], F32, tag="s6")
                nc.vector.bn_stats(out=s6, in_=r2tok[:, i, :])
                nc.vector.bn_aggr(out=mvR[:, i, :], in_=s6)
                lnv = small.tile([P, 1], F32, tag="lnv")
                nc.scalar.activation(out=lnv, in_=mvR[:, i, 1:2], func=AF.Ln,
                                     bias=eps_t)
                nc.scalar.activation(out=rstdR[:, i:i + 1], in_=lnv,
                                     func=AF.Exp, scale=-0.5)
            for i in range(NT):
                ro = stg.tile([P, D], BF16, name="ro", tag="ro")
                nc.vector.tensor_scalar(
                    out=ro, in0=r2tok[:, i, :], scalar1=mvR[:, i, 0:1],
                    scalar2=rstdR[:, i:i + 1], op0=OP.subtract, op1=OP.mult)
                nc.vector.tensor_mul(out=ro, in0=ro, in1=gb[3])
                nc.vector.tensor_add(out=out_sb[:, i, :], in0=obe[:, i, :],
                                     in1=ro)
                if i % 4 == 3:
                    nc.sync.dma_start(
                        out=out_dram.rearrange(
                            "(t p) d -> p t d", p=P)[:, i - 3:i + 1, :],
                        in_=out_sb[:, i - 3:i + 1, :])

    nc.compile()
    return nc


def _get_program():
    if "nc" not in _prog_cache:
        _prog_cache["nc"] = _build_program()
    return _prog_cache["nc"]


def kernel(**inputs):
    nc = _get_program()
    f32 = np.float32
    bf16 = ml_dtypes.bfloat16
    f8 = ml_dtypes.float8_e4m3fn

    H = np.asarray(inputs["H"], dtype=f32)
    A = np.asarray(inputs["A"], dtype=f32)
    g1 = np.asarray(inputs["g1"], dtype=f32)
    be1 = np.asarray(inputs["be1"], dtype=f32)
    WO = np.asarray(inputs["W_O"], dtype=f32)
    # fold LN1 gain/bias into the output projection
    WO_fold = WO * np.tile(g1, HEADS)[:, None]
    bO = np.tile(be1, HEADS) @ WO

    BT = np.asarray(inputs["B_bias"], dtype=f32).transpose(0, 2, 1)
    base = {
        "bt": np.ascontiguousarray(BT * 16.0).astype(f8),
        "wq": (np.asarray(inputs["W_Q"], dtype=f32) * DK).astype(bf16),
        "wk": np.asarray(inputs["W_K"], dtype=f32).astype(bf16),
        "wv": np.asarray(inputs["W_V"], dtype=f32).astype(bf16),
        "bqr": np.ascontiguousarray(
            (np.asarray(inputs["b_Q"], dtype=f32) * DK).reshape(NT, P).T),
        "bkr": np.ascontiguousarray(
            np.asarray(inputs["b_K"], dtype=f32).reshape(NT, P).T),
        "bv": np.asarray(inputs["b_V"], dtype=f32),
        "wo": WO_fold.astype(bf16),
        "bo": bO.reshape(1, D).astype(bf16),
        "w1": (np.asarray(inputs["g2"], dtype=f32)[:, None]
               * np.asarray(inputs["W1"], dtype=f32)).astype(bf16),
        "w2": np.asarray(inputs["W2"], dtype=f32).astype(bf16),
        "b1": (np.asarray(inputs["b1"], dtype=f32)
               + np.asarray(inputs["be2"], dtype=f32)
               @ np.asarray(inputs["W1"], dtype=f32)).reshape(D, 1),
        "b2": np.asarray(inputs["b2"], dtype=f32).reshape(D, 1),
    }
    for i in (0, 2, 3):
        base[f"g{i}"] = np.asarray(inputs[f"g{i}"], dtype=f32).astype(bf16)
        base[f"be{i}"] = np.asarray(inputs[f"be{i}"], dtype=f32).astype(bf16)
    base["be2"] = (np.asarray(inputs["be2"], dtype=f32)
                   + np.asarray(inputs["be3"], dtype=f32)).astype(bf16)

    in_maps = []
    for c in range(B):
        m = dict(base)
        m["h"] = H[c].astype(bf16)
        m["at"] = np.ascontiguousarray(A[c].T).astype(bf16)
        in_maps.append(m)

    res = run_bass_kernel_spmd(nc, in_maps, list(range(B)))
    out = np.stack([res.results[c]["out"] for c in range(B)], axis=0)
    return out.astype(np.float32)


if __name__ == "__main__":
    nc = _get_program()
    print("program built ok")
    from concourse.timeline_sim import TimelineSim
    ns = TimelineSim(nc, trace=False).simulate()
    print(f"TimelineSim: {ns:.0f} ns")
